# revision 11
# baseline (speedup 1.0000x reference)
"""Trainium2 Bass kernel for nn_Attention (dense transformer attention block).

Full inputs -> full output. Internally: 8 NeuronCores, 2 data-parallel groups
(batch) x 4-way tensor-parallel (heads). Each core computes 8 heads for one
batch element. The wo projection is redistributed with a single 8-rank
AllToAll per 512-token slice: each core ships its heads' attention output
(oT, feature-major) for token-quarter q to ranks q and q+4, and afterwards
holds the FULL 4096-feature oT for one 128-token quarter of each batch --
it then computes y for those rows over one 2048-wide d-half (group 0 takes
d 0:2048, group 1 d 2048:4096). No ReduceScatter; the A2A moves 2 MB/rank
at ~24 us (mesh) and the sc1 A2A hides under the sc0 wo pass.

Compute in bf16 on the TensorEngine (fp32 matmul is 4x slower), fp32 PSUM
accumulation. All operand layouts are pre-rearranged host-side so every
device DMA is a contiguous per-partition block:
  - projections:  qT/kT = (w-tile).T @ xhatT-tile   -> [feature, seq] layout
  - scores:       scoresT[t, s] = kT-tile.T @ qT    (softmax along partitions)
  - Z:            ones[128,128].T @ probs           -> Z broadcast to all rows
  - PV:           oT[dh, s] = v-tile.T @ probsT
  - wo:           y[t, d] = oT-recv-tile.T @ woT-slab (K=4096 in one psum)
Causal structure is exploited: score tiles that are fully masked are skipped
(scores/exp/Z/PV), and only the 4 diagonal-band tiles per query chunk get a
post-exp 0/1 multiply (from 4 precomputed [128,512] masks). RoPE pairs are
deinterleaved host-side (even dims first); the 1/sqrt(128) score scale is
folded into the q-side cos/sin tables.
"""

import sys

import numpy as np

for _p in ("/opt/trn_rl_repo",):
    if _p not in sys.path:
        sys.path.insert(0, _p)

import ml_dtypes

BF16 = ml_dtypes.bfloat16

D = 4096      # model dim
S = 1024      # decoder sequence length
E = 512       # encoder length
T = E + S     # total key length
H = 8         # heads per core (32 total / 4-way TP)
DH = 128      # head dim
O = H * DH    # per-core projection width = 1024
NDT = D // 128
NEG = -1e9
A2A_GROUP = [[0, 1, 2, 3, 4, 5, 6, 7]]
DHALF = 2048  # per-core output d-half width

_CACHE = {}
LAST_EXEC_NS = None


def _build(no_collective=False):
    import concourse.mybir as mybir
    import concourse.tile as tile
    from concourse import bacc

    bf16 = mybir.dt.bfloat16

    nc = bacc.Bacc(
        "TRN2",
        target_bir_lowering=False,
        debug=False,
        num_devices=8,
    )

    P = {}
    for name, shape in [
        ("x_r", [128, NDT * T]),        # xhatT slabs: cols dt*T + t
        ("wq_r", [128, NDT * O]),       # Q pass slabs: cols p*8192 + n*256 + c
        ("wk_r", [128, NDT * O]),       # K pass slabs: same geometry
        ("wv_r", [128, NDT * O]),       # V slabs: cols oc*16384 + n*512 + c
        ("wo_r", [128, 2 * 32 * 1024]), # woT slabs: cols p*32768 + kt*1024 + d
        ("csq_cos", [128, S]),
        ("csq_sin", [128, S]),
        ("csk_cos", [128, S]),
        ("csk_sin", [128, S]),
        ("dmask", [128, 4 * 512]),      # 4 diagonal-band masks
        ("ones", [128, 128]),
    ]:
        P[name] = nc.declare_dram_parameter(name, shape, bf16, isOutput=False)
    # rows: sc*256 + b*128 + t ; cols: d within this core's d-half
    out = nc.declare_dram_parameter("out", [512, DHALF], bf16, isOutput=True)

    with tile.TileContext(nc) as tc:
        _emit(nc, tc, P, out, no_collective=no_collective)
    nc.compile()
    return nc


def _emit(nc, tc, P, out, no_collective=False):
    import concourse.mybir as mybir
    from concourse.bass import ds

    bf16 = mybir.dt.bfloat16
    fp32 = mybir.dt.float32
    AF = mybir.ActivationFunctionType

    with tc.tile_pool(name="res", bufs=1) as res, \
         tc.tile_pool(name="dram", bufs=1, space="DRAM") as dram:
        onesb = res.tile([128, 128], bf16, tag="onesb")
        dmsk = res.tile([128, 4 * 512], bf16, tag="dmsk")  # 0/1 keep masks

        # A2A staging: rows j*128 + dh (j = dest rank), cols h*128 + t
        a2a_in = [
            dram.tile([1024, 1024], bf16, tag=f"ai{sc}", name=f"a2a_in{sc}")
            for sc in range(2)
        ]
        a2a_out = [
            dram.tile([1024, 1024], bf16, tag=f"ao{sc}", name=f"a2a_out{sc}")
            for sc in range(2)
        ]

        nc.gpsimd.dma_start(out=dmsk[:, :], in_=P["dmask"][:, :])
        nc.gpsimd.dma_start(out=onesb[:, :], in_=P["ones"][:, :])

        with tc.tile_pool(name="qkv", bufs=1) as qkv:
          qT = qkv.tile([128, H * S], bf16, tag="qT")     # cols h*S + s
          kT = qkv.tile([128, H * T], bf16, tag="kT")     # cols h*T + t
          vsb = qkv.tile([128, 12 * O], bf16, tag="vsb")  # cols tt*O + o

          # ---------------- phase 1: projections + rope ----------------
          with tc.tile_pool(name="xpool", bufs=1) as xpool, \
               tc.tile_pool(name="tabpool", bufs=1) as tabpool, \
               tc.tile_pool(name="wpool", bufs=3) as wpool, \
               tc.tile_pool(name="rtmp", bufs=2) as rtmp, \
               tc.tile_pool(name="ps1", bufs=8, space="PSUM") as ps1:
            # first Q weight half-slab ahead of everything on the sync queue
            wslab_q = [0]

            def wslab(src, off, n, name):
                wr = wpool.tile([128, n], bf16, tag="wr", name=name)
                eng = nc.sync if wslab_q[0] % 2 == 0 else nc.scalar
                wslab_q[0] += 1
                eng.dma_start(out=wr[:, :], in_=P[src][:, ds(off, n)])
                return wr

            wr_q00 = wslab("wq_r", 0, 16 * 256, "wr_q0_0")
            xh = []
            for dt in range(NDT):
                xt = xpool.tile([128, T], bf16, tag=f"xh{dt}", name=f"xh{dt}")
                (nc.scalar if dt % 2 == 0 else nc.gpsimd).dma_start(
                    out=xt[:, :], in_=P["x_r"][:, ds(dt * T, T)]
                )
                xh.append(xt)
            csqc = tabpool.tile([128, S], bf16, tag="csqc")
            csqs = tabpool.tile([128, S], bf16, tag="csqs")
            cskc = tabpool.tile([128, S], bf16, tag="cskc")
            csks = tabpool.tile([128, S], bf16, tag="csks")
            nc.scalar.dma_start(out=csqc[:, :], in_=P["csq_cos"][:, :])
            nc.scalar.dma_start(out=csqs[:, :], in_=P["csq_sin"][:, :])
            nc.gpsimd.dma_start(out=cskc[:, :], in_=P["csk_cos"][:, :])
            nc.gpsimd.dma_start(out=csks[:, :], in_=P["csk_sin"][:, :])

            # RoPE: tables are full-height with the 64-row block duplicated
            # (cos) or sign-split (-sin; +sin), so every TensorTensor is
            # partition-aligned. The half-swap goes through an SBUF-SBUF DMA.
            def rope(buf, base, cos, sin, tag):
                swp = rtmp.tile([128, S], bf16, tag="swp", name=f"swp_{tag}")
                nc.gpsimd.dma_start(
                    out=swp[ds(0, 64), :], in_=buf[ds(64, 64), ds(base, S)]
                )
                nc.gpsimd.dma_start(
                    out=swp[ds(64, 64), :], in_=buf[ds(0, 64), ds(base, S)]
                )
                nc.vector.tensor_mul(swp[:, :], swp[:, :], sin[:, :])
                nc.vector.tensor_mul(
                    buf[:, ds(base, S)], buf[:, ds(base, S)], cos[:, :]
                )
                nc.vector.tensor_add(
                    buf[:, ds(base, S)], buf[:, ds(base, S)], swp[:, :]
                )

            # Q: 4 passes x (2 o_tiles x 2 s_chunks); K: 4 passes x
            # (2 o_tiles x 3 t_chunks). Weight slabs stream in 16-dt halves.
            for src, nch, xoff, obuf, ostride in (
                ("wq_r", 2, E, qT, S),
                ("wk_r", 3, 0, kT, T),
            ):
                for p in range(4):
                    ps = [
                        [ps1.tile([128, 512], fp32, tag="ps1",
                                  name=f"ps_{src}_{p}_{oi}_{cc}")
                         for cc in range(nch)]
                        for oi in range(2)
                    ]
                    for half in range(2):
                        if src == "wq_r" and p == 0 and half == 0:
                            wr = wr_q00
                        else:
                            wr = wslab(
                                src, (p * 2 + half) * 16 * 256, 16 * 256,
                                f"wr_{src}_{p}_{half}",
                            )
                        for dtl in range(16):
                            dt = half * 16 + dtl
                            for oi in range(2):
                                for cc in range(nch):
                                    nc.tensor.matmul(
                                        ps[oi][cc][:, :],
                                        wr[:, ds(dtl * 256 + oi * 128, 128)],
                                        xh[dt][:, ds(xoff + cc * 512, 512)],
                                        start=(dt == 0),
                                        stop=(dt == NDT - 1),
                                    )
                    for oi in range(2):
                        h = 2 * p + oi
                        for cc in range(nch):
                            nc.scalar.copy(
                                obuf[:, ds(h * ostride + cc * 512, 512)],
                                ps[oi][cc][:, :],
                            )
                        if src == "wq_r":
                            rope(qT, h * S, csqc, csqs, f"q{h}")
                        else:
                            rope(kT, h * T + E, cskc, csks, f"k{h}")

            # V (x-stationary): 2 o_chunks x 2 t_groups of 6 tiles; weight
            # slabs re-streamed per t_group in two 16-dt halves
            for oc in range(2):
                for tg in range(2):
                    tb = tg * 6
                    psv = [ps1.tile([128, 512], fp32, tag="ps1",
                                    name=f"psv_{oc}_{tg}_{ti}")
                           for ti in range(6)]
                    for qr in range(4):
                        wr = wpool.tile(
                            [128, 8 * 512], bf16, tag="wr",
                            name=f"wr_v{oc}_{tg}_{qr}",
                        )
                        nc.sync.dma_start(
                            out=wr[:, :],
                            in_=P["wv_r"][
                                :, ds(oc * NDT * 512 + qr * 8 * 512, 8 * 512)
                            ],
                        )
                        for dtl in range(8):
                            dt = qr * 8 + dtl
                            for ti in range(6):
                                nc.tensor.matmul(
                                    psv[ti][:, :],
                                    xh[dt][:, ds((tb + ti) * 128, 128)],
                                    wr[:, ds(dtl * 512, 512)],
                                    start=(dt == 0),
                                    stop=(dt == NDT - 1),
                                )
                    for ti in range(6):
                        nc.scalar.copy(
                            vsb[:, ds((tb + ti) * O + oc * 512, 512)],
                            psv[ti][:, :],
                        )

          # wo pass-1 slabs + sc0 recv tiles prefetch during attention; this
          # pool sits in the (dead) phase-1 x region so its DMAs only wait on
          # the last projection matmul, not on attention.
          with tc.tile_pool(name="wpre", bufs=1) as wpre:
            slab1 = [
                wpre.tile([128, 1024], bf16, tag=f"sl1_{kt}", name=f"slab1_{kt}")
                for kt in range(32)
            ]
            rt0 = [
                wpre.tile([128, 1024], bf16, tag=f"rt0_{i}", name=f"rt0_{i}")
                for i in range(8)
            ]
            # stream pass-1 slabs during attention, all on gpsimd: its only
            # later work is the A2A triggers, so ring-credit waits here
            # never stall a compute-feeding queue (scalar's exp stream
            # stalled ~18us when half of these sat on it)
            for kt in range(32):
                nc.gpsimd.dma_start(
                    out=slab1[kt][:, :],
                    in_=P["wo_r"][:, ds(kt * 1024, 1024)],
                )

            # -------- phase 2: attention (softmax along partitions) --------
            # Per (sc, h): tile list = 4 encoder tiles + decoder tiles that
            # are not fully masked (sc0: 4, sc1: 8). Scores into paired psum
            # banks, exp over the pair, Z via ones-stationary matmul, PV
            # accumulation, then one reciprocal + one mul. After each head's
            # oT is ready it is staged to the A2A input (quarters duplicated
            # to ranks q and q+4); the A2A for a slice fires after its 8th
            # head.
            with tc.tile_pool(name="opool", bufs=1) as opool, \
                 tc.tile_pool(name="ppool", bufs=3) as ppool, \
                 tc.tile_pool(name="zpool", bufs=2) as zpool, \
                 tc.tile_pool(name="psS", bufs=3, space="PSUM") as psS, \
                 tc.tile_pool(name="psZ", bufs=1, space="PSUM") as psZ, \
                 tc.tile_pool(name="psV", bufs=1, space="PSUM") as psV:
              oT = opool.tile([128, H * S], bf16, tag="oT")  # cols h*S + s

              def tiles_for(sc):
                  # (tt, diag_j): tt indexes kT/vsb t-tiles; diag_j is the
                  # diagonal-mask index or None. Fully-masked tiles skipped.
                  lst = [(tt, None) for tt in range(4)]  # encoder
                  if sc == 0:
                      lst += [(4 + j, j) for j in range(4)]
                  else:
                      lst += [(tt, None) for tt in range(4, 8)]
                      lst += [(8 + j, j) for j in range(4)]
                  return lst

              def emit_A(sc, h, pbuf):
                  tl = tiles_for(sc)
                  for k0 in range(0, len(tl), 2):
                      pr = psS.tile([128, 1024], fp32, tag="psS",
                                    name=f"sc{sc}h{h}p{k0}")
                      for half in range(2):
                          tt, dj = tl[k0 + half]
                          nc.tensor.matmul(
                              pr[:, ds(half * 512, 512)],
                              kT[:, ds(h * T + tt * 128, 128)],
                              qT[:, ds(h * S + sc * 512, 512)],
                              start=True,
                              stop=True,
                          )
                      nc.scalar.activation(
                          pbuf[:, ds(k0 * 512, 1024)], pr[:, :], AF.Exp
                      )
                      # causal zeroing of the diagonal-band tiles, post-exp
                      for half in range(2):
                          tt, dj = tl[k0 + half]
                          if dj is not None:
                              nc.vector.tensor_mul(
                                  pbuf[:, ds((k0 + half) * 512, 512)],
                                  pbuf[:, ds((k0 + half) * 512, 512)],
                                  dmsk[:, ds(dj * 512, 512)],
                              )

              def emit_B(sc, h, pbuf):
                  tl = tiles_for(sc)
                  n = len(tl)
                  zp = psZ.tile([128, 512], fp32, tag="psZ", name=f"z{sc}{h}")
                  for k, (tt, _) in enumerate(tl):
                      nc.tensor.matmul(
                          zp[:, :],
                          onesb[:, :],
                          pbuf[:, ds(k * 512, 512)],
                          start=(k == 0),
                          stop=(k == n - 1),
                      )
                  zr = zpool.tile([128, 512], fp32, tag="zr", name=f"zr{sc}{h}")
                  nc.vector.reciprocal_approx_fast(zr[:, :], zp[:, :])
                  pv = psV.tile([128, 512], fp32, tag="psV", name=f"pv{sc}{h}")
                  for k, (tt, _) in enumerate(tl):
                      nc.tensor.matmul(
                          pv[:, :],
                          vsb[:, ds(tt * O + h * 128, 128)],
                          pbuf[:, ds(k * 512, 512)],
                          start=(k == 0),
                          stop=(k == n - 1),
                      )
                  nc.vector.tensor_mul(
                      oT[:, ds(h * S + sc * 512, 512)], pv[:, :], zr[:, :]
                  )
                  # stage this head's slice into the A2A input: token quarter
                  # q goes to dest-rank rows q*128 and (4+q)*128
                  for q in range(4):
                      src = oT[:, ds(h * S + sc * 512 + q * 128, 128)]
                      for dup in range(2):
                          nc.sync.dma_start(
                              out=a2a_in[sc][
                                  ds((dup * 4 + q) * 128, 128),
                                  ds(h * 128, 128),
                              ],
                              in_=src,
                          )

              def fire_a2a(sc):
                  if no_collective:
                      nc.gpsimd.dma_start(
                          out=a2a_out[sc][:, :], in_=a2a_in[sc][:, :]
                      )
                  else:
                      nc.gpsimd.collective_compute(
                          "AllToAll",
                          mybir.AluOpType.bypass,
                          replica_groups=A2A_GROUP,
                          ins=[a2a_in[sc][:, :].opt()],
                          outs=[a2a_out[sc][:, :].opt()],
                      )

              # software pipeline: 2-head lookahead on A emits
              pend = []
              b_count = 0
              for sc in range(2):
                  for h in range(H):
                      pbuf = ppool.tile(
                          [128, 12 * 512], bf16, tag="p", name=f"pb{sc}{h}"
                      )
                      emit_A(sc, h, pbuf)
                      pend.append((sc, h, pbuf))
                      if len(pend) == 3:
                          s0, h0, pb0 = pend.pop(0)
                          emit_B(s0, h0, pb0)
                          b_count += 1
                          if b_count == 8:
                              fire_a2a(0)
              for s0, h0, pb0 in pend:
                  emit_B(s0, h0, pb0)
                  b_count += 1
              # A2A#2 first: the gpsimd queue sits in the A2A#1 completion
              # wait, and a collective op also waits for completion, so any
              # load placed between the two triggers would delay the second
              # trigger past its own transfer time.
              fire_a2a(1)
              # sc0 recv tiles on sync (free once staging ends), in wo
              # consumption order so the first k-tiles arrive first
              for i in (0, 4, 1, 5, 2, 6, 3, 7):
                  nc.sync.dma_start(
                      out=rt0[i][:, :],
                      in_=a2a_out[0][ds(i * 128, 128), :],
                  )

            # ---------------- phase 3: wo (y = oT_full.T @ woT) ----------
            # pass 1: d-chunks 0-1 (slabs resident from prefetch); sc0 first
            # (independent of A2A#2, hides it), then sc1. pass 2: d-chunks
            # 2-3 with freshly streamed slabs, sc0+sc1 jointly.
            with tc.tile_pool(name="wo2", bufs=8) as wo2, \
                 tc.tile_pool(name="rt1p", bufs=1) as rt1p, \
                 tc.tile_pool(name="ypool", bufs=1) as ypool, \
                 tc.tile_pool(name="psW", bufs=8, space="PSUM") as psW:
                rt1 = [
                    rt1p.tile([128, 1024], bf16, tag=f"rt1_{i}",
                              name=f"rt1_{i}")
                    for i in range(8)
                ]
                # sc1 recv: in consumption order (kt walks ig with both
                # batches), split over gpsimd (parked right behind the
                # A2A#2 completion wait) and sync (idle after staging)
                for k, i in enumerate((0, 4, 1, 5, 2, 6, 3, 7)):
                    (nc.gpsimd if k % 2 == 0 else nc.sync).dma_start(
                        out=rt1[i][:, :], in_=a2a_out[1][ds(i * 128, 128), :]
                    )
                rts = [rt0, rt1]
                yt = {
                    (sc, b): ypool.tile(
                        [128, DHALF], bf16, tag=f"y{sc}{b}", name=f"y{sc}{b}"
                    )
                    for sc in range(2) for b in range(2)
                }

                def wo_block(sc, dcs, slabs):
                    # psum[t, d] accumulated over all 32 k-tiles
                    pw = {
                        (b, dc): psW.tile([128, 512], fp32, tag="psW",
                                          name=f"pw{sc}{b}{dc}")
                        for b in range(2) for dc in dcs
                    }
                    for kt in range(32):
                        ig, h = divmod(kt, 8)
                        for b in range(2):
                            stat = rts[sc][b * 4 + ig][:, ds(h * 128, 128)]
                            for dc in dcs:
                                nc.tensor.matmul(
                                    pw[(b, dc)][:, :],
                                    stat,
                                    slabs[kt][:, ds((dc % 2) * 512, 512)],
                                    start=(kt == 0),
                                    stop=(kt == 31),
                                )
                    for b in range(2):
                        for dc in dcs:
                            nc.scalar.copy(
                                yt[(sc, b)][:, ds(dc * 512, 512)],
                                pw[(b, dc)][:, :],
                            )

                # pass 1 (resident slabs): sc0 then sc1
                wo_block(0, (0, 1), slab1)
                wo_block(1, (0, 1), slab1)
                # pass 2: stream the other d-half of each slab; one JOINT
                # kt loop over both slices so the rotating slab slots are
                # fully consumed before their reuse (no FIFO inversion)
                pw2 = {
                    (sc, b, dc): psW.tile([128, 512], fp32, tag="psW",
                                          name=f"p2w{sc}{b}{dc}")
                    for sc in range(2) for b in range(2) for dc in (2, 3)
                }
                for kt in range(32):
                    sl = wo2.tile([128, 1024], bf16, tag="sl2",
                                  name=f"slab2_{kt}")
                    (nc.scalar if kt % 2 == 0 else nc.sync).dma_start(
                        out=sl[:, :],
                        in_=P["wo_r"][:, ds(32768 + kt * 1024, 1024)],
                    )
                    ig, h = divmod(kt, 8)
                    for sc in range(2):
                        for b in range(2):
                            stat = rts[sc][b * 4 + ig][:, ds(h * 128, 128)]
                            for dc in (2, 3):
                                nc.tensor.matmul(
                                    pw2[(sc, b, dc)][:, :],
                                    stat,
                                    sl[:, ds((dc % 2) * 512, 512)],
                                    start=(kt == 0),
                                    stop=(kt == 31),
                                )
                for sc in range(2):
                    for b in range(2):
                        for dc in (2, 3):
                            nc.scalar.copy(
                                yt[(sc, b)][:, ds(dc * 512, 512)],
                                pw2[(sc, b, dc)][:, :],
                            )
                        nc.sync.dma_start(
                            out=out[ds(sc * 256 + b * 128, 128), :],
                            in_=yt[(sc, b)][:, :],
                        )


def _prep_in_maps(x, freqs_cos, freqs_sin, mask, encoder_output, wq, wk, wv, wo):
    x = np.asarray(x, np.float32)
    encoder_output = np.asarray(encoder_output, np.float32)
    freqs_cos = np.asarray(freqs_cos, np.float32)
    freqs_sin = np.asarray(freqs_sin, np.float32)
    wq = np.asarray(wq, np.float32)
    wk = np.asarray(wk, np.float32)
    wv = np.asarray(wv, np.float32)
    wo = np.asarray(wo, np.float32)

    def perm(w):  # deinterleave rope pairs per head: even dims first
        w4 = w.reshape(H, 64, 2, D)
        return np.ascontiguousarray(w4.transpose(0, 2, 1, 3)).reshape(O, D)

    def slab256(wT):  # [D, O] -> [128, 4*32*256]: pass p, dt n, col c
        w4 = wT.reshape(NDT, 128, 4, 256)            # [n, part, p, c]
        return np.ascontiguousarray(
            w4.transpose(1, 2, 0, 3)
        ).reshape(128, NDT * O)

    def slab512(wT):  # [D, O] -> [128, 2*32*512]: oc, dt n, col c
        w4 = wT.reshape(NDT, 128, 2, 512)
        return np.ascontiguousarray(
            w4.transpose(1, 2, 0, 3)
        ).reshape(128, NDT * O)

    alpha = 1.0 / np.sqrt(DH)
    cosT = freqs_cos.T  # [64, S]
    sinT = freqs_sin.T
    csq_cos = (np.concatenate([cosT, cosT], 0) * alpha).astype(BF16)
    csq_sin = (np.concatenate([-sinT, sinT], 0) * alpha).astype(BF16)
    csk_cos = np.concatenate([cosT, cosT], 0).astype(BF16)
    csk_sin = np.concatenate([-sinT, sinT], 0).astype(BF16)

    # 4 diagonal-band keep-masks (0/1, applied post-exp):
    # dmask[t, j*512+s] = 0 if s < t + j*128 else 1
    t_i = np.arange(128)[:, None]
    s_i = np.arange(512)[None, :]
    dmask = np.concatenate(
        [np.where(s_i < t_i + j * 128, 0.0, 1.0) for j in range(4)], axis=1
    ).astype(BF16)
    ones = np.ones((128, 128), BF16)

    # woT slabs: full wo.T (k = head*128+dh on partitions per k-tile), this
    # core's d-half, split into two 1024-wide passes
    woT = np.ascontiguousarray(wo.T).reshape(32, 128, D)  # [kt, dh, dout]

    in_maps = []
    for c in range(8):
        g, r = divmod(c, 4)
        dhalf = g
        sl = slice(r * O, (r + 1) * O)
        xhat = np.concatenate([encoder_output[g], x[g]], axis=0)  # [T, D]
        xhatT = xhat.T.astype(BF16)                               # [D, T]
        x_r = np.ascontiguousarray(
            xhatT.reshape(NDT, 128, T).transpose(1, 0, 2)
        ).reshape(128, NDT * T)
        wqT = perm(wq[sl]).T.astype(BF16)   # [D, O]
        wkT = perm(wk[sl]).T.astype(BF16)
        wvT = wv[sl].T.astype(BF16)
        wo_c = woT[:, :, dhalf * DHALF:(dhalf + 1) * DHALF]  # [32,128,2048]
        wo_r = np.ascontiguousarray(
            wo_c.reshape(32, 128, 2, 1024).transpose(1, 2, 0, 3)
        ).reshape(128, 2 * 32 * 1024).astype(BF16)
        in_maps.append(
            {
                "x_r": x_r,
                "wq_r": slab256(wqT),
                "wk_r": slab256(wkT),
                "wv_r": slab512(wvT),
                "wo_r": wo_r,
                "csq_cos": csq_cos,
                "csq_sin": csq_sin,
                "csk_cos": csk_cos,
                "csk_sin": csk_sin,
                "dmask": dmask,
                "ones": ones,
            }
        )
    return in_maps


def _gather(outs):
    full = np.zeros((2, S, D), np.float32)
    for c in range(8):
        g, q = divmod(c, 4)
        dhalf = g
        o = np.asarray(outs[c]).astype(np.float32)  # [512, 2048]
        for sc in range(2):
            for b in range(2):
                rows = o[sc * 256 + b * 128: sc * 256 + b * 128 + 128]
                full[b, sc * 512 + q * 128: sc * 512 + q * 128 + 128,
                     dhalf * DHALF:(dhalf + 1) * DHALF] = rows
    return full


def kernel(x, start_pos, freqs_cos, freqs_sin, mask, encoder_output, wq, wk, wv, wo):
    global LAST_EXEC_NS
    from concourse.bass_utils import run_bass_kernel_spmd

    if "nc" not in _CACHE:
        _CACHE["nc"] = _build()
    nc = _CACHE["nc"]

    in_maps = _prep_in_maps(
        x, freqs_cos, freqs_sin, mask, encoder_output, wq, wk, wv, wo
    )
    res = run_bass_kernel_spmd(nc, in_maps, core_ids=list(range(8)))
    LAST_EXEC_NS = res.exec_time_ns
    return _gather([res.results[c]["out"] for c in range(8)])


# revision 17
# speedup vs baseline: 1.0405x; 1.0405x over previous
"""Trainium2 Bass kernel for nn_Attention (dense transformer attention block).

Full inputs -> full output. Internally: 8 NeuronCores, 2 data-parallel groups
(batch) x 4-way tensor-parallel (heads). Each core computes 8 heads for one
batch element. The wo projection is redistributed with a single 8-rank
AllToAll per 512-token slice: each core ships its heads' attention output
(oT, feature-major) for token-quarter q to ranks q and q+4, and afterwards
holds the FULL 4096-feature oT for one 128-token quarter of each batch --
it then computes y for those rows over one 2048-wide d-half (group 0 takes
d 0:2048, group 1 d 2048:4096). No ReduceScatter; the A2A moves 2 MB/rank
at ~24 us (mesh) and the sc1 A2A hides under the sc0 wo pass.

Compute in bf16 on the TensorEngine (fp32 matmul is 4x slower), fp32 PSUM
accumulation. All operand layouts are pre-rearranged host-side so every
device DMA is a contiguous per-partition block:
  - projections:  qT/kT = (w-tile).T @ xhatT-tile   -> [feature, seq] layout
  - scores:       scoresT[t, s] = kT-tile.T @ qT    (softmax along partitions)
  - Z:            ones[128,128].T @ probs           -> Z broadcast to all rows
  - PV:           oT[dh, s] = v-tile.T @ probsT
  - wo:           y[t, d] = oT-recv-tile.T @ woT-slab (K=4096 in one psum)
Causal structure is exploited: score tiles that are fully masked are skipped
(scores/exp/Z/PV), and only the 4 diagonal-band tiles per query chunk get a
post-exp 0/1 multiply (from 4 precomputed [128,512] masks). RoPE pairs are
deinterleaved host-side (even dims first); the 1/sqrt(128) score scale is
folded into the q-side cos/sin tables.
"""

import sys

import numpy as np

for _p in ("/opt/trn_rl_repo",):
    if _p not in sys.path:
        sys.path.insert(0, _p)

import ml_dtypes

BF16 = ml_dtypes.bfloat16

D = 4096      # model dim
S = 1024      # decoder sequence length
E = 512       # encoder length
T = E + S     # total key length
H = 8         # heads per core (32 total / 4-way TP)
DH = 128      # head dim
O = H * DH    # per-core projection width = 1024
NDT = D // 128
NEG = -1e9
A2A_GROUP = [[0, 1, 2, 3, 4, 5, 6, 7]]
DHALF = 2048  # per-core output d-half width

_CACHE = {}
LAST_EXEC_NS = None


def _build(no_collective=False):
    import concourse.mybir as mybir
    import concourse.tile as tile
    from concourse import bacc

    bf16 = mybir.dt.bfloat16

    nc = bacc.Bacc(
        "TRN2",
        target_bir_lowering=False,
        debug=False,
        num_devices=8,
    )

    P = {}
    for name, shape in [
        ("x_r", [128, NDT * T]),        # xhatT slabs: cols dt*T + t
        ("wq_r", [128, NDT * O]),       # Q pass slabs: cols p*8192 + n*256 + c
        ("wk_r", [128, NDT * O]),       # K pass slabs: same geometry
        ("wv_r", [128, NDT * O]),       # V slabs: cols oc*16384 + n*512 + c
        ("wo_r", [128, 2 * 32 * 1024]), # woT slabs: cols p*32768 + kt*1024 + d
        ("csq_cos", [128, S]),
        ("csq_sin", [128, S]),
        ("csk_cos", [128, S]),
        ("csk_sin", [128, S]),
        ("dmask", [128, 4 * 512]),      # 4 diagonal-band masks
        ("ones", [128, 128]),
    ]:
        P[name] = nc.declare_dram_parameter(name, shape, bf16, isOutput=False)
    # rows: sc*256 + b*128 + t ; cols: d within this core's d-half
    out = nc.declare_dram_parameter("out", [512, DHALF], bf16, isOutput=True)

    with tile.TileContext(nc) as tc:
        _emit(nc, tc, P, out, no_collective=no_collective)
    nc.compile()
    return nc


def _emit(nc, tc, P, out, no_collective=False):
    import concourse.mybir as mybir
    from concourse.bass import ds

    bf16 = mybir.dt.bfloat16
    fp32 = mybir.dt.float32
    AF = mybir.ActivationFunctionType

    with tc.tile_pool(name="res", bufs=1) as res, \
         tc.tile_pool(name="dram", bufs=1, space="DRAM") as dram:
        onesb = res.tile([128, 128], bf16, tag="onesb")
        dmsk = res.tile([128, 4 * 512], bf16, tag="dmsk")  # 0/1 keep masks

        # A2A staging: rows j*128 + dh (j = dest rank), cols h*128 + t
        a2a_in = [
            dram.tile([1024, 1024], bf16, tag=f"ai{sc}", name=f"a2a_in{sc}")
            for sc in range(2)
        ]
        a2a_out = [
            dram.tile([1024, 1024], bf16, tag=f"ao{sc}", name=f"a2a_out{sc}")
            for sc in range(2)
        ]

        nc.gpsimd.dma_start(out=dmsk[:, :], in_=P["dmask"][:, :])
        nc.gpsimd.dma_start(out=onesb[:, :], in_=P["ones"][:, :])

        # tiny warm-up collective (fired a little into phase 1): absorbs the
        # ~11us first-collective spin-up on the CC stream during projections
        warm_in = dram.tile([8, 128], bf16, tag="wi", name="warm_in")
        warm_out = dram.tile([8, 128], bf16, tag="wo", name="warm_out")

        def fire_warmup():
            if no_collective:
                return
            nc.gpsimd.dma_start(out=warm_in[:, :], in_=P["ones"][ds(0, 8), :])
            nc.gpsimd.collective_compute(
                "AllToAll",
                mybir.AluOpType.bypass,
                replica_groups=A2A_GROUP,
                ins=[warm_in[:, :].opt()],
                outs=[warm_out[:, :].opt()],
            )

        with tc.tile_pool(name="qkv", bufs=1) as qkv:
          qT = qkv.tile([128, H * S], bf16, tag="qT")     # cols h*S + s
          kT = qkv.tile([128, H * T], bf16, tag="kT")     # cols h*T + t
          vsb = qkv.tile([128, 12 * O], bf16, tag="vsb")  # cols tt*O + o

          # ---------------- phase 1: projections + rope ----------------
          with tc.tile_pool(name="xpool", bufs=1) as xpool, \
               tc.tile_pool(name="tabpool", bufs=1) as tabpool, \
               tc.tile_pool(name="wpool", bufs=3) as wpool, \
               tc.tile_pool(name="rtmp", bufs=2) as rtmp, \
               tc.tile_pool(name="ps1", bufs=8, space="PSUM") as ps1:
            # first Q weight half-slab ahead of everything on the sync queue
            def wslab(src, off, n, name):
                wr = wpool.tile([128, n], bf16, tag="wr", name=name)
                nc.sync.dma_start(out=wr[:, :], in_=P[src][:, ds(off, n)])
                return wr

            wr_q00 = wslab("wq_r", 0, 16 * 256, "wr_q0_0")
            xh = []
            for dt in range(NDT):
                xt = xpool.tile([128, T], bf16, tag=f"xh{dt}", name=f"xh{dt}")
                (nc.scalar if dt % 2 == 0 else nc.gpsimd).dma_start(
                    out=xt[:, :], in_=P["x_r"][:, ds(dt * T, T)]
                )
                xh.append(xt)
            csqc = tabpool.tile([128, S], bf16, tag="csqc")
            csqs = tabpool.tile([128, S], bf16, tag="csqs")
            cskc = tabpool.tile([128, S], bf16, tag="cskc")
            csks = tabpool.tile([128, S], bf16, tag="csks")
            nc.scalar.dma_start(out=csqc[:, :], in_=P["csq_cos"][:, :])
            nc.scalar.dma_start(out=csqs[:, :], in_=P["csq_sin"][:, :])
            nc.gpsimd.dma_start(out=cskc[:, :], in_=P["csk_cos"][:, :])
            nc.gpsimd.dma_start(out=csks[:, :], in_=P["csk_sin"][:, :])
            fire_warmup()

            # RoPE: tables are full-height with the 64-row block duplicated
            # (cos) or sign-split (-sin; +sin), so every TensorTensor is
            # partition-aligned. The half-swap goes through an SBUF-SBUF DMA.
            def rope(buf, base, cos, sin, tag):
                swp = rtmp.tile([128, S], bf16, tag="swp", name=f"swp_{tag}")
                nc.gpsimd.dma_start(
                    out=swp[ds(0, 64), :], in_=buf[ds(64, 64), ds(base, S)]
                )
                nc.gpsimd.dma_start(
                    out=swp[ds(64, 64), :], in_=buf[ds(0, 64), ds(base, S)]
                )
                nc.vector.tensor_mul(swp[:, :], swp[:, :], sin[:, :])
                nc.vector.tensor_mul(
                    buf[:, ds(base, S)], buf[:, ds(base, S)], cos[:, :]
                )
                nc.vector.tensor_add(
                    buf[:, ds(base, S)], buf[:, ds(base, S)], swp[:, :]
                )

            # Q: 4 passes x (2 o_tiles x 2 s_chunks); K: 4 passes x
            # (2 o_tiles x 3 t_chunks). Weight slabs stream in 16-dt halves.
            for src, nch, xoff, obuf, ostride in (
                ("wq_r", 2, E, qT, S),
                ("wk_r", 3, 0, kT, T),
            ):
                for p in range(4):
                    ps = [
                        [ps1.tile([128, 512], fp32, tag="ps1",
                                  name=f"ps_{src}_{p}_{oi}_{cc}")
                         for cc in range(nch)]
                        for oi in range(2)
                    ]
                    for half in range(2):
                        if src == "wq_r" and p == 0 and half == 0:
                            wr = wr_q00
                        else:
                            wr = wslab(
                                src, (p * 2 + half) * 16 * 256, 16 * 256,
                                f"wr_{src}_{p}_{half}",
                            )
                        for dtl in range(16):
                            dt = half * 16 + dtl
                            for oi in range(2):
                                for cc in range(nch):
                                    nc.tensor.matmul(
                                        ps[oi][cc][:, :],
                                        wr[:, ds(dtl * 256 + oi * 128, 128)],
                                        xh[dt][:, ds(xoff + cc * 512, 512)],
                                        start=(dt == 0),
                                        stop=(dt == NDT - 1),
                                    )
                    for oi in range(2):
                        h = 2 * p + oi
                        for cc in range(nch):
                            nc.scalar.copy(
                                obuf[:, ds(h * ostride + cc * 512, 512)],
                                ps[oi][cc][:, :],
                            )
                        if src == "wq_r":
                            rope(qT, h * S, csqc, csqs, f"q{h}")
                        else:
                            rope(kT, h * T + E, cskc, csks, f"k{h}")

            # V (x-stationary): 2 o_chunks x 2 t_groups of 6 tiles; weight
            # slabs re-streamed per t_group in two 16-dt halves
            for oc in range(2):
                for tg in range(2):
                    tb = tg * 6
                    psv = [ps1.tile([128, 512], fp32, tag="ps1",
                                    name=f"psv_{oc}_{tg}_{ti}")
                           for ti in range(6)]
                    for qr in range(4):
                        wr = wpool.tile(
                            [128, 8 * 512], bf16, tag="wr",
                            name=f"wr_v{oc}_{tg}_{qr}",
                        )
                        nc.sync.dma_start(
                            out=wr[:, :],
                            in_=P["wv_r"][
                                :, ds(oc * NDT * 512 + qr * 8 * 512, 8 * 512)
                            ],
                        )
                        for dtl in range(8):
                            dt = qr * 8 + dtl
                            for ti in range(6):
                                nc.tensor.matmul(
                                    psv[ti][:, :],
                                    xh[dt][:, ds((tb + ti) * 128, 128)],
                                    wr[:, ds(dtl * 512, 512)],
                                    start=(dt == 0),
                                    stop=(dt == NDT - 1),
                                )
                    for ti in range(6):
                        nc.scalar.copy(
                            vsb[:, ds((tb + ti) * O + oc * 512, 512)],
                            psv[ti][:, :],
                        )

          # wo pass-1 slabs + sc0 recv tiles prefetch during attention; this
          # pool sits in the (dead) phase-1 x region so its DMAs only wait on
          # the last projection matmul, not on attention.
          with tc.tile_pool(name="wpre", bufs=1) as wpre:
            slab1 = [
                wpre.tile([128, 1024], bf16, tag=f"sl1_{kt}", name=f"slab1_{kt}")
                for kt in range(32)
            ]
            rt0 = [
                wpre.tile([128, 1024], bf16, tag=f"rt0_{i}", name=f"rt0_{i}")
                for i in range(8)
            ]
            # stream pass-1 slabs during attention, all on gpsimd: its only
            # later work is the A2A triggers, so ring-credit waits here
            # never stall a compute-feeding queue (scalar's exp stream
            # stalled ~18us when half of these sat on it)
            for kt in range(32):
                nc.gpsimd.dma_start(
                    out=slab1[kt][:, :],
                    in_=P["wo_r"][:, ds(kt * 1024, 1024)],
                )

            # -------- phase 2: attention (softmax along partitions) --------
            # Per (sc, h): tile list = 4 encoder tiles + decoder tiles that
            # are not fully masked (sc0: 4, sc1: 8). Scores into paired psum
            # banks, exp over the pair, Z via ones-stationary matmul, PV
            # accumulation, then one reciprocal + one mul. After each head's
            # oT is ready it is staged to the A2A input (quarters duplicated
            # to ranks q and q+4); the A2A for a slice fires after its 8th
            # head.
            with tc.tile_pool(name="opool", bufs=1) as opool, \
                 tc.tile_pool(name="ppool", bufs=3) as ppool, \
                 tc.tile_pool(name="zpool", bufs=2) as zpool, \
                 tc.tile_pool(name="psS", bufs=3, space="PSUM") as psS, \
                 tc.tile_pool(name="psZ", bufs=1, space="PSUM") as psZ, \
                 tc.tile_pool(name="psV", bufs=1, space="PSUM") as psV:
              oT = opool.tile([128, H * S], bf16, tag="oT")  # cols h*S + s

              def tiles_for(sc):
                  # (tt, diag_j): tt indexes kT/vsb t-tiles; diag_j is the
                  # diagonal-mask index or None. Fully-masked tiles skipped.
                  lst = [(tt, None) for tt in range(4)]  # encoder
                  if sc == 0:
                      lst += [(4 + j, j) for j in range(4)]
                  else:
                      lst += [(tt, None) for tt in range(4, 8)]
                      lst += [(8 + j, j) for j in range(4)]
                  return lst

              def emit_A(sc, h, pbuf):
                  tl = tiles_for(sc)
                  for k0 in range(0, len(tl), 2):
                      pr = psS.tile([128, 1024], fp32, tag="psS",
                                    name=f"sc{sc}h{h}p{k0}")
                      for half in range(2):
                          tt, dj = tl[k0 + half]
                          nc.tensor.matmul(
                              pr[:, ds(half * 512, 512)],
                              kT[:, ds(h * T + tt * 128, 128)],
                              qT[:, ds(h * S + sc * 512, 512)],
                              start=True,
                              stop=True,
                          )
                      nc.scalar.activation(
                          pbuf[:, ds(k0 * 512, 1024)], pr[:, :], AF.Exp
                      )
                      # causal zeroing of the diagonal-band tiles, post-exp
                      for half in range(2):
                          tt, dj = tl[k0 + half]
                          if dj is not None:
                              nc.vector.tensor_mul(
                                  pbuf[:, ds((k0 + half) * 512, 512)],
                                  pbuf[:, ds((k0 + half) * 512, 512)],
                                  dmsk[:, ds(dj * 512, 512)],
                              )

              def emit_B(sc, h, pbuf):
                  tl = tiles_for(sc)
                  n = len(tl)
                  zp = psZ.tile([128, 512], fp32, tag="psZ", name=f"z{sc}{h}")
                  for k, (tt, _) in enumerate(tl):
                      nc.tensor.matmul(
                          zp[:, :],
                          onesb[:, :],
                          pbuf[:, ds(k * 512, 512)],
                          start=(k == 0),
                          stop=(k == n - 1),
                      )
                  zr = zpool.tile([128, 512], fp32, tag="zr", name=f"zr{sc}{h}")
                  nc.vector.reciprocal_approx_fast(zr[:, :], zp[:, :])
                  pv = psV.tile([128, 512], fp32, tag="psV", name=f"pv{sc}{h}")
                  for k, (tt, _) in enumerate(tl):
                      nc.tensor.matmul(
                          pv[:, :],
                          vsb[:, ds(tt * O + h * 128, 128)],
                          pbuf[:, ds(k * 512, 512)],
                          start=(k == 0),
                          stop=(k == n - 1),
                      )
                  nc.vector.tensor_mul(
                      oT[:, ds(h * S + sc * 512, 512)], pv[:, :], zr[:, :]
                  )
                  # stage this head's slice into the A2A input: token quarter
                  # q goes to dest-rank rows q*128 (batch-group 0 dests) and
                  # (4+q)*128 (group 1); one 3-dim DMA per dest group
                  src = oT[:, ds(h * S + sc * 512, 512)].rearrange(
                      "dh (q t) -> dh q t", q=4
                  )
                  for g in range(2):
                      dst = a2a_in[sc][
                          ds(g * 512, 512), ds(h * 128, 128)
                      ].rearrange("(q dh) t -> dh q t", q=4)
                      nc.sync.dma_start(out=dst, in_=src)

              def fire_a2a(sc):
                  if no_collective:
                      nc.gpsimd.dma_start(
                          out=a2a_out[sc][:, :], in_=a2a_in[sc][:, :]
                      )
                  else:
                      nc.gpsimd.collective_compute(
                          "AllToAll",
                          mybir.AluOpType.bypass,
                          replica_groups=A2A_GROUP,
                          ins=[a2a_in[sc][:, :].opt()],
                          outs=[a2a_out[sc][:, :].opt()],
                      )

              # software pipeline: 2-head lookahead on A emits
              pend = []
              b_count = 0
              for sc in range(2):
                  for h in range(H):
                      pbuf = ppool.tile(
                          [128, 12 * 512], bf16, tag="p", name=f"pb{sc}{h}"
                      )
                      emit_A(sc, h, pbuf)
                      pend.append((sc, h, pbuf))
                      if len(pend) == 3:
                          s0, h0, pb0 = pend.pop(0)
                          emit_B(s0, h0, pb0)
                          b_count += 1
                          if b_count == 8:
                              fire_a2a(0)
              for s0, h0, pb0 in pend:
                  emit_B(s0, h0, pb0)
                  b_count += 1
              # A2A#2 first: the gpsimd queue sits in the A2A#1 completion
              # wait, and a collective op also waits for completion, so any
              # load placed between the two triggers would delay the second
              # trigger past its own transfer time.
              fire_a2a(1)
              # sc0 recv tiles on sync (free once staging ends), in wo
              # consumption order so the first k-tiles arrive first
              for i in (0, 4, 1, 5, 2, 6, 3, 7):
                  nc.sync.dma_start(
                      out=rt0[i][:, :],
                      in_=a2a_out[0][ds(i * 128, 128), :],
                  )

            # ---------------- phase 3: wo (y = oT_full.T @ woT) ----------
            # pass 1: d-chunks 0-1 (slabs resident from prefetch); sc0 first
            # (independent of A2A#2, hides it), then sc1. pass 2: d-chunks
            # 2-3 with freshly streamed slabs, sc0+sc1 jointly.
            with tc.tile_pool(name="wo2", bufs=8) as wo2, \
                 tc.tile_pool(name="rt1p", bufs=1) as rt1p, \
                 tc.tile_pool(name="ypool", bufs=1) as ypool, \
                 tc.tile_pool(name="psW", bufs=8, space="PSUM") as psW:
                rt1 = [
                    rt1p.tile([128, 1024], bf16, tag=f"rt1_{i}",
                              name=f"rt1_{i}")
                    for i in range(8)
                ]
                # sc1 recv: in consumption order (kt walks ig with both
                # batches), split over gpsimd (parked right behind the
                # A2A#2 completion wait) and sync (idle after staging)
                for k, i in enumerate((0, 4, 1, 5, 2, 6, 3, 7)):
                    (nc.gpsimd if k % 2 == 0 else nc.sync).dma_start(
                        out=rt1[i][:, :], in_=a2a_out[1][ds(i * 128, 128), :]
                    )
                rts = [rt0, rt1]
                yt = {
                    (sc, b): ypool.tile(
                        [128, DHALF], bf16, tag=f"y{sc}{b}", name=f"y{sc}{b}"
                    )
                    for sc in range(2) for b in range(2)
                }

                def wo_block(sc, dcs, slabs):
                    # psum[t, d] accumulated over all 32 k-tiles
                    pw = {
                        (b, dc): psW.tile([128, 512], fp32, tag="psW",
                                          name=f"pw{sc}{b}{dc}")
                        for b in range(2) for dc in dcs
                    }
                    for kt in range(32):
                        ig, h = divmod(kt, 8)
                        for b in range(2):
                            stat = rts[sc][b * 4 + ig][:, ds(h * 128, 128)]
                            for dc in dcs:
                                nc.tensor.matmul(
                                    pw[(b, dc)][:, :],
                                    stat,
                                    slabs[kt][:, ds((dc % 2) * 512, 512)],
                                    start=(kt == 0),
                                    stop=(kt == 31),
                                )
                    for b in range(2):
                        for dc in dcs:
                            nc.scalar.copy(
                                yt[(sc, b)][:, ds(dc * 512, 512)],
                                pw[(b, dc)][:, :],
                            )

                # pass 1 (resident slabs): sc0 then sc1
                wo_block(0, (0, 1), slab1)
                wo_block(1, (0, 1), slab1)
                # pass 2: stream the other d-half of each slab; one JOINT
                # kt loop over both slices so the rotating slab slots are
                # fully consumed before their reuse (no FIFO inversion)
                pw2 = {
                    (sc, b, dc): psW.tile([128, 512], fp32, tag="psW",
                                          name=f"p2w{sc}{b}{dc}")
                    for sc in range(2) for b in range(2) for dc in (2, 3)
                }
                for kt in range(32):
                    sl = wo2.tile([128, 1024], bf16, tag="sl2",
                                  name=f"slab2_{kt}")
                    (nc.scalar if kt % 2 == 0 else nc.sync).dma_start(
                        out=sl[:, :],
                        in_=P["wo_r"][:, ds(32768 + kt * 1024, 1024)],
                    )
                    ig, h = divmod(kt, 8)
                    for sc in range(2):
                        for b in range(2):
                            stat = rts[sc][b * 4 + ig][:, ds(h * 128, 128)]
                            for dc in (2, 3):
                                nc.tensor.matmul(
                                    pw2[(sc, b, dc)][:, :],
                                    stat,
                                    sl[:, ds((dc % 2) * 512, 512)],
                                    start=(kt == 0),
                                    stop=(kt == 31),
                                )
                for sc in range(2):
                    for b in range(2):
                        for dc in (2, 3):
                            nc.scalar.copy(
                                yt[(sc, b)][:, ds(dc * 512, 512)],
                                pw2[(sc, b, dc)][:, :],
                            )
                        nc.sync.dma_start(
                            out=out[ds(sc * 256 + b * 128, 128), :],
                            in_=yt[(sc, b)][:, :],
                        )


def _prep_in_maps(x, freqs_cos, freqs_sin, mask, encoder_output, wq, wk, wv, wo):
    x = np.asarray(x, np.float32)
    encoder_output = np.asarray(encoder_output, np.float32)
    freqs_cos = np.asarray(freqs_cos, np.float32)
    freqs_sin = np.asarray(freqs_sin, np.float32)
    wq = np.asarray(wq, np.float32)
    wk = np.asarray(wk, np.float32)
    wv = np.asarray(wv, np.float32)
    wo = np.asarray(wo, np.float32)

    def perm(w):  # deinterleave rope pairs per head: even dims first
        w4 = w.reshape(H, 64, 2, D)
        return np.ascontiguousarray(w4.transpose(0, 2, 1, 3)).reshape(O, D)

    def slab256(wT):  # [D, O] -> [128, 4*32*256]: pass p, dt n, col c
        w4 = wT.reshape(NDT, 128, 4, 256)            # [n, part, p, c]
        return np.ascontiguousarray(
            w4.transpose(1, 2, 0, 3)
        ).reshape(128, NDT * O)

    def slab512(wT):  # [D, O] -> [128, 2*32*512]: oc, dt n, col c
        w4 = wT.reshape(NDT, 128, 2, 512)
        return np.ascontiguousarray(
            w4.transpose(1, 2, 0, 3)
        ).reshape(128, NDT * O)

    alpha = 1.0 / np.sqrt(DH)
    cosT = freqs_cos.T  # [64, S]
    sinT = freqs_sin.T
    csq_cos = (np.concatenate([cosT, cosT], 0) * alpha).astype(BF16)
    csq_sin = (np.concatenate([-sinT, sinT], 0) * alpha).astype(BF16)
    csk_cos = np.concatenate([cosT, cosT], 0).astype(BF16)
    csk_sin = np.concatenate([-sinT, sinT], 0).astype(BF16)

    # 4 diagonal-band keep-masks (0/1, applied post-exp):
    # dmask[t, j*512+s] = 0 if s < t + j*128 else 1
    t_i = np.arange(128)[:, None]
    s_i = np.arange(512)[None, :]
    dmask = np.concatenate(
        [np.where(s_i < t_i + j * 128, 0.0, 1.0) for j in range(4)], axis=1
    ).astype(BF16)
    ones = np.ones((128, 128), BF16)

    # woT slabs: full wo.T (k = head*128+dh on partitions per k-tile), this
    # core's d-half, split into two 1024-wide passes
    woT = np.ascontiguousarray(wo.T).reshape(32, 128, D)  # [kt, dh, dout]

    in_maps = []
    for c in range(8):
        g, r = divmod(c, 4)
        dhalf = g
        sl = slice(r * O, (r + 1) * O)
        xhat = np.concatenate([encoder_output[g], x[g]], axis=0)  # [T, D]
        xhatT = xhat.T.astype(BF16)                               # [D, T]
        x_r = np.ascontiguousarray(
            xhatT.reshape(NDT, 128, T).transpose(1, 0, 2)
        ).reshape(128, NDT * T)
        wqT = perm(wq[sl]).T.astype(BF16)   # [D, O]
        wkT = perm(wk[sl]).T.astype(BF16)
        wvT = wv[sl].T.astype(BF16)
        wo_c = woT[:, :, dhalf * DHALF:(dhalf + 1) * DHALF]  # [32,128,2048]
        wo_r = np.ascontiguousarray(
            wo_c.reshape(32, 128, 2, 1024).transpose(1, 2, 0, 3)
        ).reshape(128, 2 * 32 * 1024).astype(BF16)
        in_maps.append(
            {
                "x_r": x_r,
                "wq_r": slab256(wqT),
                "wk_r": slab256(wkT),
                "wv_r": slab512(wvT),
                "wo_r": wo_r,
                "csq_cos": csq_cos,
                "csq_sin": csq_sin,
                "csk_cos": csk_cos,
                "csk_sin": csk_sin,
                "dmask": dmask,
                "ones": ones,
            }
        )
    return in_maps


def _gather(outs):
    full = np.zeros((2, S, D), np.float32)
    for c in range(8):
        g, q = divmod(c, 4)
        dhalf = g
        o = np.asarray(outs[c]).astype(np.float32)  # [512, 2048]
        for sc in range(2):
            for b in range(2):
                rows = o[sc * 256 + b * 128: sc * 256 + b * 128 + 128]
                full[b, sc * 512 + q * 128: sc * 512 + q * 128 + 128,
                     dhalf * DHALF:(dhalf + 1) * DHALF] = rows
    return full


def kernel(x, start_pos, freqs_cos, freqs_sin, mask, encoder_output, wq, wk, wv, wo):
    global LAST_EXEC_NS
    from concourse.bass_utils import run_bass_kernel_spmd

    if "nc" not in _CACHE:
        _CACHE["nc"] = _build()
    nc = _CACHE["nc"]

    in_maps = _prep_in_maps(
        x, freqs_cos, freqs_sin, mask, encoder_output, wq, wk, wv, wo
    )
    res = run_bass_kernel_spmd(nc, in_maps, core_ids=list(range(8)))
    LAST_EXEC_NS = res.exec_time_ns
    return _gather([res.results[c]["out"] for c in range(8)])


# revision 20
# speedup vs baseline: 1.0566x; 1.0154x over previous
"""Trainium2 Bass kernel for nn_Attention (dense transformer attention block).

Full inputs -> full output. Internally: 8 NeuronCores, 2 data-parallel groups
(batch) x 4-way tensor-parallel (heads). Each core computes 8 heads for one
batch element. The wo projection is redistributed with a single 8-rank
AllToAll per 512-token slice: each core ships its heads' attention output
(oT, feature-major) for token-quarter q to ranks q and q+4, and afterwards
holds the FULL 4096-feature oT for one 128-token quarter of each batch --
it then computes y for those rows over one 2048-wide d-half (group 0 takes
d 0:2048, group 1 d 2048:4096). No ReduceScatter; the A2A moves 2 MB/rank
at ~24 us (mesh) and the sc1 A2A hides under the sc0 wo pass.

Compute in bf16 on the TensorEngine (fp32 matmul is 4x slower), fp32 PSUM
accumulation. All operand layouts are pre-rearranged host-side so every
device DMA is a contiguous per-partition block:
  - projections:  qT/kT = (w-tile).T @ xhatT-tile   -> [feature, seq] layout
  - scores:       scoresT[t, s] = kT-tile.T @ qT    (softmax along partitions)
  - Z:            ones[128,128].T @ probs           -> Z broadcast to all rows
  - PV:           oT[dh, s] = v-tile.T @ probsT
  - wo:           y[t, d] = oT-recv-tile.T @ woT-slab (K=4096 in one psum)
Causal structure is exploited: score tiles that are fully masked are skipped
(scores/exp/Z/PV), and only the 4 diagonal-band tiles per query chunk get a
post-exp 0/1 multiply (from 4 precomputed [128,512] masks). RoPE pairs are
deinterleaved host-side (even dims first); the 1/sqrt(128) score scale is
folded into the q-side cos/sin tables.
"""

import sys

import numpy as np

for _p in ("/opt/trn_rl_repo",):
    if _p not in sys.path:
        sys.path.insert(0, _p)

import ml_dtypes

BF16 = ml_dtypes.bfloat16

D = 4096      # model dim
S = 1024      # decoder sequence length
E = 512       # encoder length
T = E + S     # total key length
H = 8         # heads per core (32 total / 4-way TP)
DH = 128      # head dim
O = H * DH    # per-core projection width = 1024
NDT = D // 128
NEG = -1e9
A2A_GROUP = [[0, 1, 2, 3, 4, 5, 6, 7]]
DHALF = 2048  # per-core output d-half width

_CACHE = {}
LAST_EXEC_NS = None


def _build(no_collective=False):
    import concourse.mybir as mybir
    import concourse.tile as tile
    from concourse import bacc

    bf16 = mybir.dt.bfloat16

    nc = bacc.Bacc(
        "TRN2",
        target_bir_lowering=False,
        debug=False,
        num_devices=8,
    )

    P = {}
    for name, shape in [
        ("x_r", [128, NDT * T]),        # xhatT slabs: cols dt*T + t
        ("wq_r", [128, NDT * O]),       # Q pass slabs: cols p*8192 + n*256 + c
        ("wk_r", [128, NDT * O]),       # K pass slabs: same geometry
        ("wv_r", [128, NDT * O]),       # V slabs: cols oc*16384 + n*512 + c
        ("wo_r", [128, 2 * 32 * 1024]), # woT slabs: cols p*32768 + kt*1024 + d
        ("csq_cos", [128, S]),
        ("csq_sin", [128, S]),
        ("csk_cos", [128, S]),
        ("csk_sin", [128, S]),
        ("dmask", [128, 4 * 512]),      # 4 diagonal-band masks
        ("ones", [128, 128]),
    ]:
        P[name] = nc.declare_dram_parameter(name, shape, bf16, isOutput=False)
    # rows: sc*256 + b*128 + t ; cols: d within this core's d-half
    out = nc.declare_dram_parameter("out", [512, DHALF], bf16, isOutput=True)

    with tile.TileContext(nc) as tc:
        _emit(nc, tc, P, out, no_collective=no_collective)
    nc.compile()
    return nc


def _emit(nc, tc, P, out, no_collective=False):
    import concourse.mybir as mybir
    from concourse.bass import ds

    bf16 = mybir.dt.bfloat16
    fp32 = mybir.dt.float32
    AF = mybir.ActivationFunctionType

    with tc.tile_pool(name="res", bufs=1) as res, \
         tc.tile_pool(name="dram", bufs=1, space="DRAM") as dram:
        onesb = res.tile([128, 128], bf16, tag="onesb")
        dmsk = res.tile([128, 4 * 512], bf16, tag="dmsk")  # 0/1 keep masks

        # A2A staging: rows j*128 + dh (j = dest rank), cols h*128 + t
        a2a_in = [
            dram.tile([1024, 1024], bf16, tag=f"ai{sc}", name=f"a2a_in{sc}")
            for sc in range(2)
        ]
        a2a_out = [
            dram.tile([1024, 1024], bf16, tag=f"ao{sc}", name=f"a2a_out{sc}")
            for sc in range(2)
        ]

        # dmsk/onesb are loaded late in phase 1 (see below) so they don't
        # delay the startup-critical x/weight streams

        # tiny warm-up collective (fired a little into phase 1): absorbs the
        # ~11us first-collective spin-up on the CC stream during projections
        warm_in = dram.tile([8, 128], bf16, tag="wi", name="warm_in")
        warm_out = dram.tile([8, 128], bf16, tag="wo", name="warm_out")

        def fire_warmup():
            if no_collective:
                return
            nc.gpsimd.dma_start(out=warm_in[:, :], in_=P["ones"][ds(0, 8), :])
            nc.gpsimd.collective_compute(
                "AllToAll",
                mybir.AluOpType.bypass,
                replica_groups=A2A_GROUP,
                ins=[warm_in[:, :].opt()],
                outs=[warm_out[:, :].opt()],
            )

        with tc.tile_pool(name="qkv", bufs=1) as qkv:
          qT = qkv.tile([128, H * S], bf16, tag="qT")     # cols h*S + s
          kT = qkv.tile([128, H * T], bf16, tag="kT")     # cols h*T + t
          vsb = qkv.tile([128, 12 * O], bf16, tag="vsb")  # cols tt*O + o

          # ---------------- phase 1: projections + rope ----------------
          with tc.tile_pool(name="xpool", bufs=1) as xpool, \
               tc.tile_pool(name="tabpool", bufs=1) as tabpool, \
               tc.tile_pool(name="wpool", bufs=3) as wpool, \
               tc.tile_pool(name="rtmp", bufs=2) as rtmp, \
               tc.tile_pool(name="ps1", bufs=8, space="PSUM") as ps1:
            # first Q weight half-slab ahead of everything on the sync queue
            def wslab(src, off, n, name):
                wr = wpool.tile([128, n], bf16, tag="wr", name=name)
                nc.sync.dma_start(out=wr[:, :], in_=P[src][:, ds(off, n)])
                return wr

            # first Q slab in 4 chunks: the first matmuls only need the
            # first columns, so chunked arrival starts compute early
            wr_q00 = wpool.tile([128, 16 * 256], bf16, tag="wr",
                                name="wr_q0_0")
            for ch in range(4):
                nc.sync.dma_start(
                    out=wr_q00[:, ds(ch * 1024, 1024)],
                    in_=P["wq_r"][:, ds(ch * 1024, 1024)],
                )
            # x slabs: decoder columns first (all the Q pass needs), encoder
            # columns follow (first needed by the K pass ~70us in)
            xh = []
            for dt in range(NDT):
                xt = xpool.tile([128, T], bf16, tag=f"xh{dt}", name=f"xh{dt}")
                (nc.scalar if dt % 2 == 0 else nc.gpsimd).dma_start(
                    out=xt[:, ds(E, S)], in_=P["x_r"][:, ds(dt * T + E, S)]
                )
                xh.append(xt)
                if dt == 7:
                    # rope tables: q tables needed at the first rope (~35us)
                    csqc = tabpool.tile([128, S], bf16, tag="csqc")
                    csqs = tabpool.tile([128, S], bf16, tag="csqs")
                    cskc = tabpool.tile([128, S], bf16, tag="cskc")
                    csks = tabpool.tile([128, S], bf16, tag="csks")
                    nc.scalar.dma_start(out=csqc[:, :], in_=P["csq_cos"][:, :])
                    nc.gpsimd.dma_start(out=csqs[:, :], in_=P["csq_sin"][:, :])
            for dt in range(NDT):
                (nc.scalar if dt % 2 == 0 else nc.gpsimd).dma_start(
                    out=xh[dt][:, ds(0, E)], in_=P["x_r"][:, ds(dt * T, E)]
                )
                if dt == 7:
                    nc.scalar.dma_start(out=cskc[:, :], in_=P["csk_cos"][:, :])
                    nc.gpsimd.dma_start(out=csks[:, :], in_=P["csk_sin"][:, :])
            fire_warmup()
            nc.gpsimd.dma_start(out=dmsk[:, :], in_=P["dmask"][:, :])
            nc.gpsimd.dma_start(out=onesb[:, :], in_=P["ones"][:, :])

            # RoPE: tables are full-height with the 64-row block duplicated
            # (cos) or sign-split (-sin; +sin), so every TensorTensor is
            # partition-aligned. The half-swap goes through an SBUF-SBUF DMA.
            def rope(buf, base, cos, sin, tag):
                swp = rtmp.tile([128, S], bf16, tag="swp", name=f"swp_{tag}")
                nc.gpsimd.dma_start(
                    out=swp[ds(0, 64), :], in_=buf[ds(64, 64), ds(base, S)]
                )
                nc.gpsimd.dma_start(
                    out=swp[ds(64, 64), :], in_=buf[ds(0, 64), ds(base, S)]
                )
                nc.vector.tensor_mul(swp[:, :], swp[:, :], sin[:, :])
                nc.vector.tensor_mul(
                    buf[:, ds(base, S)], buf[:, ds(base, S)], cos[:, :]
                )
                nc.vector.tensor_add(
                    buf[:, ds(base, S)], buf[:, ds(base, S)], swp[:, :]
                )

            # Q: 4 passes x (2 o_tiles x 2 s_chunks); K: 4 passes x
            # (2 o_tiles x 3 t_chunks). Weight slabs stream in 16-dt halves.
            for src, nch, xoff, obuf, ostride in (
                ("wq_r", 2, E, qT, S),
                ("wk_r", 3, 0, kT, T),
            ):
                for p in range(4):
                    ps = [
                        [ps1.tile([128, 512], fp32, tag="ps1",
                                  name=f"ps_{src}_{p}_{oi}_{cc}")
                         for cc in range(nch)]
                        for oi in range(2)
                    ]
                    for half in range(2):
                        if src == "wq_r" and p == 0 and half == 0:
                            wr = wr_q00
                        else:
                            wr = wslab(
                                src, (p * 2 + half) * 16 * 256, 16 * 256,
                                f"wr_{src}_{p}_{half}",
                            )
                        for dtl in range(16):
                            dt = half * 16 + dtl
                            for oi in range(2):
                                for cc in range(nch):
                                    nc.tensor.matmul(
                                        ps[oi][cc][:, :],
                                        wr[:, ds(dtl * 256 + oi * 128, 128)],
                                        xh[dt][:, ds(xoff + cc * 512, 512)],
                                        start=(dt == 0),
                                        stop=(dt == NDT - 1),
                                    )
                    for oi in range(2):
                        h = 2 * p + oi
                        for cc in range(nch):
                            nc.scalar.copy(
                                obuf[:, ds(h * ostride + cc * 512, 512)],
                                ps[oi][cc][:, :],
                            )
                        if src == "wq_r":
                            rope(qT, h * S, csqc, csqs, f"q{h}")
                        else:
                            rope(kT, h * T + E, cskc, csks, f"k{h}")

            # V (x-stationary): 2 o_chunks x 2 t_groups of 6 tiles; weight
            # slabs re-streamed per t_group in two 16-dt halves
            for oc in range(2):
                for tg in range(2):
                    tb = tg * 6
                    psv = [ps1.tile([128, 512], fp32, tag="ps1",
                                    name=f"psv_{oc}_{tg}_{ti}")
                           for ti in range(6)]
                    for qr in range(4):
                        wr = wpool.tile(
                            [128, 8 * 512], bf16, tag="wr",
                            name=f"wr_v{oc}_{tg}_{qr}",
                        )
                        nc.sync.dma_start(
                            out=wr[:, :],
                            in_=P["wv_r"][
                                :, ds(oc * NDT * 512 + qr * 8 * 512, 8 * 512)
                            ],
                        )
                        for dtl in range(8):
                            dt = qr * 8 + dtl
                            for ti in range(6):
                                nc.tensor.matmul(
                                    psv[ti][:, :],
                                    xh[dt][:, ds((tb + ti) * 128, 128)],
                                    wr[:, ds(dtl * 512, 512)],
                                    start=(dt == 0),
                                    stop=(dt == NDT - 1),
                                )
                    for ti in range(6):
                        nc.scalar.copy(
                            vsb[:, ds((tb + ti) * O + oc * 512, 512)],
                            psv[ti][:, :],
                        )

          # wo pass-1 slabs + sc0 recv tiles prefetch during attention; this
          # pool sits in the (dead) phase-1 x region so its DMAs only wait on
          # the last projection matmul, not on attention.
          with tc.tile_pool(name="wpre", bufs=1) as wpre:
            slab1 = [
                wpre.tile([128, 1024], bf16, tag=f"sl1_{kt}", name=f"slab1_{kt}")
                for kt in range(32)
            ]
            rt0 = [
                wpre.tile([128, 1024], bf16, tag=f"rt0_{i}", name=f"rt0_{i}")
                for i in range(8)
            ]
            # stream pass-1 slabs during attention, all on gpsimd: its only
            # later work is the A2A triggers, so ring-credit waits here
            # never stall a compute-feeding queue (scalar's exp stream
            # stalled ~18us when half of these sat on it)
            for kt in range(32):
                nc.gpsimd.dma_start(
                    out=slab1[kt][:, :],
                    in_=P["wo_r"][:, ds(kt * 1024, 1024)],
                )

            # -------- phase 2: attention (softmax along partitions) --------
            # Per (sc, h): tile list = 4 encoder tiles + decoder tiles that
            # are not fully masked (sc0: 4, sc1: 8). Scores into paired psum
            # banks, exp over the pair, Z via ones-stationary matmul, PV
            # accumulation, then one reciprocal + one mul. After each head's
            # oT is ready it is staged to the A2A input (quarters duplicated
            # to ranks q and q+4); the A2A for a slice fires after its 8th
            # head.
            with tc.tile_pool(name="opool", bufs=1) as opool, \
                 tc.tile_pool(name="ppool", bufs=3) as ppool, \
                 tc.tile_pool(name="zpool", bufs=2) as zpool, \
                 tc.tile_pool(name="psS", bufs=3, space="PSUM") as psS, \
                 tc.tile_pool(name="psZ", bufs=1, space="PSUM") as psZ, \
                 tc.tile_pool(name="psV", bufs=1, space="PSUM") as psV:
              oT = opool.tile([128, H * S], bf16, tag="oT")  # cols h*S + s

              def tiles_for(sc):
                  # (tt, diag_j): tt indexes kT/vsb t-tiles; diag_j is the
                  # diagonal-mask index or None. Fully-masked tiles skipped.
                  lst = [(tt, None) for tt in range(4)]  # encoder
                  if sc == 0:
                      lst += [(4 + j, j) for j in range(4)]
                  else:
                      lst += [(tt, None) for tt in range(4, 8)]
                      lst += [(8 + j, j) for j in range(4)]
                  return lst

              def emit_A(sc, h, pbuf):
                  tl = tiles_for(sc)
                  for k0 in range(0, len(tl), 2):
                      pr = psS.tile([128, 1024], fp32, tag="psS",
                                    name=f"sc{sc}h{h}p{k0}")
                      for half in range(2):
                          tt, dj = tl[k0 + half]
                          nc.tensor.matmul(
                              pr[:, ds(half * 512, 512)],
                              kT[:, ds(h * T + tt * 128, 128)],
                              qT[:, ds(h * S + sc * 512, 512)],
                              start=True,
                              stop=True,
                          )
                      nc.scalar.activation(
                          pbuf[:, ds(k0 * 512, 1024)], pr[:, :], AF.Exp
                      )
                      # causal zeroing of the diagonal-band tiles, post-exp
                      for half in range(2):
                          tt, dj = tl[k0 + half]
                          if dj is not None:
                              nc.vector.tensor_mul(
                                  pbuf[:, ds((k0 + half) * 512, 512)],
                                  pbuf[:, ds((k0 + half) * 512, 512)],
                                  dmsk[:, ds(dj * 512, 512)],
                              )

              def emit_B(sc, h, pbuf):
                  tl = tiles_for(sc)
                  n = len(tl)
                  zp = psZ.tile([128, 512], fp32, tag="psZ", name=f"z{sc}{h}")
                  for k, (tt, _) in enumerate(tl):
                      nc.tensor.matmul(
                          zp[:, :],
                          onesb[:, :],
                          pbuf[:, ds(k * 512, 512)],
                          start=(k == 0),
                          stop=(k == n - 1),
                      )
                  zr = zpool.tile([128, 512], fp32, tag="zr", name=f"zr{sc}{h}")
                  nc.vector.reciprocal_approx_fast(zr[:, :], zp[:, :])
                  pv = psV.tile([128, 512], fp32, tag="psV", name=f"pv{sc}{h}")
                  for k, (tt, _) in enumerate(tl):
                      nc.tensor.matmul(
                          pv[:, :],
                          vsb[:, ds(tt * O + h * 128, 128)],
                          pbuf[:, ds(k * 512, 512)],
                          start=(k == 0),
                          stop=(k == n - 1),
                      )
                  nc.vector.tensor_mul(
                      oT[:, ds(h * S + sc * 512, 512)], pv[:, :], zr[:, :]
                  )
                  # stage this head's slice into the A2A input: token quarter
                  # q goes to dest-rank rows q*128 (batch-group 0 dests) and
                  # (4+q)*128 (group 1); one 3-dim DMA per dest group
                  src = oT[:, ds(h * S + sc * 512, 512)].rearrange(
                      "dh (q t) -> dh q t", q=4
                  )
                  for g in range(2):
                      dst = a2a_in[sc][
                          ds(g * 512, 512), ds(h * 128, 128)
                      ].rearrange("(q dh) t -> dh q t", q=4)
                      nc.sync.dma_start(out=dst, in_=src)

              def fire_a2a(sc):
                  if no_collective:
                      nc.gpsimd.dma_start(
                          out=a2a_out[sc][:, :], in_=a2a_in[sc][:, :]
                      )
                  else:
                      nc.gpsimd.collective_compute(
                          "AllToAll",
                          mybir.AluOpType.bypass,
                          replica_groups=A2A_GROUP,
                          ins=[a2a_in[sc][:, :].opt()],
                          outs=[a2a_out[sc][:, :].opt()],
                      )

              # software pipeline: 2-head lookahead on A emits
              pend = []
              b_count = 0
              for sc in range(2):
                  for h in range(H):
                      pbuf = ppool.tile(
                          [128, 12 * 512], bf16, tag="p", name=f"pb{sc}{h}"
                      )
                      emit_A(sc, h, pbuf)
                      pend.append((sc, h, pbuf))
                      if len(pend) == 3:
                          s0, h0, pb0 = pend.pop(0)
                          emit_B(s0, h0, pb0)
                          b_count += 1
                          if b_count == 8:
                              fire_a2a(0)
              for s0, h0, pb0 in pend:
                  emit_B(s0, h0, pb0)
                  b_count += 1
              # A2A#2 first: the gpsimd queue sits in the A2A#1 completion
              # wait, and a collective op also waits for completion, so any
              # load placed between the two triggers would delay the second
              # trigger past its own transfer time.
              fire_a2a(1)
              # sc0 recv tiles on sync (free once staging ends), in wo
              # consumption order so the first k-tiles arrive first
              for i in (0, 4, 1, 5, 2, 6, 3, 7):
                  nc.sync.dma_start(
                      out=rt0[i][:, :],
                      in_=a2a_out[0][ds(i * 128, 128), :],
                  )

            # ---------------- phase 3: wo (y = oT_full.T @ woT) ----------
            # pass 1: d-chunks 0-1 (slabs resident from prefetch); sc0 first
            # (independent of A2A#2, hides it), then sc1. pass 2: d-chunks
            # 2-3 with freshly streamed slabs, sc0+sc1 jointly.
            with tc.tile_pool(name="wo2", bufs=8) as wo2, \
                 tc.tile_pool(name="rt1p", bufs=1) as rt1p, \
                 tc.tile_pool(name="ypool", bufs=1) as ypool, \
                 tc.tile_pool(name="psW", bufs=8, space="PSUM") as psW:
                rt1 = [
                    rt1p.tile([128, 1024], bf16, tag=f"rt1_{i}",
                              name=f"rt1_{i}")
                    for i in range(8)
                ]
                # sc1 recv: in consumption order (kt walks ig with both
                # batches), split over gpsimd (parked right behind the
                # A2A#2 completion wait) and sync (idle after staging)
                for k, i in enumerate((0, 4, 1, 5, 2, 6, 3, 7)):
                    (nc.gpsimd if k % 2 == 0 else nc.sync).dma_start(
                        out=rt1[i][:, :], in_=a2a_out[1][ds(i * 128, 128), :]
                    )
                rts = [rt0, rt1]
                yt = {
                    (sc, b): ypool.tile(
                        [128, DHALF], bf16, tag=f"y{sc}{b}", name=f"y{sc}{b}"
                    )
                    for sc in range(2) for b in range(2)
                }

                def wo_block(sc, dcs, slabs):
                    # psum[t, d] accumulated over all 32 k-tiles
                    pw = {
                        (b, dc): psW.tile([128, 512], fp32, tag="psW",
                                          name=f"pw{sc}{b}{dc}")
                        for b in range(2) for dc in dcs
                    }
                    for kt in range(32):
                        ig, h = divmod(kt, 8)
                        for b in range(2):
                            stat = rts[sc][b * 4 + ig][:, ds(h * 128, 128)]
                            for dc in dcs:
                                nc.tensor.matmul(
                                    pw[(b, dc)][:, :],
                                    stat,
                                    slabs[kt][:, ds((dc % 2) * 512, 512)],
                                    start=(kt == 0),
                                    stop=(kt == 31),
                                )
                    for b in range(2):
                        for dc in dcs:
                            nc.scalar.copy(
                                yt[(sc, b)][:, ds(dc * 512, 512)],
                                pw[(b, dc)][:, :],
                            )

                # pass 1 (resident slabs): sc0 then sc1
                wo_block(0, (0, 1), slab1)
                wo_block(1, (0, 1), slab1)
                # pass 2: stream the other d-half of each slab; one JOINT
                # kt loop over both slices so the rotating slab slots are
                # fully consumed before their reuse (no FIFO inversion)
                pw2 = {
                    (sc, b, dc): psW.tile([128, 512], fp32, tag="psW",
                                          name=f"p2w{sc}{b}{dc}")
                    for sc in range(2) for b in range(2) for dc in (2, 3)
                }
                for kt in range(32):
                    sl = wo2.tile([128, 1024], bf16, tag="sl2",
                                  name=f"slab2_{kt}")
                    # gpsimd/sync are both parked behind the A2A#2
                    # completion here, so these 8MB of transfers cannot
                    # contend with the collective itself
                    (nc.gpsimd if kt % 2 == 0 else nc.sync).dma_start(
                        out=sl[:, :],
                        in_=P["wo_r"][:, ds(32768 + kt * 1024, 1024)],
                    )
                    ig, h = divmod(kt, 8)
                    for sc in range(2):
                        for b in range(2):
                            stat = rts[sc][b * 4 + ig][:, ds(h * 128, 128)]
                            for dc in (2, 3):
                                nc.tensor.matmul(
                                    pw2[(sc, b, dc)][:, :],
                                    stat,
                                    sl[:, ds((dc % 2) * 512, 512)],
                                    start=(kt == 0),
                                    stop=(kt == 31),
                                )
                for sc in range(2):
                    for b in range(2):
                        for dc in (2, 3):
                            nc.scalar.copy(
                                yt[(sc, b)][:, ds(dc * 512, 512)],
                                pw2[(sc, b, dc)][:, :],
                            )
                        nc.sync.dma_start(
                            out=out[ds(sc * 256 + b * 128, 128), :],
                            in_=yt[(sc, b)][:, :],
                        )


def _prep_in_maps(x, freqs_cos, freqs_sin, mask, encoder_output, wq, wk, wv, wo):
    x = np.asarray(x, np.float32)
    encoder_output = np.asarray(encoder_output, np.float32)
    freqs_cos = np.asarray(freqs_cos, np.float32)
    freqs_sin = np.asarray(freqs_sin, np.float32)
    wq = np.asarray(wq, np.float32)
    wk = np.asarray(wk, np.float32)
    wv = np.asarray(wv, np.float32)
    wo = np.asarray(wo, np.float32)

    def perm(w):  # deinterleave rope pairs per head: even dims first
        w4 = w.reshape(H, 64, 2, D)
        return np.ascontiguousarray(w4.transpose(0, 2, 1, 3)).reshape(O, D)

    def slab256(wT):  # [D, O] -> [128, 4*32*256]: pass p, dt n, col c
        w4 = wT.reshape(NDT, 128, 4, 256)            # [n, part, p, c]
        return np.ascontiguousarray(
            w4.transpose(1, 2, 0, 3)
        ).reshape(128, NDT * O)

    def slab512(wT):  # [D, O] -> [128, 2*32*512]: oc, dt n, col c
        w4 = wT.reshape(NDT, 128, 2, 512)
        return np.ascontiguousarray(
            w4.transpose(1, 2, 0, 3)
        ).reshape(128, NDT * O)

    alpha = 1.0 / np.sqrt(DH)
    cosT = freqs_cos.T  # [64, S]
    sinT = freqs_sin.T
    csq_cos = (np.concatenate([cosT, cosT], 0) * alpha).astype(BF16)
    csq_sin = (np.concatenate([-sinT, sinT], 0) * alpha).astype(BF16)
    csk_cos = np.concatenate([cosT, cosT], 0).astype(BF16)
    csk_sin = np.concatenate([-sinT, sinT], 0).astype(BF16)

    # 4 diagonal-band keep-masks (0/1, applied post-exp):
    # dmask[t, j*512+s] = 0 if s < t + j*128 else 1
    t_i = np.arange(128)[:, None]
    s_i = np.arange(512)[None, :]
    dmask = np.concatenate(
        [np.where(s_i < t_i + j * 128, 0.0, 1.0) for j in range(4)], axis=1
    ).astype(BF16)
    ones = np.ones((128, 128), BF16)

    # woT slabs: full wo.T (k = head*128+dh on partitions per k-tile), this
    # core's d-half, split into two 1024-wide passes
    woT = np.ascontiguousarray(wo.T).reshape(32, 128, D)  # [kt, dh, dout]

    in_maps = []
    for c in range(8):
        g, r = divmod(c, 4)
        dhalf = g
        sl = slice(r * O, (r + 1) * O)
        xhat = np.concatenate([encoder_output[g], x[g]], axis=0)  # [T, D]
        xhatT = xhat.T.astype(BF16)                               # [D, T]
        x_r = np.ascontiguousarray(
            xhatT.reshape(NDT, 128, T).transpose(1, 0, 2)
        ).reshape(128, NDT * T)
        wqT = perm(wq[sl]).T.astype(BF16)   # [D, O]
        wkT = perm(wk[sl]).T.astype(BF16)
        wvT = wv[sl].T.astype(BF16)
        wo_c = woT[:, :, dhalf * DHALF:(dhalf + 1) * DHALF]  # [32,128,2048]
        wo_r = np.ascontiguousarray(
            wo_c.reshape(32, 128, 2, 1024).transpose(1, 2, 0, 3)
        ).reshape(128, 2 * 32 * 1024).astype(BF16)
        in_maps.append(
            {
                "x_r": x_r,
                "wq_r": slab256(wqT),
                "wk_r": slab256(wkT),
                "wv_r": slab512(wvT),
                "wo_r": wo_r,
                "csq_cos": csq_cos,
                "csq_sin": csq_sin,
                "csk_cos": csk_cos,
                "csk_sin": csk_sin,
                "dmask": dmask,
                "ones": ones,
            }
        )
    return in_maps


def _gather(outs):
    full = np.zeros((2, S, D), np.float32)
    for c in range(8):
        g, q = divmod(c, 4)
        dhalf = g
        o = np.asarray(outs[c]).astype(np.float32)  # [512, 2048]
        for sc in range(2):
            for b in range(2):
                rows = o[sc * 256 + b * 128: sc * 256 + b * 128 + 128]
                full[b, sc * 512 + q * 128: sc * 512 + q * 128 + 128,
                     dhalf * DHALF:(dhalf + 1) * DHALF] = rows
    return full


def kernel(x, start_pos, freqs_cos, freqs_sin, mask, encoder_output, wq, wk, wv, wo):
    global LAST_EXEC_NS
    from concourse.bass_utils import run_bass_kernel_spmd

    if "nc" not in _CACHE:
        _CACHE["nc"] = _build()
    nc = _CACHE["nc"]

    in_maps = _prep_in_maps(
        x, freqs_cos, freqs_sin, mask, encoder_output, wq, wk, wv, wo
    )
    res = run_bass_kernel_spmd(nc, in_maps, core_ids=list(range(8)))
    LAST_EXEC_NS = res.exec_time_ns
    return _gather([res.results[c]["out"] for c in range(8)])


# revision 24
# speedup vs baseline: 1.0595x; 1.0028x over previous
"""Trainium2 Bass kernel for nn_Attention (dense transformer attention block).

Full inputs -> full output. Internally: 8 NeuronCores, 2 data-parallel groups
(batch) x 4-way tensor-parallel (heads). Each core computes 8 heads for one
batch element. The wo projection is redistributed with a single 8-rank
AllToAll per 512-token slice: each core ships its heads' attention output
(oT, feature-major) for token-quarter q to ranks q and q+4, and afterwards
holds the FULL 4096-feature oT for one 128-token quarter of each batch --
it then computes y for those rows over one 2048-wide d-half (group 0 takes
d 0:2048, group 1 d 2048:4096). No ReduceScatter; the A2A moves 2 MB/rank
at ~24 us (mesh) and the sc1 A2A hides under the sc0 wo pass.

Compute in bf16 on the TensorEngine (fp32 matmul is 4x slower), fp32 PSUM
accumulation. All operand layouts are pre-rearranged host-side so every
device DMA is a contiguous per-partition block:
  - projections:  qT/kT = (w-tile).T @ xhatT-tile   -> [feature, seq] layout
  - scores:       scoresT[t, s] = kT-tile.T @ qT    (softmax along partitions)
  - Z:            ones[128,128].T @ probs           -> Z broadcast to all rows
  - PV:           oT[dh, s] = v-tile.T @ probsT
  - wo:           y[t, d] = oT-recv-tile.T @ woT-slab (K=4096 in one psum)
Causal structure is exploited: score tiles that are fully masked are skipped
(scores/exp/Z/PV), and only the 4 diagonal-band tiles per query chunk get a
post-exp 0/1 multiply (from 4 precomputed [128,512] masks). RoPE pairs are
deinterleaved host-side (even dims first); the 1/sqrt(128) score scale is
folded into the q-side cos/sin tables.
"""

import sys

import numpy as np

for _p in ("/opt/trn_rl_repo",):
    if _p not in sys.path:
        sys.path.insert(0, _p)

import ml_dtypes

BF16 = ml_dtypes.bfloat16

D = 4096      # model dim
S = 1024      # decoder sequence length
E = 512       # encoder length
T = E + S     # total key length
H = 8         # heads per core (32 total / 4-way TP)
DH = 128      # head dim
O = H * DH    # per-core projection width = 1024
NDT = D // 128
NEG = -1e9
A2A_GROUP = [[0, 1, 2, 3, 4, 5, 6, 7]]
DHALF = 2048  # per-core output d-half width

_CACHE = {}
LAST_EXEC_NS = None


def _build(no_collective=False):
    import concourse.mybir as mybir
    import concourse.tile as tile
    from concourse import bacc

    bf16 = mybir.dt.bfloat16

    nc = bacc.Bacc(
        "TRN2",
        target_bir_lowering=False,
        debug=False,
        num_devices=8,
    )

    P = {}
    for name, shape in [
        ("x_r", [128, NDT * T]),        # xhatT slabs: cols dt*T + t
        ("wq_r", [128, NDT * O]),       # Q pass slabs: cols p*8192 + n*256 + c
        ("wk_r", [128, NDT * O]),       # K pass slabs: same geometry
        ("wv_r", [128, NDT * O]),       # V slabs: cols oc*16384 + n*512 + c
        ("wo_r", [128, 2 * 32 * 1024]), # woT slabs: cols p*32768 + kt*1024 + d
        ("csq_cos", [128, S]),
        ("csq_sin", [128, S]),
        ("csk_cos", [128, S]),
        ("csk_sin", [128, S]),
        ("dmask", [128, 4 * 512]),      # 4 diagonal-band masks
        ("ones", [128, 128]),
    ]:
        P[name] = nc.declare_dram_parameter(name, shape, bf16, isOutput=False)
    # rows: sc*256 + b*128 + t ; cols: d within this core's d-half
    out = nc.declare_dram_parameter("out", [512, DHALF], bf16, isOutput=True)

    with tile.TileContext(nc) as tc:
        _emit(nc, tc, P, out, no_collective=no_collective)
    nc.compile()
    return nc


def _emit(nc, tc, P, out, no_collective=False):
    import concourse.mybir as mybir
    from concourse.bass import ds

    bf16 = mybir.dt.bfloat16
    fp32 = mybir.dt.float32
    AF = mybir.ActivationFunctionType

    with tc.tile_pool(name="res", bufs=1) as res, \
         tc.tile_pool(name="dram", bufs=1, space="DRAM") as dram:
        onesb = res.tile([128, 128], bf16, tag="onesb")
        dmsk = res.tile([128, 4 * 512], bf16, tag="dmsk")  # 0/1 keep masks

        # A2A staging: rows j*128 + dh (j = dest rank), cols h*128 + t
        a2a_in = [
            dram.tile([1024, 1024], bf16, tag=f"ai{sc}", name=f"a2a_in{sc}")
            for sc in range(2)
        ]
        a2a_out = [
            dram.tile([1024, 1024], bf16, tag=f"ao{sc}", name=f"a2a_out{sc}")
            for sc in range(2)
        ]

        # dmsk/onesb are loaded late in phase 1 (see below) so they don't
        # delay the startup-critical x/weight streams

        # tiny warm-up collective (fired a little into phase 1): absorbs the
        # ~11us first-collective spin-up on the CC stream during projections
        warm_in = dram.tile([8, 128], bf16, tag="wi", name="warm_in")
        warm_out = dram.tile([8, 128], bf16, tag="wo", name="warm_out")

        def fire_warmup():
            if no_collective:
                return
            nc.gpsimd.dma_start(out=warm_in[:, :], in_=P["ones"][ds(0, 8), :])
            nc.gpsimd.collective_compute(
                "AllToAll",
                mybir.AluOpType.bypass,
                replica_groups=A2A_GROUP,
                ins=[warm_in[:, :].opt()],
                outs=[warm_out[:, :].opt()],
            )

        with tc.tile_pool(name="qkv", bufs=1) as qkv:
          qT = qkv.tile([128, H * S], bf16, tag="qT")     # cols h*S + s
          kT = qkv.tile([128, H * T], bf16, tag="kT")     # cols h*T + t
          vsb = qkv.tile([128, 12 * O], bf16, tag="vsb")  # cols tt*O + o

          # ---------------- phase 1: projections + rope ----------------
          with tc.tile_pool(name="xpool", bufs=1) as xpool, \
               tc.tile_pool(name="tabpool", bufs=1) as tabpool, \
               tc.tile_pool(name="wpool", bufs=4) as wpool, \
               tc.tile_pool(name="rtmp", bufs=2) as rtmp, \
               tc.tile_pool(name="ps1", bufs=8, space="PSUM") as ps1:
            # first Q weight half-slab ahead of everything on the sync queue
            def wslab(src, off, n, name):
                wr = wpool.tile([128, n], bf16, tag="wr", name=name)
                nc.sync.dma_start(out=wr[:, :], in_=P[src][:, ds(off, n)])
                return wr

            # first two Q slabs (p0/p1 half 0) in chunks: the first matmuls
            # only need the first columns, so chunked arrival starts
            # compute early
            wr_q0 = {}
            for pi in range(2):
                wr = wpool.tile([128, 16 * 256], bf16, tag="wr",
                                name=f"wr_q{pi}_0")
                for ch in range(2):
                    nc.sync.dma_start(
                        out=wr[:, ds(ch * 2048, 2048)],
                        in_=P["wq_r"][:, ds(pi * 8192 + ch * 2048, 2048)],
                    )
                wr_q0[pi] = wr
            # x slabs: decoder columns first (all the Q pass needs), encoder
            # columns follow (first needed by the K pass ~70us in)
            xh = []
            for dt in range(NDT):
                xt = xpool.tile([128, T], bf16, tag=f"xh{dt}", name=f"xh{dt}")
                (nc.scalar if dt % 2 == 0 else nc.gpsimd).dma_start(
                    out=xt[:, ds(E, S)], in_=P["x_r"][:, ds(dt * T + E, S)]
                )
                xh.append(xt)
                if dt == 7:
                    # rope tables: q tables needed at the first rope (~35us);
                    # the k tables rotate into the same slots later (the
                    # slot WAR makes them wait for the last q rope)
                    csqc = tabpool.tile([128, S], bf16, tag="csc",
                                        name="csqc")
                    csqs = tabpool.tile([128, S], bf16, tag="css",
                                        name="csqs")
                    nc.scalar.dma_start(out=csqc[:, :], in_=P["csq_cos"][:, :])
                    nc.gpsimd.dma_start(out=csqs[:, :], in_=P["csq_sin"][:, :])
            for dt in range(NDT):
                (nc.scalar if dt % 2 == 0 else nc.gpsimd).dma_start(
                    out=xh[dt][:, ds(0, E)], in_=P["x_r"][:, ds(dt * T, E)]
                )
            fire_warmup()
            nc.gpsimd.dma_start(out=dmsk[:, :], in_=P["dmask"][:, :])
            nc.gpsimd.dma_start(out=onesb[:, :], in_=P["ones"][:, :])

            # RoPE: tables are full-height with the 64-row block duplicated
            # (cos) or sign-split (-sin; +sin), so every TensorTensor is
            # partition-aligned. The half-swap goes through an SBUF-SBUF DMA.
            def rope(buf, base, cos, sin, tag):
                swp = rtmp.tile([128, S], bf16, tag="swp", name=f"swp_{tag}")
                nc.gpsimd.dma_start(
                    out=swp[ds(0, 64), :], in_=buf[ds(64, 64), ds(base, S)]
                )
                nc.gpsimd.dma_start(
                    out=swp[ds(64, 64), :], in_=buf[ds(0, 64), ds(base, S)]
                )
                nc.vector.tensor_mul(swp[:, :], swp[:, :], sin[:, :])
                nc.vector.tensor_mul(
                    buf[:, ds(base, S)], buf[:, ds(base, S)], cos[:, :]
                )
                nc.vector.tensor_add(
                    buf[:, ds(base, S)], buf[:, ds(base, S)], swp[:, :]
                )

            # Q: two super-passes of two weight-passes each (8 psum banks),
            # dt OUTERMOST so x-slab consumption spreads over the whole
            # super-pass instead of demanding all 32 slabs in 34us
            for sp in range(2):
                ps = [
                    [
                        [ps1.tile([128, 512], fp32, tag="ps1",
                                  name=f"ps_q_{sp}_{pi}_{oi}_{cc}")
                         for cc in range(2)]
                        for oi in range(2)
                    ]
                    for pi in range(2)
                ]
                wrs = {}
                for half in range(2):
                    for pi in range(2):
                        p = 2 * sp + pi
                        if sp == 0 and half == 0:
                            wrs[(pi, 0)] = wr_q0[pi]
                        else:
                            wrs[(pi, half)] = wslab(
                                "wq_r", (p * 2 + half) * 16 * 256, 16 * 256,
                                f"wr_q{p}_{half}",
                            )
                for dt in range(NDT):
                    half, dtl = divmod(dt, 16)
                    for pi in range(2):
                        for oi in range(2):
                            for cc in range(2):
                                nc.tensor.matmul(
                                    ps[pi][oi][cc][:, :],
                                    wrs[(pi, half)][
                                        :, ds(dtl * 256 + oi * 128, 128)
                                    ],
                                    xh[dt][:, ds(E + cc * 512, 512)],
                                    start=(dt == 0),
                                    stop=(dt == NDT - 1),
                                )
                for pi in range(2):
                    p = 2 * sp + pi
                    for oi in range(2):
                        h = 2 * p + oi
                        for cc in range(2):
                            nc.scalar.copy(
                                qT[:, ds(h * S + cc * 512, 512)],
                                ps[pi][oi][cc][:, :],
                            )
                        rope(qT, h * S, csqc, csqs, f"q{h}")

            # k rope tables rotate into the q tables' slots; emitted here so
            # their slot-WAR wait (last q rope) sits on an idle queue moment
            cskc = tabpool.tile([128, S], bf16, tag="csc", name="cskc")
            csks = tabpool.tile([128, S], bf16, tag="css", name="csks")
            nc.scalar.dma_start(out=cskc[:, :], in_=P["csk_cos"][:, :])
            nc.gpsimd.dma_start(out=csks[:, :], in_=P["csk_sin"][:, :])

            # K: 4 passes x (2 o_tiles x 3 t_chunks), slabs in 16-dt halves
            for p in range(4):
                ps = [
                    [ps1.tile([128, 512], fp32, tag="ps1",
                              name=f"ps_k_{p}_{oi}_{cc}")
                     for cc in range(3)]
                    for oi in range(2)
                ]
                for half in range(2):
                    wr = wslab(
                        "wk_r", (p * 2 + half) * 16 * 256, 16 * 256,
                        f"wr_k{p}_{half}",
                    )
                    for dtl in range(16):
                        dt = half * 16 + dtl
                        for oi in range(2):
                            for cc in range(3):
                                nc.tensor.matmul(
                                    ps[oi][cc][:, :],
                                    wr[:, ds(dtl * 256 + oi * 128, 128)],
                                    xh[dt][:, ds(cc * 512, 512)],
                                    start=(dt == 0),
                                    stop=(dt == NDT - 1),
                                )
                for oi in range(2):
                    h = 2 * p + oi
                    for cc in range(3):
                        nc.scalar.copy(
                            kT[:, ds(h * T + cc * 512, 512)],
                            ps[oi][cc][:, :],
                        )
                    rope(kT, h * T + E, cskc, csks, f"k{h}")

            # V (x-stationary): 2 o_chunks x 2 t_groups of 6 tiles; weight
            # slabs re-streamed per t_group in two 16-dt halves
            for oc in range(2):
                for tg in range(2):
                    tb = tg * 6
                    psv = [ps1.tile([128, 512], fp32, tag="ps1",
                                    name=f"psv_{oc}_{tg}_{ti}")
                           for ti in range(6)]
                    for qr in range(4):
                        wr = wpool.tile(
                            [128, 8 * 512], bf16, tag="wr",
                            name=f"wr_v{oc}_{tg}_{qr}",
                        )
                        nc.sync.dma_start(
                            out=wr[:, :],
                            in_=P["wv_r"][
                                :, ds(oc * NDT * 512 + qr * 8 * 512, 8 * 512)
                            ],
                        )
                        for dtl in range(8):
                            dt = qr * 8 + dtl
                            for ti in range(6):
                                nc.tensor.matmul(
                                    psv[ti][:, :],
                                    xh[dt][:, ds((tb + ti) * 128, 128)],
                                    wr[:, ds(dtl * 512, 512)],
                                    start=(dt == 0),
                                    stop=(dt == NDT - 1),
                                )
                    for ti in range(6):
                        nc.scalar.copy(
                            vsb[:, ds((tb + ti) * O + oc * 512, 512)],
                            psv[ti][:, :],
                        )

          # wo pass-1 slabs + sc0 recv tiles prefetch during attention; this
          # pool sits in the (dead) phase-1 x region so its DMAs only wait on
          # the last projection matmul, not on attention.
          with tc.tile_pool(name="wpre", bufs=1) as wpre:
            slab1 = [
                wpre.tile([128, 1024], bf16, tag=f"sl1_{kt}", name=f"slab1_{kt}")
                for kt in range(32)
            ]
            rt0 = [
                wpre.tile([128, 1024], bf16, tag=f"rt0_{i}", name=f"rt0_{i}")
                for i in range(8)
            ]
            # stream pass-1 slabs during attention, all on gpsimd: its only
            # later work is the A2A triggers, so ring-credit waits here
            # never stall a compute-feeding queue (scalar's exp stream
            # stalled ~18us when half of these sat on it)
            for kt in range(32):
                nc.gpsimd.dma_start(
                    out=slab1[kt][:, :],
                    in_=P["wo_r"][:, ds(kt * 1024, 1024)],
                )

            # -------- phase 2: attention (softmax along partitions) --------
            # Per (sc, h): tile list = 4 encoder tiles + decoder tiles that
            # are not fully masked (sc0: 4, sc1: 8). Scores into paired psum
            # banks, exp over the pair, Z via ones-stationary matmul, PV
            # accumulation, then one reciprocal + one mul. After each head's
            # oT is ready it is staged to the A2A input (quarters duplicated
            # to ranks q and q+4); the A2A for a slice fires after its 8th
            # head.
            with tc.tile_pool(name="opool", bufs=1) as opool, \
                 tc.tile_pool(name="ppool", bufs=3) as ppool, \
                 tc.tile_pool(name="zpool", bufs=2) as zpool, \
                 tc.tile_pool(name="psS", bufs=3, space="PSUM") as psS, \
                 tc.tile_pool(name="psZ", bufs=1, space="PSUM") as psZ, \
                 tc.tile_pool(name="psV", bufs=1, space="PSUM") as psV:
              oT = opool.tile([128, H * S], bf16, tag="oT")  # cols h*S + s

              def tiles_for(sc):
                  # (tt, diag_j): tt indexes kT/vsb t-tiles; diag_j is the
                  # diagonal-mask index or None. Fully-masked tiles skipped.
                  lst = [(tt, None) for tt in range(4)]  # encoder
                  if sc == 0:
                      lst += [(4 + j, j) for j in range(4)]
                  else:
                      lst += [(tt, None) for tt in range(4, 8)]
                      lst += [(8 + j, j) for j in range(4)]
                  return lst

              def emit_A(sc, h, pbuf):
                  tl = tiles_for(sc)
                  for k0 in range(0, len(tl), 2):
                      pr = psS.tile([128, 1024], fp32, tag="psS",
                                    name=f"sc{sc}h{h}p{k0}")
                      for half in range(2):
                          tt, dj = tl[k0 + half]
                          nc.tensor.matmul(
                              pr[:, ds(half * 512, 512)],
                              kT[:, ds(h * T + tt * 128, 128)],
                              qT[:, ds(h * S + sc * 512, 512)],
                              start=True,
                              stop=True,
                          )
                      nc.scalar.activation(
                          pbuf[:, ds(k0 * 512, 1024)], pr[:, :], AF.Exp
                      )
                      # causal zeroing of the diagonal-band tiles, post-exp
                      for half in range(2):
                          tt, dj = tl[k0 + half]
                          if dj is not None:
                              nc.vector.tensor_mul(
                                  pbuf[:, ds((k0 + half) * 512, 512)],
                                  pbuf[:, ds((k0 + half) * 512, 512)],
                                  dmsk[:, ds(dj * 512, 512)],
                              )

              def emit_B(sc, h, pbuf):
                  tl = tiles_for(sc)
                  n = len(tl)
                  zp = psZ.tile([128, 512], fp32, tag="psZ", name=f"z{sc}{h}")
                  for k, (tt, _) in enumerate(tl):
                      nc.tensor.matmul(
                          zp[:, :],
                          onesb[:, :],
                          pbuf[:, ds(k * 512, 512)],
                          start=(k == 0),
                          stop=(k == n - 1),
                      )
                  zr = zpool.tile([128, 512], fp32, tag="zr", name=f"zr{sc}{h}")
                  nc.vector.reciprocal_approx_fast(zr[:, :], zp[:, :])
                  pv = psV.tile([128, 512], fp32, tag="psV", name=f"pv{sc}{h}")
                  for k, (tt, _) in enumerate(tl):
                      nc.tensor.matmul(
                          pv[:, :],
                          vsb[:, ds(tt * O + h * 128, 128)],
                          pbuf[:, ds(k * 512, 512)],
                          start=(k == 0),
                          stop=(k == n - 1),
                      )
                  nc.vector.tensor_mul(
                      oT[:, ds(h * S + sc * 512, 512)], pv[:, :], zr[:, :]
                  )
                  # stage this head's slice into the A2A input: token quarter
                  # q goes to dest-rank rows q*128 (batch-group 0 dests) and
                  # (4+q)*128 (group 1); one 3-dim DMA per dest group
                  src = oT[:, ds(h * S + sc * 512, 512)].rearrange(
                      "dh (q t) -> dh q t", q=4
                  )
                  for g in range(2):
                      dst = a2a_in[sc][
                          ds(g * 512, 512), ds(h * 128, 128)
                      ].rearrange("(q dh) t -> dh q t", q=4)
                      nc.sync.dma_start(out=dst, in_=src)

              def fire_a2a(sc):
                  if no_collective:
                      nc.gpsimd.dma_start(
                          out=a2a_out[sc][:, :], in_=a2a_in[sc][:, :]
                      )
                  else:
                      nc.gpsimd.collective_compute(
                          "AllToAll",
                          mybir.AluOpType.bypass,
                          replica_groups=A2A_GROUP,
                          ins=[a2a_in[sc][:, :].opt()],
                          outs=[a2a_out[sc][:, :].opt()],
                      )

              # software pipeline: 2-head lookahead on A emits
              pend = []
              b_count = 0
              for sc in range(2):
                  for h in range(H):
                      pbuf = ppool.tile(
                          [128, 12 * 512], bf16, tag="p", name=f"pb{sc}{h}"
                      )
                      emit_A(sc, h, pbuf)
                      pend.append((sc, h, pbuf))
                      if len(pend) == 3:
                          s0, h0, pb0 = pend.pop(0)
                          emit_B(s0, h0, pb0)
                          b_count += 1
                          if b_count == 8:
                              fire_a2a(0)
              for s0, h0, pb0 in pend:
                  emit_B(s0, h0, pb0)
                  b_count += 1
              # A2A#2 first: the gpsimd queue sits in the A2A#1 completion
              # wait, and a collective op also waits for completion, so any
              # load placed between the two triggers would delay the second
              # trigger past its own transfer time.
              fire_a2a(1)
              # sc0 recv tiles on sync (free once staging ends), in wo
              # consumption order so the first k-tiles arrive first
              for i in (0, 4, 1, 5, 2, 6, 3, 7):
                  nc.sync.dma_start(
                      out=rt0[i][:, :],
                      in_=a2a_out[0][ds(i * 128, 128), :],
                  )

            # ---------------- phase 3: wo (y = oT_full.T @ woT) ----------
            # pass 1: d-chunks 0-1 (slabs resident from prefetch); sc0 first
            # (independent of A2A#2, hides it), then sc1. pass 2: d-chunks
            # 2-3 with freshly streamed slabs, sc0+sc1 jointly.
            with tc.tile_pool(name="wo2", bufs=8) as wo2, \
                 tc.tile_pool(name="rt1p", bufs=1) as rt1p, \
                 tc.tile_pool(name="ypool", bufs=1) as ypool, \
                 tc.tile_pool(name="psW", bufs=8, space="PSUM") as psW:
                rt1 = [
                    rt1p.tile([128, 1024], bf16, tag=f"rt1_{i}",
                              name=f"rt1_{i}")
                    for i in range(8)
                ]
                # sc1 recv: in consumption order (kt walks ig with both
                # batches), split over gpsimd (parked right behind the
                # A2A#2 completion wait) and sync (idle after staging)
                for k, i in enumerate((0, 4, 1, 5, 2, 6, 3, 7)):
                    (nc.gpsimd if k % 2 == 0 else nc.sync).dma_start(
                        out=rt1[i][:, :], in_=a2a_out[1][ds(i * 128, 128), :]
                    )
                rts = [rt0, rt1]
                yt = {
                    (sc, b): ypool.tile(
                        [128, DHALF], bf16, tag=f"y{sc}{b}", name=f"y{sc}{b}"
                    )
                    for sc in range(2) for b in range(2)
                }

                def wo_block(sc, dcs, slabs):
                    # psum[t, d] accumulated over all 32 k-tiles
                    pw = {
                        (b, dc): psW.tile([128, 512], fp32, tag="psW",
                                          name=f"pw{sc}{b}{dc}")
                        for b in range(2) for dc in dcs
                    }
                    for kt in range(32):
                        ig, h = divmod(kt, 8)
                        for b in range(2):
                            stat = rts[sc][b * 4 + ig][:, ds(h * 128, 128)]
                            for dc in dcs:
                                nc.tensor.matmul(
                                    pw[(b, dc)][:, :],
                                    stat,
                                    slabs[kt][:, ds((dc % 2) * 512, 512)],
                                    start=(kt == 0),
                                    stop=(kt == 31),
                                )
                    for b in range(2):
                        for dc in dcs:
                            nc.scalar.copy(
                                yt[(sc, b)][:, ds(dc * 512, 512)],
                                pw[(b, dc)][:, :],
                            )

                # pass 1 (resident slabs): sc0 then sc1
                wo_block(0, (0, 1), slab1)
                wo_block(1, (0, 1), slab1)
                # pass 2: stream the other d-half of each slab; one JOINT
                # kt loop over both slices so the rotating slab slots are
                # fully consumed before their reuse (no FIFO inversion)
                pw2 = {
                    (sc, b, dc): psW.tile([128, 512], fp32, tag="psW",
                                          name=f"p2w{sc}{b}{dc}")
                    for sc in range(2) for b in range(2) for dc in (2, 3)
                }
                for kt in range(32):
                    sl = wo2.tile([128, 1024], bf16, tag="sl2",
                                  name=f"slab2_{kt}")
                    # gpsimd/sync are both parked behind the A2A#2
                    # completion here, so these 8MB of transfers cannot
                    # contend with the collective itself
                    (nc.gpsimd if kt % 2 == 0 else nc.sync).dma_start(
                        out=sl[:, :],
                        in_=P["wo_r"][:, ds(32768 + kt * 1024, 1024)],
                    )
                    ig, h = divmod(kt, 8)
                    for sc in range(2):
                        for b in range(2):
                            stat = rts[sc][b * 4 + ig][:, ds(h * 128, 128)]
                            for dc in (2, 3):
                                nc.tensor.matmul(
                                    pw2[(sc, b, dc)][:, :],
                                    stat,
                                    sl[:, ds((dc % 2) * 512, 512)],
                                    start=(kt == 0),
                                    stop=(kt == 31),
                                )
                for sc in range(2):
                    for b in range(2):
                        for dc in (2, 3):
                            nc.scalar.copy(
                                yt[(sc, b)][:, ds(dc * 512, 512)],
                                pw2[(sc, b, dc)][:, :],
                            )
                        nc.sync.dma_start(
                            out=out[ds(sc * 256 + b * 128, 128), :],
                            in_=yt[(sc, b)][:, :],
                        )


def _prep_in_maps(x, freqs_cos, freqs_sin, mask, encoder_output, wq, wk, wv, wo):
    x = np.asarray(x, np.float32)
    encoder_output = np.asarray(encoder_output, np.float32)
    freqs_cos = np.asarray(freqs_cos, np.float32)
    freqs_sin = np.asarray(freqs_sin, np.float32)
    wq = np.asarray(wq, np.float32)
    wk = np.asarray(wk, np.float32)
    wv = np.asarray(wv, np.float32)
    wo = np.asarray(wo, np.float32)

    def perm(w):  # deinterleave rope pairs per head: even dims first
        w4 = w.reshape(H, 64, 2, D)
        return np.ascontiguousarray(w4.transpose(0, 2, 1, 3)).reshape(O, D)

    def slab256(wT):  # [D, O] -> [128, 4*32*256]: pass p, dt n, col c
        w4 = wT.reshape(NDT, 128, 4, 256)            # [n, part, p, c]
        return np.ascontiguousarray(
            w4.transpose(1, 2, 0, 3)
        ).reshape(128, NDT * O)

    def slab512(wT):  # [D, O] -> [128, 2*32*512]: oc, dt n, col c
        w4 = wT.reshape(NDT, 128, 2, 512)
        return np.ascontiguousarray(
            w4.transpose(1, 2, 0, 3)
        ).reshape(128, NDT * O)

    alpha = 1.0 / np.sqrt(DH)
    cosT = freqs_cos.T  # [64, S]
    sinT = freqs_sin.T
    csq_cos = (np.concatenate([cosT, cosT], 0) * alpha).astype(BF16)
    csq_sin = (np.concatenate([-sinT, sinT], 0) * alpha).astype(BF16)
    csk_cos = np.concatenate([cosT, cosT], 0).astype(BF16)
    csk_sin = np.concatenate([-sinT, sinT], 0).astype(BF16)

    # 4 diagonal-band keep-masks (0/1, applied post-exp):
    # dmask[t, j*512+s] = 0 if s < t + j*128 else 1
    t_i = np.arange(128)[:, None]
    s_i = np.arange(512)[None, :]
    dmask = np.concatenate(
        [np.where(s_i < t_i + j * 128, 0.0, 1.0) for j in range(4)], axis=1
    ).astype(BF16)
    ones = np.ones((128, 128), BF16)

    # woT slabs: full wo.T (k = head*128+dh on partitions per k-tile), this
    # core's d-half, split into two 1024-wide passes
    woT = np.ascontiguousarray(wo.T).reshape(32, 128, D)  # [kt, dh, dout]

    in_maps = []
    for c in range(8):
        g, r = divmod(c, 4)
        dhalf = g
        sl = slice(r * O, (r + 1) * O)
        xhat = np.concatenate([encoder_output[g], x[g]], axis=0)  # [T, D]
        xhatT = xhat.T.astype(BF16)                               # [D, T]
        x_r = np.ascontiguousarray(
            xhatT.reshape(NDT, 128, T).transpose(1, 0, 2)
        ).reshape(128, NDT * T)
        wqT = perm(wq[sl]).T.astype(BF16)   # [D, O]
        wkT = perm(wk[sl]).T.astype(BF16)
        wvT = wv[sl].T.astype(BF16)
        wo_c = woT[:, :, dhalf * DHALF:(dhalf + 1) * DHALF]  # [32,128,2048]
        wo_r = np.ascontiguousarray(
            wo_c.reshape(32, 128, 2, 1024).transpose(1, 2, 0, 3)
        ).reshape(128, 2 * 32 * 1024).astype(BF16)
        in_maps.append(
            {
                "x_r": x_r,
                "wq_r": slab256(wqT),
                "wk_r": slab256(wkT),
                "wv_r": slab512(wvT),
                "wo_r": wo_r,
                "csq_cos": csq_cos,
                "csq_sin": csq_sin,
                "csk_cos": csk_cos,
                "csk_sin": csk_sin,
                "dmask": dmask,
                "ones": ones,
            }
        )
    return in_maps


def _gather(outs):
    full = np.zeros((2, S, D), np.float32)
    for c in range(8):
        g, q = divmod(c, 4)
        dhalf = g
        o = np.asarray(outs[c]).astype(np.float32)  # [512, 2048]
        for sc in range(2):
            for b in range(2):
                rows = o[sc * 256 + b * 128: sc * 256 + b * 128 + 128]
                full[b, sc * 512 + q * 128: sc * 512 + q * 128 + 128,
                     dhalf * DHALF:(dhalf + 1) * DHALF] = rows
    return full


def kernel(x, start_pos, freqs_cos, freqs_sin, mask, encoder_output, wq, wk, wv, wo):
    global LAST_EXEC_NS
    from concourse.bass_utils import run_bass_kernel_spmd

    if "nc" not in _CACHE:
        _CACHE["nc"] = _build()
    nc = _CACHE["nc"]

    in_maps = _prep_in_maps(
        x, freqs_cos, freqs_sin, mask, encoder_output, wq, wk, wv, wo
    )
    res = run_bass_kernel_spmd(nc, in_maps, core_ids=list(range(8)))
    LAST_EXEC_NS = res.exec_time_ns
    return _gather([res.results[c]["out"] for c in range(8)])


# revision 31
# speedup vs baseline: 1.0865x; 1.0255x over previous
"""Trainium2 Bass kernel for nn_Attention (dense transformer attention block).

Full inputs -> full output. Internally: 8 NeuronCores, 2 data-parallel groups
(batch) x 4-way tensor-parallel (heads). Each core computes 8 heads for one
batch element. The wo projection is redistributed with a single 8-rank
AllToAll per 512-token slice: each core ships its heads' attention output
(oT, feature-major) for token-quarter q to ranks q and q+4, and afterwards
holds the FULL 4096-feature oT for one 128-token quarter of each batch --
it then computes y for those rows over one 2048-wide d-half (group 0 takes
d 0:2048, group 1 d 2048:4096). No ReduceScatter; the A2A moves 2 MB/rank
at ~24 us (mesh) and the sc1 A2A hides under the sc0 wo pass.

Compute in bf16 on the TensorEngine (fp32 matmul is 4x slower), fp32 PSUM
accumulation. All operand layouts are pre-rearranged host-side so every
device DMA is a contiguous per-partition block:
  - projections:  qT/kT = (w-tile).T @ xhatT-tile   -> [feature, seq] layout
  - scores:       scoresT[t, s] = kT-tile.T @ qT    (softmax along partitions)
  - Z:            ones[128,128].T @ probs           -> Z broadcast to all rows
  - PV:           oT[dh, s] = v-tile.T @ probsT
  - wo:           y[t, d] = oT-recv-tile.T @ woT-slab (K=4096 in one psum)
Causal structure is exploited: score tiles that are fully masked are skipped
(scores/exp/Z/PV), and only the 4 diagonal-band tiles per query chunk get a
post-exp 0/1 multiply (from 4 precomputed [128,512] masks). RoPE pairs are
deinterleaved host-side (even dims first); the 1/sqrt(128) score scale is
folded into the q-side cos/sin tables.
"""

import sys

import numpy as np

for _p in ("/opt/trn_rl_repo",):
    if _p not in sys.path:
        sys.path.insert(0, _p)

import ml_dtypes

BF16 = ml_dtypes.bfloat16

D = 4096      # model dim
S = 1024      # decoder sequence length
E = 512       # encoder length
T = E + S     # total key length
H = 8         # heads per core (32 total / 4-way TP)
DH = 128      # head dim
O = H * DH    # per-core projection width = 1024
NDT = D // 128
NEG = -1e9
A2A_GROUP = [[0, 1, 2, 3, 4, 5, 6, 7]]
DHALF = 2048  # per-core output d-half width

_CACHE = {}
LAST_EXEC_NS = None


def _build(no_collective=False):
    import concourse.mybir as mybir
    import concourse.tile as tile
    from concourse import bacc

    bf16 = mybir.dt.bfloat16

    nc = bacc.Bacc(
        "TRN2",
        target_bir_lowering=False,
        debug=False,
        num_devices=8,
    )

    P = {}
    for name, shape in [
        ("x_r", [128, NDT * T]),        # xhatT slabs: cols dt*T + t
        ("wq_r", [128, NDT * O]),       # Q pass slabs: cols p*8192 + n*256 + c
        ("wk_r", [128, NDT * O]),       # K pass slabs: same geometry
        ("wv_r", [128, NDT * O]),       # V slabs: cols oc*16384 + n*512 + c
        ("wo_r", [128, 2 * 32 * 1024]), # woT slabs: cols p*32768 + kt*1024 + d
        ("csq_cos", [128, S]),
        ("csq_sin", [128, S]),
        ("csk_cos", [128, S]),
        ("csk_sin", [128, S]),
        ("dmask", [128, 4 * 512]),      # 4 diagonal-band masks
        ("ones", [128, 128]),
    ]:
        P[name] = nc.declare_dram_parameter(name, shape, bf16, isOutput=False)
    # rows: sc*256 + b*128 + t ; cols: d within this core's d-half
    out = nc.declare_dram_parameter("out", [512, DHALF], bf16, isOutput=True)

    with tile.TileContext(nc) as tc:
        _emit(nc, tc, P, out, no_collective=no_collective)
    nc.compile()
    return nc


def _emit(nc, tc, P, out, no_collective=False):
    import concourse.mybir as mybir
    from concourse.bass import ds

    bf16 = mybir.dt.bfloat16
    fp32 = mybir.dt.float32
    AF = mybir.ActivationFunctionType

    with tc.tile_pool(name="res", bufs=1) as res, \
         tc.tile_pool(name="dram", bufs=1, space="DRAM") as dram:
        onesb = res.tile([128, 128], bf16, tag="onesb")
        dmsk = res.tile([128, 4 * 512], bf16, tag="dmsk")  # 0/1 keep masks

        # A2A staging: rows j*128 + dh (j = dest rank), cols h*128 + t
        a2a_in = [
            dram.tile([1024, 1024], bf16, tag=f"ai{sc}", name=f"a2a_in{sc}")
            for sc in range(2)
        ]
        a2a_out = [
            dram.tile([1024, 1024], bf16, tag=f"ao{sc}", name=f"a2a_out{sc}")
            for sc in range(2)
        ]

        # dmsk/onesb are loaded late in phase 1 (see below) so they don't
        # delay the startup-critical x/weight streams

        # tiny warm-up collective (fired a little into phase 1): absorbs the
        # ~11us first-collective spin-up on the CC stream during projections
        warm_in = dram.tile([8, 128], bf16, tag="wi", name="warm_in")
        warm_out = dram.tile([8, 128], bf16, tag="wo", name="warm_out")

        def fire_warmup():
            if no_collective:
                return
            nc.gpsimd.dma_start(out=warm_in[:, :], in_=P["ones"][ds(0, 8), :])
            nc.gpsimd.collective_compute(
                "AllToAll",
                mybir.AluOpType.bypass,
                replica_groups=A2A_GROUP,
                ins=[warm_in[:, :].opt()],
                outs=[warm_out[:, :].opt()],
            )

        with tc.tile_pool(name="qkv", bufs=1) as qkv:
          qT = qkv.tile([128, H * S], bf16, tag="qT")     # cols h*S + s
          kT = qkv.tile([128, H * T], bf16, tag="kT")     # cols h*T + t
          vsb = qkv.tile([128, 12 * O], bf16, tag="vsb")  # cols tt*O + o

          # ---------------- phase 1: projections + rope ----------------
          with tc.tile_pool(name="xpool", bufs=1) as xpool, \
               tc.tile_pool(name="tabpool", bufs=1) as tabpool, \
               tc.tile_pool(name="wpool", bufs=4) as wpool, \
               tc.tile_pool(name="rtmp", bufs=2) as rtmp, \
               tc.tile_pool(name="ps1", bufs=8, space="PSUM") as ps1:
            # first Q weight half-slab ahead of everything on the sync queue
            def wslab(src, off, n, name):
                wr = wpool.tile([128, n], bf16, tag="wr", name=name)
                nc.sync.dma_start(out=wr[:, :], in_=P[src][:, ds(off, n)])
                return wr

            # first two Q slabs (p0/p1 half 0) in chunks: the first matmuls
            # only need the first columns, so chunked arrival starts
            # compute early
            wr_q0 = {}
            for pi in range(2):
                wr = wpool.tile([128, 16 * 256], bf16, tag="wr",
                                name=f"wr_q{pi}_0")
                for ch in range(2):
                    nc.sync.dma_start(
                        out=wr[:, ds(ch * 2048, 2048)],
                        in_=P["wq_r"][:, ds(pi * 8192 + ch * 2048, 2048)],
                    )
                wr_q0[pi] = wr
            # x slabs: decoder columns first (all the Q pass needs), encoder
            # columns follow (first needed by the K pass ~70us in)
            xh = []
            for dt in range(NDT):
                xt = xpool.tile([128, T], bf16, tag=f"xh{dt}", name=f"xh{dt}")
                (nc.scalar if dt % 2 == 0 else nc.gpsimd).dma_start(
                    out=xt[:, ds(E, S)], in_=P["x_r"][:, ds(dt * T + E, S)]
                )
                xh.append(xt)
                if dt == 7:
                    # rope tables: q tables needed at the first rope (~35us);
                    # the k tables rotate into the same slots later (the
                    # slot WAR makes them wait for the last q rope)
                    csqc = tabpool.tile([128, S], bf16, tag="csc",
                                        name="csqc")
                    csqs = tabpool.tile([128, S], bf16, tag="css",
                                        name="csqs")
                    nc.scalar.dma_start(out=csqc[:, :], in_=P["csq_cos"][:, :])
                    nc.gpsimd.dma_start(out=csqs[:, :], in_=P["csq_sin"][:, :])
            for dt in range(NDT):
                (nc.scalar if dt % 2 == 0 else nc.gpsimd).dma_start(
                    out=xh[dt][:, ds(0, E)], in_=P["x_r"][:, ds(dt * T, E)]
                )
            fire_warmup()
            nc.gpsimd.dma_start(out=dmsk[:, :], in_=P["dmask"][:, :])
            nc.gpsimd.dma_start(out=onesb[:, :], in_=P["ones"][:, :])

            # RoPE: tables are full-height with the 64-row block duplicated
            # (cos) or sign-split (-sin; +sin), so every TensorTensor is
            # partition-aligned. The half-swap goes through an SBUF-SBUF DMA.
            def rope(buf, base, cos, sin, tag):
                swp = rtmp.tile([128, S], bf16, tag="swp", name=f"swp_{tag}")
                nc.gpsimd.dma_start(
                    out=swp[ds(0, 64), :], in_=buf[ds(64, 64), ds(base, S)]
                )
                nc.gpsimd.dma_start(
                    out=swp[ds(64, 64), :], in_=buf[ds(0, 64), ds(base, S)]
                )
                nc.vector.tensor_mul(swp[:, :], swp[:, :], sin[:, :])
                nc.vector.tensor_mul(
                    buf[:, ds(base, S)], buf[:, ds(base, S)], cos[:, :]
                )
                nc.vector.tensor_add(
                    buf[:, ds(base, S)], buf[:, ds(base, S)], swp[:, :]
                )

            # Q: two super-passes of two weight-passes each (8 psum banks),
            # dt OUTERMOST so x-slab consumption spreads over the whole
            # super-pass instead of demanding all 32 slabs in 34us
            for sp in range(2):
                ps = [
                    [
                        [ps1.tile([128, 512], fp32, tag="ps1",
                                  name=f"ps_q_{sp}_{pi}_{oi}_{cc}")
                         for cc in range(2)]
                        for oi in range(2)
                    ]
                    for pi in range(2)
                ]
                wrs = {}
                for half in range(2):
                    for pi in range(2):
                        p = 2 * sp + pi
                        if sp == 0 and half == 0:
                            wrs[(pi, 0)] = wr_q0[pi]
                        else:
                            wrs[(pi, half)] = wslab(
                                "wq_r", (p * 2 + half) * 16 * 256, 16 * 256,
                                f"wr_q{p}_{half}",
                            )
                for dt in range(NDT):
                    half, dtl = divmod(dt, 16)
                    for pi in range(2):
                        for oi in range(2):
                            for cc in range(2):
                                nc.tensor.matmul(
                                    ps[pi][oi][cc][:, :],
                                    wrs[(pi, half)][
                                        :, ds(dtl * 256 + oi * 128, 128)
                                    ],
                                    xh[dt][:, ds(E + cc * 512, 512)],
                                    start=(dt == 0),
                                    stop=(dt == NDT - 1),
                                )
                for pi in range(2):
                    p = 2 * sp + pi
                    for oi in range(2):
                        h = 2 * p + oi
                        for cc in range(2):
                            nc.scalar.copy(
                                qT[:, ds(h * S + cc * 512, 512)],
                                ps[pi][oi][cc][:, :],
                            )
                        rope(qT, h * S, csqc, csqs, f"q{h}")

            # k rope tables rotate into the q tables' slots; emitted here so
            # their slot-WAR wait (last q rope) sits on an idle queue moment
            cskc = tabpool.tile([128, S], bf16, tag="csc", name="cskc")
            csks = tabpool.tile([128, S], bf16, tag="css", name="csks")
            nc.scalar.dma_start(out=cskc[:, :], in_=P["csk_cos"][:, :])
            nc.gpsimd.dma_start(out=csks[:, :], in_=P["csk_sin"][:, :])

            # K: 4 passes x (2 o_tiles x 3 t_chunks), slabs in 16-dt halves
            for p in range(4):
                ps = [
                    [ps1.tile([128, 512], fp32, tag="ps1",
                              name=f"ps_k_{p}_{oi}_{cc}")
                     for cc in range(3)]
                    for oi in range(2)
                ]
                for half in range(2):
                    wr = wslab(
                        "wk_r", (p * 2 + half) * 16 * 256, 16 * 256,
                        f"wr_k{p}_{half}",
                    )
                    for dtl in range(16):
                        dt = half * 16 + dtl
                        for oi in range(2):
                            for cc in range(3):
                                nc.tensor.matmul(
                                    ps[oi][cc][:, :],
                                    wr[:, ds(dtl * 256 + oi * 128, 128)],
                                    xh[dt][:, ds(cc * 512, 512)],
                                    start=(dt == 0),
                                    stop=(dt == NDT - 1),
                                )
                for oi in range(2):
                    h = 2 * p + oi
                    for cc in range(3):
                        nc.scalar.copy(
                            kT[:, ds(h * T + cc * 512, 512)],
                            ps[oi][cc][:, :],
                        )
                    rope(kT, h * T + E, cskc, csks, f"k{h}")

            # V (x-stationary): 2 o_chunks x 2 t_groups of 6 tiles; weight
            # slabs re-streamed per t_group in two 16-dt halves
            for oc in range(2):
                for tg in range(2):
                    tb = tg * 6
                    psv = [ps1.tile([128, 512], fp32, tag="ps1",
                                    name=f"psv_{oc}_{tg}_{ti}")
                           for ti in range(6)]
                    for qr in range(4):
                        wr = wpool.tile(
                            [128, 8 * 512], bf16, tag="wr",
                            name=f"wr_v{oc}_{tg}_{qr}",
                        )
                        nc.sync.dma_start(
                            out=wr[:, :],
                            in_=P["wv_r"][
                                :, ds(oc * NDT * 512 + qr * 8 * 512, 8 * 512)
                            ],
                        )
                        for dtl in range(8):
                            dt = qr * 8 + dtl
                            for ti in range(6):
                                nc.tensor.matmul(
                                    psv[ti][:, :],
                                    xh[dt][:, ds((tb + ti) * 128, 128)],
                                    wr[:, ds(dtl * 512, 512)],
                                    start=(dt == 0),
                                    stop=(dt == NDT - 1),
                                )
                    for ti in range(6):
                        nc.scalar.copy(
                            vsb[:, ds((tb + ti) * O + oc * 512, 512)],
                            psv[ti][:, :],
                        )

          # wo pass-1 slabs (d-chunks 0-2) prefetch during attention; this
          # pool sits in the (dead) phase-1 x region so its DMAs only wait
          # on the last projection matmul, not on attention.
          with tc.tile_pool(name="wpre", bufs=1) as wpre:
            slab1 = [
                wpre.tile([128, 1536], bf16, tag=f"sl1_{kt}", name=f"slab1_{kt}")
                for kt in range(32)
            ]
            # stream pass-1 slabs during attention, all on gpsimd: its only
            # later work is the A2A triggers, so ring-credit waits here
            # never stall a compute-feeding queue (scalar's exp stream
            # stalled ~18us when half of these sat on it)
            for kt in range(32):
                nc.gpsimd.dma_start(
                    out=slab1[kt][:, :],
                    in_=P["wo_r"][:, ds(kt * 1536, 1536)],
                )

            # -------- phase 2: attention (softmax along partitions) --------
            # Per (sc, h): tile list = 4 encoder tiles + decoder tiles that
            # are not fully masked (sc0: 4, sc1: 8). Scores into paired psum
            # banks, exp over the pair, Z via ones-stationary matmul, PV
            # accumulation, then one reciprocal + one mul. After each head's
            # oT is ready it is staged to the A2A input (quarters duplicated
            # to ranks q and q+4); the A2A for a slice fires after its 8th
            # head.
            with tc.tile_pool(name="opool", bufs=3) as opool, \
                 tc.tile_pool(name="ppool", bufs=3) as ppool, \
                 tc.tile_pool(name="zpool", bufs=2) as zpool, \
                 tc.tile_pool(name="psS", bufs=3, space="PSUM") as psS, \
                 tc.tile_pool(name="psZ", bufs=1, space="PSUM") as psZ, \
                 tc.tile_pool(name="psV", bufs=1, space="PSUM") as psV:

              def tiles_for(sc):
                  # (tt, diag_j): tt indexes kT/vsb t-tiles; diag_j is the
                  # diagonal-mask index or None. Fully-masked tiles skipped.
                  lst = [(tt, None) for tt in range(4)]  # encoder
                  if sc == 0:
                      lst += [(4 + j, j) for j in range(4)]
                  else:
                      lst += [(tt, None) for tt in range(4, 8)]
                      lst += [(8 + j, j) for j in range(4)]
                  return lst

              def emit_A(sc, h, pbuf):
                  tl = tiles_for(sc)
                  for k0 in range(0, len(tl), 2):
                      pr = psS.tile([128, 1024], fp32, tag="psS",
                                    name=f"sc{sc}h{h}p{k0}")
                      for half in range(2):
                          tt, dj = tl[k0 + half]
                          nc.tensor.matmul(
                              pr[:, ds(half * 512, 512)],
                              kT[:, ds(h * T + tt * 128, 128)],
                              qT[:, ds(h * S + sc * 512, 512)],
                              start=True,
                              stop=True,
                          )
                      nc.scalar.activation(
                          pbuf[:, ds(k0 * 512, 1024)], pr[:, :], AF.Exp
                      )
                      # causal zeroing of the diagonal-band tiles, post-exp
                      for half in range(2):
                          tt, dj = tl[k0 + half]
                          if dj is not None:
                              nc.vector.tensor_mul(
                                  pbuf[:, ds((k0 + half) * 512, 512)],
                                  pbuf[:, ds((k0 + half) * 512, 512)],
                                  dmsk[:, ds(dj * 512, 512)],
                              )

              def emit_B(sc, h, pbuf):
                  tl = tiles_for(sc)
                  n = len(tl)
                  zp = psZ.tile([128, 512], fp32, tag="psZ", name=f"z{sc}{h}")
                  for k, (tt, _) in enumerate(tl):
                      nc.tensor.matmul(
                          zp[:, :],
                          onesb[:, :],
                          pbuf[:, ds(k * 512, 512)],
                          start=(k == 0),
                          stop=(k == n - 1),
                      )
                  zr = zpool.tile([128, 512], fp32, tag="zr", name=f"zr{sc}{h}")
                  nc.vector.reciprocal_approx_fast(zr[:, :], zp[:, :])
                  pv = psV.tile([128, 512], fp32, tag="psV", name=f"pv{sc}{h}")
                  for k, (tt, _) in enumerate(tl):
                      nc.tensor.matmul(
                          pv[:, :],
                          vsb[:, ds(tt * O + h * 128, 128)],
                          pbuf[:, ds(k * 512, 512)],
                          start=(k == 0),
                          stop=(k == n - 1),
                      )
                  ot = opool.tile([128, 512], bf16, tag="oT",
                                  name=f"oT{sc}{h}")
                  nc.vector.tensor_mul(ot[:, :], pv[:, :], zr[:, :])
                  # stage this head's slice into the A2A input: token quarter
                  # q goes to dest-rank rows q*128 (batch-group 0 dests) and
                  # (4+q)*128 (group 1); one 3-dim DMA per dest group
                  src = ot[:, :].rearrange("dh (q t) -> dh q t", q=4)
                  for g in range(2):
                      dst = a2a_in[sc][
                          ds(g * 512, 512), ds(h * 128, 128)
                      ].rearrange("(q dh) t -> dh q t", q=4)
                      nc.sync.dma_start(out=dst, in_=src)

              def fire_a2a(sc):
                  if no_collective:
                      nc.gpsimd.dma_start(
                          out=a2a_out[sc][:, :], in_=a2a_in[sc][:, :]
                      )
                  else:
                      nc.gpsimd.collective_compute(
                          "AllToAll",
                          mybir.AluOpType.bypass,
                          replica_groups=A2A_GROUP,
                          ins=[a2a_in[sc][:, :].opt()],
                          outs=[a2a_out[sc][:, :].opt()],
                      )

              # software pipeline: 2-head lookahead on A emits
              pend = []
              b_count = 0
              for sc in range(2):
                  for h in range(H):
                      pbuf = ppool.tile(
                          [128, 12 * 512], bf16, tag="p", name=f"pb{sc}{h}"
                      )
                      emit_A(sc, h, pbuf)
                      pend.append((sc, h, pbuf))
                      if len(pend) == 3:
                          s0, h0, pb0 = pend.pop(0)
                          emit_B(s0, h0, pb0)
                          b_count += 1
                          if b_count == 8:
                              fire_a2a(0)
              for s0, h0, pb0 in pend:
                  emit_B(s0, h0, pb0)
                  b_count += 1
              # A2A#2 first: the gpsimd queue sits in the A2A#1 completion
              # wait, and a collective op also waits for completion, so any
              # load placed between the two triggers would delay the second
              # trigger past its own transfer time.
              fire_a2a(1)

            # ---------------- phase 3: wo (y = oT_full.T @ woT) ----------
            # pass 1: d-chunks 0-2 (slabs resident from prefetch); sc0 first
            # (independent of A2A#2, its ~50us hides the collective), then
            # sc1. pass 2: d-chunk 3 with freshly streamed slabs, jointly.
            with tc.tile_pool(name="wo2", bufs=8) as wo2, \
                 tc.tile_pool(name="rtp", bufs=1) as rtp, \
                 tc.tile_pool(name="ypool", bufs=3) as ypool, \
                 tc.tile_pool(name="psW", bufs=8, space="PSUM") as psW:
                rt0 = [
                    rtp.tile([128, 1024], bf16, tag=f"rt0_{i}",
                             name=f"rt0_{i}")
                    for i in range(8)
                ]
                rt1 = [
                    rtp.tile([128, 1024], bf16, tag=f"rt1_{i}",
                             name=f"rt1_{i}")
                    for i in range(8)
                ]
                # sc0 recv on sync (free once staging ends), consumption
                # order so the first k-tiles arrive first
                for i in (0, 4, 1, 5, 2, 6, 3, 7):
                    nc.sync.dma_start(
                        out=rt0[i][:, :],
                        in_=a2a_out[0][ds(i * 128, 128), :],
                    )
                # sc1 recv: gpsimd is parked right behind the A2A#2
                # completion wait; sync joins it after the rt0 loads
                for k, i in enumerate((0, 4, 1, 5, 2, 6, 3, 7)):
                    (nc.gpsimd if k % 2 == 0 else nc.sync).dma_start(
                        out=rt1[i][:, :], in_=a2a_out[1][ds(i * 128, 128), :]
                    )
                rts = [rt0, rt1]

                def ycopy(sc, b, dc, pw):
                    ys = ypool.tile([128, 512], bf16, tag="ys",
                                    name=f"ys{sc}{b}{dc}")
                    nc.scalar.copy(ys[:, :], pw[:, :])
                    nc.sync.dma_start(
                        out=out[ds(sc * 256 + b * 128, 128),
                                ds(dc * 512, 512)],
                        in_=ys[:, :],
                    )

                def wo_block(sc):
                    # psum[t, d] accumulated over all 32 k-tiles
                    pw = {
                        (b, dc): psW.tile([128, 512], fp32, tag="psW",
                                          name=f"pw{sc}{b}{dc}")
                        for b in range(2) for dc in range(3)
                    }
                    for kt in range(32):
                        ig, h = divmod(kt, 8)
                        for b in range(2):
                            stat = rts[sc][b * 4 + ig][:, ds(h * 128, 128)]
                            for dc in range(3):
                                nc.tensor.matmul(
                                    pw[(b, dc)][:, :],
                                    stat,
                                    slab1[kt][:, ds(dc * 512, 512)],
                                    start=(kt == 0),
                                    stop=(kt == 31),
                                )
                    for b in range(2):
                        for dc in range(3):
                            ycopy(sc, b, dc, pw[(b, dc)])

                # pass 1 (resident slabs): sc0 then sc1
                wo_block(0)
                wo_block(1)
                # pass 2: stream d-chunk 3 of each slab; one JOINT kt loop
                # over both slices so the rotating slab slots are fully
                # consumed before their reuse (no FIFO inversion)
                pw2 = {
                    (sc, b): psW.tile([128, 512], fp32, tag="psW",
                                      name=f"p2w{sc}{b}")
                    for sc in range(2) for b in range(2)
                }
                for kt in range(32):
                    sl = wo2.tile([128, 512], bf16, tag="sl2",
                                  name=f"slab2_{kt}")
                    # gpsimd/sync are both parked behind the A2A#2
                    # completion here, so these transfers cannot contend
                    # with the collective itself
                    (nc.gpsimd if kt % 2 == 0 else nc.sync).dma_start(
                        out=sl[:, :],
                        in_=P["wo_r"][:, ds(49152 + kt * 512, 512)],
                    )
                    ig, h = divmod(kt, 8)
                    for sc in range(2):
                        for b in range(2):
                            stat = rts[sc][b * 4 + ig][:, ds(h * 128, 128)]
                            nc.tensor.matmul(
                                pw2[(sc, b)][:, :],
                                stat,
                                sl[:, :],
                                start=(kt == 0),
                                stop=(kt == 31),
                            )
                for sc in range(2):
                    for b in range(2):
                        ycopy(sc, b, 3, pw2[(sc, b)])


def _prep_in_maps(x, freqs_cos, freqs_sin, mask, encoder_output, wq, wk, wv, wo):
    x = np.asarray(x, np.float32)
    encoder_output = np.asarray(encoder_output, np.float32)
    freqs_cos = np.asarray(freqs_cos, np.float32)
    freqs_sin = np.asarray(freqs_sin, np.float32)
    wq = np.asarray(wq, np.float32)
    wk = np.asarray(wk, np.float32)
    wv = np.asarray(wv, np.float32)
    wo = np.asarray(wo, np.float32)

    def perm(w):  # deinterleave rope pairs per head: even dims first
        w4 = w.reshape(H, 64, 2, D)
        return np.ascontiguousarray(w4.transpose(0, 2, 1, 3)).reshape(O, D)

    def slab256(wT):  # [D, O] -> [128, 4*32*256]: pass p, dt n, col c
        w4 = wT.reshape(NDT, 128, 4, 256)            # [n, part, p, c]
        return np.ascontiguousarray(
            w4.transpose(1, 2, 0, 3)
        ).reshape(128, NDT * O)

    def slab512(wT):  # [D, O] -> [128, 2*32*512]: oc, dt n, col c
        w4 = wT.reshape(NDT, 128, 2, 512)
        return np.ascontiguousarray(
            w4.transpose(1, 2, 0, 3)
        ).reshape(128, NDT * O)

    alpha = 1.0 / np.sqrt(DH)
    cosT = freqs_cos.T  # [64, S]
    sinT = freqs_sin.T
    csq_cos = (np.concatenate([cosT, cosT], 0) * alpha).astype(BF16)
    csq_sin = (np.concatenate([-sinT, sinT], 0) * alpha).astype(BF16)
    csk_cos = np.concatenate([cosT, cosT], 0).astype(BF16)
    csk_sin = np.concatenate([-sinT, sinT], 0).astype(BF16)

    # 4 diagonal-band keep-masks (0/1, applied post-exp):
    # dmask[t, j*512+s] = 0 if s < t + j*128 else 1
    t_i = np.arange(128)[:, None]
    s_i = np.arange(512)[None, :]
    dmask = np.concatenate(
        [np.where(s_i < t_i + j * 128, 0.0, 1.0) for j in range(4)], axis=1
    ).astype(BF16)
    ones = np.ones((128, 128), BF16)

    # woT slabs: full wo.T (k = head*128+dh on partitions per k-tile), this
    # core's d-half, split into two 1024-wide passes
    woT = np.ascontiguousarray(wo.T).reshape(32, 128, D)  # [kt, dh, dout]

    in_maps = []
    for c in range(8):
        g, r = divmod(c, 4)
        dhalf = g
        sl = slice(r * O, (r + 1) * O)
        xhat = np.concatenate([encoder_output[g], x[g]], axis=0)  # [T, D]
        xhatT = xhat.T.astype(BF16)                               # [D, T]
        x_r = np.ascontiguousarray(
            xhatT.reshape(NDT, 128, T).transpose(1, 0, 2)
        ).reshape(128, NDT * T)
        wqT = perm(wq[sl]).T.astype(BF16)   # [D, O]
        wkT = perm(wk[sl]).T.astype(BF16)
        wvT = wv[sl].T.astype(BF16)
        wo_c = woT[:, :, dhalf * DHALF:(dhalf + 1) * DHALF]  # [32,128,2048]
        wo_a = np.ascontiguousarray(
            wo_c[:, :, :1536].transpose(1, 0, 2)
        ).reshape(128, 32 * 1536)
        wo_b = np.ascontiguousarray(
            wo_c[:, :, 1536:].transpose(1, 0, 2)
        ).reshape(128, 32 * 512)
        wo_r = np.concatenate([wo_a, wo_b], axis=1).astype(BF16)
        in_maps.append(
            {
                "x_r": x_r,
                "wq_r": slab256(wqT),
                "wk_r": slab256(wkT),
                "wv_r": slab512(wvT),
                "wo_r": wo_r,
                "csq_cos": csq_cos,
                "csq_sin": csq_sin,
                "csk_cos": csk_cos,
                "csk_sin": csk_sin,
                "dmask": dmask,
                "ones": ones,
            }
        )
    return in_maps


def _gather(outs):
    full = np.zeros((2, S, D), np.float32)
    for c in range(8):
        g, q = divmod(c, 4)
        dhalf = g
        o = np.asarray(outs[c]).astype(np.float32)  # [512, 2048]
        for sc in range(2):
            for b in range(2):
                rows = o[sc * 256 + b * 128: sc * 256 + b * 128 + 128]
                full[b, sc * 512 + q * 128: sc * 512 + q * 128 + 128,
                     dhalf * DHALF:(dhalf + 1) * DHALF] = rows
    return full


def kernel(x, start_pos, freqs_cos, freqs_sin, mask, encoder_output, wq, wk, wv, wo):
    global LAST_EXEC_NS
    from concourse.bass_utils import run_bass_kernel_spmd

    if "nc" not in _CACHE:
        _CACHE["nc"] = _build()
    nc = _CACHE["nc"]

    in_maps = _prep_in_maps(
        x, freqs_cos, freqs_sin, mask, encoder_output, wq, wk, wv, wo
    )
    res = run_bass_kernel_spmd(nc, in_maps, core_ids=list(range(8)))
    LAST_EXEC_NS = res.exec_time_ns
    return _gather([res.results[c]["out"] for c in range(8)])


# revision 34
# speedup vs baseline: 1.0870x; 1.0005x over previous
"""Trainium2 Bass kernel for nn_Attention (dense transformer attention block).

Full inputs -> full output. Internally: 8 NeuronCores, 2 data-parallel groups
(batch) x 4-way tensor-parallel (heads). Each core computes 8 heads for one
batch element. The wo projection is redistributed with a single 8-rank
AllToAll per 512-token slice: each core ships its heads' attention output
(oT, feature-major) for token-quarter q to ranks q and q+4, and afterwards
holds the FULL 4096-feature oT for one 128-token quarter of each batch --
it then computes y for those rows over one 2048-wide d-half (group 0 takes
d 0:2048, group 1 d 2048:4096). No ReduceScatter; the A2A moves 2 MB/rank
at ~24 us (mesh) and the sc1 A2A hides under the sc0 wo pass.

Compute in bf16 on the TensorEngine (fp32 matmul is 4x slower), fp32 PSUM
accumulation. All operand layouts are pre-rearranged host-side so every
device DMA is a contiguous per-partition block:
  - projections:  qT/kT = (w-tile).T @ xhatT-tile   -> [feature, seq] layout
  - scores:       scoresT[t, s] = kT-tile.T @ qT    (softmax along partitions)
  - Z:            ones[128,128].T @ probs           -> Z broadcast to all rows
  - PV:           oT[dh, s] = v-tile.T @ probsT
  - wo:           y[t, d] = oT-recv-tile.T @ woT-slab (K=4096 in one psum)
Causal structure is exploited: score tiles that are fully masked are skipped
(scores/exp/Z/PV), and only the 4 diagonal-band tiles per query chunk get a
post-exp 0/1 multiply (from 4 precomputed [128,512] masks). RoPE pairs are
deinterleaved host-side (even dims first); the 1/sqrt(128) score scale is
folded into the q-side cos/sin tables.
"""

import sys

import numpy as np

for _p in ("/opt/trn_rl_repo",):
    if _p not in sys.path:
        sys.path.insert(0, _p)

import ml_dtypes

BF16 = ml_dtypes.bfloat16

D = 4096      # model dim
S = 1024      # decoder sequence length
E = 512       # encoder length
T = E + S     # total key length
H = 8         # heads per core (32 total / 4-way TP)
DH = 128      # head dim
O = H * DH    # per-core projection width = 1024
NDT = D // 128
NEG = -1e9
A2A_GROUP = [[0, 1, 2, 3, 4, 5, 6, 7]]
DHALF = 2048  # per-core output d-half width

_CACHE = {}
LAST_EXEC_NS = None


def _build(no_collective=False):
    import concourse.mybir as mybir
    import concourse.tile as tile
    from concourse import bacc

    bf16 = mybir.dt.bfloat16

    nc = bacc.Bacc(
        "TRN2",
        target_bir_lowering=False,
        debug=False,
        num_devices=8,
    )

    P = {}
    for name, shape in [
        ("x_r", [128, NDT * T]),        # xhatT slabs: cols dt*T + t
        ("wq_r", [128, NDT * O]),       # Q pass slabs: cols p*8192 + n*256 + c
        ("wk_r", [128, NDT * O]),       # K pass slabs: same geometry
        ("wv_r", [128, NDT * O]),       # V slabs: cols oc*16384 + n*512 + c
        ("wo_r", [128, 2 * 32 * 1024]), # woT slabs: cols p*32768 + kt*1024 + d
        ("csq_cos", [128, S]),
        ("csq_sin", [128, S]),
        ("csk_cos", [128, S]),
        ("csk_sin", [128, S]),
        ("dmask", [128, 4 * 512]),      # 4 diagonal-band masks
        ("ones", [128, 128]),
    ]:
        P[name] = nc.declare_dram_parameter(name, shape, bf16, isOutput=False)
    # rows: sc*256 + b*128 + t ; cols: d within this core's d-half
    out = nc.declare_dram_parameter("out", [512, DHALF], bf16, isOutput=True)

    with tile.TileContext(nc) as tc:
        _emit(nc, tc, P, out, no_collective=no_collective)
    nc.compile()
    return nc


def _emit(nc, tc, P, out, no_collective=False):
    import concourse.mybir as mybir
    from concourse.bass import ds

    bf16 = mybir.dt.bfloat16
    fp32 = mybir.dt.float32
    AF = mybir.ActivationFunctionType

    with tc.tile_pool(name="res", bufs=1) as res, \
         tc.tile_pool(name="dram", bufs=1, space="DRAM") as dram:
        onesb = res.tile([128, 128], bf16, tag="onesb")
        dmsk = res.tile([128, 4 * 512], bf16, tag="dmsk")  # 0/1 keep masks

        # A2A staging: rows j*128 + dh (j = dest rank), cols h*128 + t
        a2a_in = [
            dram.tile([1024, 1024], bf16, tag=f"ai{sc}", name=f"a2a_in{sc}")
            for sc in range(2)
        ]
        a2a_out = [
            dram.tile([1024, 1024], bf16, tag=f"ao{sc}", name=f"a2a_out{sc}")
            for sc in range(2)
        ]

        # dmsk/onesb are loaded late in phase 1 (see below) so they don't
        # delay the startup-critical x/weight streams

        # tiny warm-up collective (fired a little into phase 1): absorbs the
        # ~11us first-collective spin-up on the CC stream during projections
        warm_in = dram.tile([8, 128], bf16, tag="wi", name="warm_in")
        warm_out = dram.tile([8, 128], bf16, tag="wo", name="warm_out")

        def fire_warmup():
            if no_collective:
                return
            nc.gpsimd.dma_start(out=warm_in[:, :], in_=P["ones"][ds(0, 8), :])
            nc.gpsimd.collective_compute(
                "AllToAll",
                mybir.AluOpType.bypass,
                replica_groups=A2A_GROUP,
                ins=[warm_in[:, :].opt()],
                outs=[warm_out[:, :].opt()],
            )

        with tc.tile_pool(name="qkv", bufs=1) as qkv:
          qT = qkv.tile([128, H * S], bf16, tag="qT")     # cols h*S + s
          kT = qkv.tile([128, H * T], bf16, tag="kT")     # cols h*T + t
          vsb = qkv.tile([128, 12 * O], bf16, tag="vsb")  # cols tt*O + o

          # ---------------- phase 1: projections + rope ----------------
          with tc.tile_pool(name="xpool", bufs=1) as xpool, \
               tc.tile_pool(name="tabpool", bufs=1) as tabpool, \
               tc.tile_pool(name="wpool", bufs=4) as wpool, \
               tc.tile_pool(name="rtmp", bufs=2) as rtmp, \
               tc.tile_pool(name="ps1", bufs=8, space="PSUM") as ps1:
            # first Q weight half-slab ahead of everything on the sync queue
            def wslab(src, off, n, name):
                wr = wpool.tile([128, n], bf16, tag="wr", name=name)
                nc.sync.dma_start(out=wr[:, :], in_=P[src][:, ds(off, n)])
                return wr

            # first two Q slabs (p0/p1 half 0) in chunks: the first matmuls
            # only need the first columns, so chunked arrival starts
            # compute early
            wr_q0 = {}
            for pi in range(2):
                wr = wpool.tile([128, 16 * 256], bf16, tag="wr",
                                name=f"wr_q{pi}_0")
                for ch in range(2):
                    nc.sync.dma_start(
                        out=wr[:, ds(ch * 2048, 2048)],
                        in_=P["wq_r"][:, ds(pi * 8192 + ch * 2048, 2048)],
                    )
                wr_q0[pi] = wr
            # x slabs: decoder columns first (all the Q pass needs), encoder
            # columns follow (first needed by the K pass ~70us in)
            xh = []
            for dt in range(NDT):
                xt = xpool.tile([128, T], bf16, tag=f"xh{dt}", name=f"xh{dt}")
                (nc.scalar if dt % 2 == 0 else nc.gpsimd).dma_start(
                    out=xt[:, ds(E, S)], in_=P["x_r"][:, ds(dt * T + E, S)]
                )
                xh.append(xt)
                if dt == 7:
                    # rope tables: q tables needed at the first rope (~35us);
                    # the k tables rotate into the same slots later (the
                    # slot WAR makes them wait for the last q rope)
                    csqc = tabpool.tile([128, S], bf16, tag="csc",
                                        name="csqc")
                    csqs = tabpool.tile([128, S], bf16, tag="css",
                                        name="csqs")
                    nc.scalar.dma_start(out=csqc[:, :], in_=P["csq_cos"][:, :])
                    nc.gpsimd.dma_start(out=csqs[:, :], in_=P["csq_sin"][:, :])
            for dt in range(NDT):
                (nc.scalar if dt % 2 == 0 else nc.gpsimd).dma_start(
                    out=xh[dt][:, ds(0, E)], in_=P["x_r"][:, ds(dt * T, E)]
                )
            fire_warmup()
            nc.gpsimd.dma_start(out=dmsk[:, :], in_=P["dmask"][:, :])
            nc.gpsimd.dma_start(out=onesb[:, :], in_=P["ones"][:, :])

            # RoPE: tables are full-height with the 64-row block duplicated
            # (cos) or sign-split (-sin; +sin), so every TensorTensor is
            # partition-aligned. The half-swap goes through an SBUF-SBUF DMA.
            def rope(buf, base, cos, sin, tag):
                swp = rtmp.tile([128, S], bf16, tag="swp", name=f"swp_{tag}")
                nc.gpsimd.dma_start(
                    out=swp[ds(0, 64), :], in_=buf[ds(64, 64), ds(base, S)]
                )
                nc.gpsimd.dma_start(
                    out=swp[ds(64, 64), :], in_=buf[ds(0, 64), ds(base, S)]
                )
                nc.vector.tensor_mul(swp[:, :], swp[:, :], sin[:, :])
                nc.vector.tensor_mul(
                    buf[:, ds(base, S)], buf[:, ds(base, S)], cos[:, :]
                )
                nc.vector.tensor_add(
                    buf[:, ds(base, S)], buf[:, ds(base, S)], swp[:, :]
                )

            # Q: two super-passes of two weight-passes each (8 psum banks),
            # dt OUTERMOST so x-slab consumption spreads over the whole
            # super-pass instead of demanding all 32 slabs in 34us
            for sp in range(2):
                ps = [
                    [
                        [ps1.tile([128, 512], fp32, tag="ps1",
                                  name=f"ps_q_{sp}_{pi}_{oi}_{cc}")
                         for cc in range(2)]
                        for oi in range(2)
                    ]
                    for pi in range(2)
                ]
                wrs = {}
                for half in range(2):
                    for pi in range(2):
                        p = 2 * sp + pi
                        if sp == 0 and half == 0:
                            wrs[(pi, 0)] = wr_q0[pi]
                        else:
                            wrs[(pi, half)] = wslab(
                                "wq_r", (p * 2 + half) * 16 * 256, 16 * 256,
                                f"wr_q{p}_{half}",
                            )
                for dt in range(NDT):
                    half, dtl = divmod(dt, 16)
                    for pi in range(2):
                        for oi in range(2):
                            for cc in range(2):
                                nc.tensor.matmul(
                                    ps[pi][oi][cc][:, :],
                                    wrs[(pi, half)][
                                        :, ds(dtl * 256 + oi * 128, 128)
                                    ],
                                    xh[dt][:, ds(E + cc * 512, 512)],
                                    start=(dt == 0),
                                    stop=(dt == NDT - 1),
                                )
                for pi in range(2):
                    p = 2 * sp + pi
                    for oi in range(2):
                        h = 2 * p + oi
                        for cc in range(2):
                            nc.scalar.copy(
                                qT[:, ds(h * S + cc * 512, 512)],
                                ps[pi][oi][cc][:, :],
                            )
                        rope(qT, h * S, csqc, csqs, f"q{h}")

            # k rope tables rotate into the q tables' slots; emitted here so
            # their slot-WAR wait (last q rope) sits on an idle queue moment
            cskc = tabpool.tile([128, S], bf16, tag="csc", name="cskc")
            csks = tabpool.tile([128, S], bf16, tag="css", name="csks")
            nc.scalar.dma_start(out=cskc[:, :], in_=P["csk_cos"][:, :])
            nc.gpsimd.dma_start(out=csks[:, :], in_=P["csk_sin"][:, :])

            # K: 4 passes x (2 o_tiles x 3 t_chunks), slabs in 16-dt halves
            for p in range(4):
                ps = [
                    [ps1.tile([128, 512], fp32, tag="ps1",
                              name=f"ps_k_{p}_{oi}_{cc}")
                     for cc in range(3)]
                    for oi in range(2)
                ]
                for half in range(2):
                    wr = wslab(
                        "wk_r", (p * 2 + half) * 16 * 256, 16 * 256,
                        f"wr_k{p}_{half}",
                    )
                    for dtl in range(16):
                        dt = half * 16 + dtl
                        for oi in range(2):
                            for cc in range(3):
                                nc.tensor.matmul(
                                    ps[oi][cc][:, :],
                                    wr[:, ds(dtl * 256 + oi * 128, 128)],
                                    xh[dt][:, ds(cc * 512, 512)],
                                    start=(dt == 0),
                                    stop=(dt == NDT - 1),
                                )
                for oi in range(2):
                    h = 2 * p + oi
                    for cc in range(3):
                        nc.scalar.copy(
                            kT[:, ds(h * T + cc * 512, 512)],
                            ps[oi][cc][:, :],
                        )
                    rope(kT, h * T + E, cskc, csks, f"k{h}")

            # V (x-stationary): 2 o_chunks x 2 t_groups of 6 tiles; weight
            # slabs re-streamed per t_group in two 16-dt halves
            for oc in range(2):
                for tg in range(2):
                    tb = tg * 6
                    psv = [ps1.tile([128, 512], fp32, tag="ps1",
                                    name=f"psv_{oc}_{tg}_{ti}")
                           for ti in range(6)]
                    for qr in range(4):
                        wr = wpool.tile(
                            [128, 8 * 512], bf16, tag="wr",
                            name=f"wr_v{oc}_{tg}_{qr}",
                        )
                        nc.sync.dma_start(
                            out=wr[:, :],
                            in_=P["wv_r"][
                                :, ds(oc * NDT * 512 + qr * 8 * 512, 8 * 512)
                            ],
                        )
                        for dtl in range(8):
                            dt = qr * 8 + dtl
                            for ti in range(6):
                                nc.tensor.matmul(
                                    psv[ti][:, :],
                                    xh[dt][:, ds((tb + ti) * 128, 128)],
                                    wr[:, ds(dtl * 512, 512)],
                                    start=(dt == 0),
                                    stop=(dt == NDT - 1),
                                )
                    for ti in range(6):
                        nc.scalar.copy(
                            vsb[:, ds((tb + ti) * O + oc * 512, 512)],
                            psv[ti][:, :],
                        )

          # wo pass-1 slabs (d-chunks 0-2) prefetch during attention; this
          # pool sits in the (dead) phase-1 x region so its DMAs only wait
          # on the last projection matmul, not on attention.
          with tc.tile_pool(name="wpre", bufs=1) as wpre:
            slab1 = [
                wpre.tile([128, 1536], bf16, tag=f"sl1_{kt}", name=f"slab1_{kt}")
                for kt in range(32)
            ]
            # stream pass-1 slabs during attention, all on gpsimd: its only
            # later work is the A2A triggers, so ring-credit waits here
            # never stall a compute-feeding queue (scalar's exp stream
            # stalled ~18us when half of these sat on it)
            for kt in range(32):
                nc.gpsimd.dma_start(
                    out=slab1[kt][:, :],
                    in_=P["wo_r"][:, ds(kt * 1536, 1536)],
                )

            # -------- phase 2: attention (softmax along partitions) --------
            # Per (sc, h): tile list = 4 encoder tiles + decoder tiles that
            # are not fully masked (sc0: 4, sc1: 8). Scores into paired psum
            # banks, exp over the pair, Z via ones-stationary matmul, PV
            # accumulation, then one reciprocal + one mul. After each head's
            # oT is ready it is staged to the A2A input (quarters duplicated
            # to ranks q and q+4); the A2A for a slice fires after its 8th
            # head.
            with tc.tile_pool(name="opool", bufs=3) as opool, \
                 tc.tile_pool(name="ppool", bufs=3) as ppool, \
                 tc.tile_pool(name="zpool", bufs=2) as zpool, \
                 tc.tile_pool(name="psS", bufs=3, space="PSUM") as psS, \
                 tc.tile_pool(name="psZ", bufs=1, space="PSUM") as psZ, \
                 tc.tile_pool(name="psV", bufs=1, space="PSUM") as psV:

              def tiles_for(sc):
                  # (tt, diag_j): tt indexes kT/vsb t-tiles; diag_j is the
                  # diagonal-mask index or None. Fully-masked tiles skipped.
                  lst = [(tt, None) for tt in range(4)]  # encoder
                  if sc == 0:
                      lst += [(4 + j, j) for j in range(4)]
                  else:
                      lst += [(tt, None) for tt in range(4, 8)]
                      lst += [(8 + j, j) for j in range(4)]
                  return lst

              def emit_A(sc, h, pbuf):
                  # diagonal-band tile j only contributes for q >= j*128;
                  # scores/exp/Z/PV all run on the reduced q-range
                  tl = tiles_for(sc)
                  for k0 in range(0, len(tl), 2):
                      pr = psS.tile([128, 1024], fp32, tag="psS",
                                    name=f"sc{sc}h{h}p{k0}")
                      full_pair = all(dj is None for _, dj in tl[k0:k0 + 2])
                      for half in range(2):
                          tt, dj = tl[k0 + half]
                          qo = 0 if dj is None else dj * 128
                          w = 512 - qo
                          nc.tensor.matmul(
                              pr[:, ds(half * 512 + qo, w)],
                              kT[:, ds(h * T + tt * 128, 128)],
                              qT[:, ds(h * S + sc * 512 + qo, w)],
                              start=True,
                              stop=True,
                          )
                      if full_pair:
                          nc.scalar.activation(
                              pbuf[:, ds(k0 * 512, 1024)], pr[:, :], AF.Exp
                          )
                      else:
                          for half in range(2):
                              tt, dj = tl[k0 + half]
                              qo = 0 if dj is None else dj * 128
                              w = 512 - qo
                              nc.scalar.activation(
                                  pbuf[:, ds((k0 + half) * 512 + qo, w)],
                                  pr[:, ds(half * 512 + qo, w)],
                                  AF.Exp,
                              )
                      # causal zeroing of the diagonal staircase chunk (the
                      # first 128 computed columns of a diag tile), post-exp
                      for half in range(2):
                          tt, dj = tl[k0 + half]
                          if dj is not None:
                              qo = dj * 128
                              nc.vector.tensor_mul(
                                  pbuf[:, ds((k0 + half) * 512 + qo, 128)],
                                  pbuf[:, ds((k0 + half) * 512 + qo, 128)],
                                  dmsk[:, ds(dj * 512 + qo, 128)],
                              )

              def emit_B(sc, h, pbuf):
                  tl = tiles_for(sc)
                  n = len(tl)
                  zp = psZ.tile([128, 512], fp32, tag="psZ", name=f"z{sc}{h}")
                  for k, (tt, dj) in enumerate(tl):
                      qo = 0 if dj is None else dj * 128
                      nc.tensor.matmul(
                          zp[:, ds(qo, 512 - qo)],
                          onesb[:, :],
                          pbuf[:, ds(k * 512 + qo, 512 - qo)],
                          start=(k == 0),
                          stop=(k == n - 1),
                      )
                  zr = zpool.tile([128, 512], fp32, tag="zr", name=f"zr{sc}{h}")
                  nc.vector.reciprocal_approx_fast(zr[:, :], zp[:, :])
                  pv = psV.tile([128, 512], fp32, tag="psV", name=f"pv{sc}{h}")
                  for k, (tt, dj) in enumerate(tl):
                      qo = 0 if dj is None else dj * 128
                      nc.tensor.matmul(
                          pv[:, ds(qo, 512 - qo)],
                          vsb[:, ds(tt * O + h * 128, 128)],
                          pbuf[:, ds(k * 512 + qo, 512 - qo)],
                          start=(k == 0),
                          stop=(k == n - 1),
                      )
                  ot = opool.tile([128, 512], bf16, tag="oT",
                                  name=f"oT{sc}{h}")
                  nc.vector.tensor_mul(ot[:, :], pv[:, :], zr[:, :])
                  # stage this head's slice into the A2A input: token quarter
                  # q goes to dest-rank rows q*128 (batch-group 0 dests) and
                  # (4+q)*128 (group 1); one 3-dim DMA per dest group
                  src = ot[:, :].rearrange("dh (q t) -> dh q t", q=4)
                  for g in range(2):
                      dst = a2a_in[sc][
                          ds(g * 512, 512), ds(h * 128, 128)
                      ].rearrange("(q dh) t -> dh q t", q=4)
                      nc.sync.dma_start(out=dst, in_=src)

              def fire_a2a(sc):
                  if no_collective:
                      nc.gpsimd.dma_start(
                          out=a2a_out[sc][:, :], in_=a2a_in[sc][:, :]
                      )
                  else:
                      nc.gpsimd.collective_compute(
                          "AllToAll",
                          mybir.AluOpType.bypass,
                          replica_groups=A2A_GROUP,
                          ins=[a2a_in[sc][:, :].opt()],
                          outs=[a2a_out[sc][:, :].opt()],
                      )

              # software pipeline: 2-head lookahead on A emits
              pend = []
              b_count = 0
              for sc in range(2):
                  for h in range(H):
                      pbuf = ppool.tile(
                          [128, 12 * 512], bf16, tag="p", name=f"pb{sc}{h}"
                      )
                      emit_A(sc, h, pbuf)
                      pend.append((sc, h, pbuf))
                      if len(pend) == 3:
                          s0, h0, pb0 = pend.pop(0)
                          emit_B(s0, h0, pb0)
                          b_count += 1
                          if b_count == 8:
                              fire_a2a(0)
              for s0, h0, pb0 in pend:
                  emit_B(s0, h0, pb0)
                  b_count += 1
              # A2A#2 first: the gpsimd queue sits in the A2A#1 completion
              # wait, and a collective op also waits for completion, so any
              # load placed between the two triggers would delay the second
              # trigger past its own transfer time.
              fire_a2a(1)

            # ---------------- phase 3: wo (y = oT_full.T @ woT) ----------
            # pass 1: d-chunks 0-2 (slabs resident from prefetch); sc0 first
            # (independent of A2A#2, its ~50us hides the collective), then
            # sc1. pass 2: d-chunk 3 with freshly streamed slabs, jointly.
            with tc.tile_pool(name="wo2", bufs=8) as wo2, \
                 tc.tile_pool(name="rtp", bufs=1) as rtp, \
                 tc.tile_pool(name="ypool", bufs=3) as ypool, \
                 tc.tile_pool(name="psW", bufs=8, space="PSUM") as psW:
                rt0 = [
                    rtp.tile([128, 1024], bf16, tag=f"rt0_{i}",
                             name=f"rt0_{i}")
                    for i in range(8)
                ]
                rt1 = [
                    rtp.tile([128, 1024], bf16, tag=f"rt1_{i}",
                             name=f"rt1_{i}")
                    for i in range(8)
                ]
                # sc0 recv split over scalar+sync (both near-free once the
                # exps/staging end), consumption order so the first k-tiles
                # arrive first
                for k, i in enumerate((0, 4, 1, 5, 2, 6, 3, 7)):
                    (nc.scalar if k % 2 == 0 else nc.sync).dma_start(
                        out=rt0[i][:, :],
                        in_=a2a_out[0][ds(i * 128, 128), :],
                    )
                # sc1 recv: gpsimd is parked right behind the A2A#2
                # completion wait; sync joins it after the rt0 loads
                for k, i in enumerate((0, 4, 1, 5, 2, 6, 3, 7)):
                    (nc.gpsimd if k % 2 == 0 else nc.sync).dma_start(
                        out=rt1[i][:, :], in_=a2a_out[1][ds(i * 128, 128), :]
                    )
                rts = [rt0, rt1]

                def ycopy(sc, b, dc, pw):
                    ys = ypool.tile([128, 512], bf16, tag="ys",
                                    name=f"ys{sc}{b}{dc}")
                    nc.scalar.copy(ys[:, :], pw[:, :])
                    nc.sync.dma_start(
                        out=out[ds(sc * 256 + b * 128, 128),
                                ds(dc * 512, 512)],
                        in_=ys[:, :],
                    )

                def wo_block(sc):
                    # psum[t, d] accumulated over all 32 k-tiles
                    pw = {
                        (b, dc): psW.tile([128, 512], fp32, tag="psW",
                                          name=f"pw{sc}{b}{dc}")
                        for b in range(2) for dc in range(3)
                    }
                    for kt in range(32):
                        ig, h = divmod(kt, 8)
                        for b in range(2):
                            stat = rts[sc][b * 4 + ig][:, ds(h * 128, 128)]
                            for dc in range(3):
                                nc.tensor.matmul(
                                    pw[(b, dc)][:, :],
                                    stat,
                                    slab1[kt][:, ds(dc * 512, 512)],
                                    start=(kt == 0),
                                    stop=(kt == 31),
                                )
                    for b in range(2):
                        for dc in range(3):
                            ycopy(sc, b, dc, pw[(b, dc)])

                # pass 1 (resident slabs): sc0 then sc1
                wo_block(0)
                wo_block(1)
                # pass 2: stream d-chunk 3 of each slab; one JOINT kt loop
                # over both slices so the rotating slab slots are fully
                # consumed before their reuse (no FIFO inversion)
                pw2 = {
                    (sc, b): psW.tile([128, 512], fp32, tag="psW",
                                      name=f"p2w{sc}{b}")
                    for sc in range(2) for b in range(2)
                }
                for kt in range(32):
                    sl = wo2.tile([128, 512], bf16, tag="sl2",
                                  name=f"slab2_{kt}")
                    # gpsimd/sync are both parked behind the A2A#2
                    # completion here, so these transfers cannot contend
                    # with the collective itself
                    (nc.gpsimd if kt % 2 == 0 else nc.sync).dma_start(
                        out=sl[:, :],
                        in_=P["wo_r"][:, ds(49152 + kt * 512, 512)],
                    )
                    ig, h = divmod(kt, 8)
                    for sc in range(2):
                        for b in range(2):
                            stat = rts[sc][b * 4 + ig][:, ds(h * 128, 128)]
                            nc.tensor.matmul(
                                pw2[(sc, b)][:, :],
                                stat,
                                sl[:, :],
                                start=(kt == 0),
                                stop=(kt == 31),
                            )
                for sc in range(2):
                    for b in range(2):
                        ycopy(sc, b, 3, pw2[(sc, b)])


def _prep_in_maps(x, freqs_cos, freqs_sin, mask, encoder_output, wq, wk, wv, wo):
    x = np.asarray(x, np.float32)
    encoder_output = np.asarray(encoder_output, np.float32)
    freqs_cos = np.asarray(freqs_cos, np.float32)
    freqs_sin = np.asarray(freqs_sin, np.float32)
    wq = np.asarray(wq, np.float32)
    wk = np.asarray(wk, np.float32)
    wv = np.asarray(wv, np.float32)
    wo = np.asarray(wo, np.float32)

    def perm(w):  # deinterleave rope pairs per head: even dims first
        w4 = w.reshape(H, 64, 2, D)
        return np.ascontiguousarray(w4.transpose(0, 2, 1, 3)).reshape(O, D)

    def slab256(wT):  # [D, O] -> [128, 4*32*256]: pass p, dt n, col c
        w4 = wT.reshape(NDT, 128, 4, 256)            # [n, part, p, c]
        return np.ascontiguousarray(
            w4.transpose(1, 2, 0, 3)
        ).reshape(128, NDT * O)

    def slab512(wT):  # [D, O] -> [128, 2*32*512]: oc, dt n, col c
        w4 = wT.reshape(NDT, 128, 2, 512)
        return np.ascontiguousarray(
            w4.transpose(1, 2, 0, 3)
        ).reshape(128, NDT * O)

    alpha = 1.0 / np.sqrt(DH)
    cosT = freqs_cos.T  # [64, S]
    sinT = freqs_sin.T
    csq_cos = (np.concatenate([cosT, cosT], 0) * alpha).astype(BF16)
    csq_sin = (np.concatenate([-sinT, sinT], 0) * alpha).astype(BF16)
    csk_cos = np.concatenate([cosT, cosT], 0).astype(BF16)
    csk_sin = np.concatenate([-sinT, sinT], 0).astype(BF16)

    # 4 diagonal-band keep-masks (0/1, applied post-exp):
    # dmask[t, j*512+s] = 0 if s < t + j*128 else 1
    t_i = np.arange(128)[:, None]
    s_i = np.arange(512)[None, :]
    dmask = np.concatenate(
        [np.where(s_i < t_i + j * 128, 0.0, 1.0) for j in range(4)], axis=1
    ).astype(BF16)
    ones = np.ones((128, 128), BF16)

    # woT slabs: full wo.T (k = head*128+dh on partitions per k-tile), this
    # core's d-half, split into two 1024-wide passes
    woT = np.ascontiguousarray(wo.T).reshape(32, 128, D)  # [kt, dh, dout]

    in_maps = []
    for c in range(8):
        g, r = divmod(c, 4)
        dhalf = g
        sl = slice(r * O, (r + 1) * O)
        xhat = np.concatenate([encoder_output[g], x[g]], axis=0)  # [T, D]
        xhatT = xhat.T.astype(BF16)                               # [D, T]
        x_r = np.ascontiguousarray(
            xhatT.reshape(NDT, 128, T).transpose(1, 0, 2)
        ).reshape(128, NDT * T)
        wqT = perm(wq[sl]).T.astype(BF16)   # [D, O]
        wkT = perm(wk[sl]).T.astype(BF16)
        wvT = wv[sl].T.astype(BF16)
        wo_c = woT[:, :, dhalf * DHALF:(dhalf + 1) * DHALF]  # [32,128,2048]
        wo_a = np.ascontiguousarray(
            wo_c[:, :, :1536].transpose(1, 0, 2)
        ).reshape(128, 32 * 1536)
        wo_b = np.ascontiguousarray(
            wo_c[:, :, 1536:].transpose(1, 0, 2)
        ).reshape(128, 32 * 512)
        wo_r = np.concatenate([wo_a, wo_b], axis=1).astype(BF16)
        in_maps.append(
            {
                "x_r": x_r,
                "wq_r": slab256(wqT),
                "wk_r": slab256(wkT),
                "wv_r": slab512(wvT),
                "wo_r": wo_r,
                "csq_cos": csq_cos,
                "csq_sin": csq_sin,
                "csk_cos": csk_cos,
                "csk_sin": csk_sin,
                "dmask": dmask,
                "ones": ones,
            }
        )
    return in_maps


def _gather(outs):
    full = np.zeros((2, S, D), np.float32)
    for c in range(8):
        g, q = divmod(c, 4)
        dhalf = g
        o = np.asarray(outs[c]).astype(np.float32)  # [512, 2048]
        for sc in range(2):
            for b in range(2):
                rows = o[sc * 256 + b * 128: sc * 256 + b * 128 + 128]
                full[b, sc * 512 + q * 128: sc * 512 + q * 128 + 128,
                     dhalf * DHALF:(dhalf + 1) * DHALF] = rows
    return full


def kernel(x, start_pos, freqs_cos, freqs_sin, mask, encoder_output, wq, wk, wv, wo):
    global LAST_EXEC_NS
    from concourse.bass_utils import run_bass_kernel_spmd

    if "nc" not in _CACHE:
        _CACHE["nc"] = _build()
    nc = _CACHE["nc"]

    in_maps = _prep_in_maps(
        x, freqs_cos, freqs_sin, mask, encoder_output, wq, wk, wv, wo
    )
    res = run_bass_kernel_spmd(nc, in_maps, core_ids=list(range(8)))
    LAST_EXEC_NS = res.exec_time_ns
    return _gather([res.results[c]["out"] for c in range(8)])


# revision 42
# speedup vs baseline: 1.0931x; 1.0056x over previous
"""Trainium2 Bass kernel for nn_Attention (dense transformer attention block).

Full inputs -> full output. Internally: 8 NeuronCores, 2 data-parallel groups
(batch) x 4-way tensor-parallel (heads). Each core computes 8 heads for one
batch element. The wo projection is redistributed with a single 8-rank
AllToAll per 512-token slice: each core ships its heads' attention output
(oT, feature-major) for token-quarter q to ranks q and q+4, and afterwards
holds the FULL 4096-feature oT for one 128-token quarter of each batch --
it then computes y for those rows over one 2048-wide d-half (group 0 takes
d 0:2048, group 1 d 2048:4096). No ReduceScatter; the A2A moves 2 MB/rank
at ~24 us (mesh) and the sc1 A2A hides under the sc0 wo pass.

Compute in bf16 on the TensorEngine (fp32 matmul is 4x slower), fp32 PSUM
accumulation. All operand layouts are pre-rearranged host-side so every
device DMA is a contiguous per-partition block:
  - projections:  qT/kT = (w-tile).T @ xhatT-tile   -> [feature, seq] layout
  - scores:       scoresT[t, s] = kT-tile.T @ qT    (softmax along partitions)
  - Z:            ones[128,128].T @ probs           -> Z broadcast to all rows
  - PV:           oT[dh, s] = v-tile.T @ probsT
  - wo:           y[t, d] = oT-recv-tile.T @ woT-slab (K=4096 in one psum)
Causal structure is exploited: score tiles that are fully masked are skipped
(scores/exp/Z/PV), and only the 4 diagonal-band tiles per query chunk get a
post-exp 0/1 multiply (from 4 precomputed [128,512] masks). RoPE pairs are
deinterleaved host-side (even dims first); the 1/sqrt(128) score scale is
folded into the q-side cos/sin tables.
"""

import sys

import numpy as np

for _p in ("/opt/trn_rl_repo",):
    if _p not in sys.path:
        sys.path.insert(0, _p)

import ml_dtypes

BF16 = ml_dtypes.bfloat16

D = 4096      # model dim
S = 1024      # decoder sequence length
E = 512       # encoder length
T = E + S     # total key length
H = 8         # heads per core (32 total / 4-way TP)
DH = 128      # head dim
O = H * DH    # per-core projection width = 1024
NDT = D // 128
NEG = -1e9
A2A_GROUP = [[0, 1, 2, 3, 4, 5, 6, 7]]
DHALF = 2048  # per-core output d-half width

_CACHE = {}
LAST_EXEC_NS = None


def _build(no_collective=False):
    import concourse.mybir as mybir
    import concourse.tile as tile
    from concourse import bacc

    bf16 = mybir.dt.bfloat16

    nc = bacc.Bacc(
        "TRN2",
        target_bir_lowering=False,
        debug=False,
        num_devices=8,
    )

    P = {}
    for name, shape in [
        ("x_r", [128, NDT * T]),        # xhatT slabs: cols dt*T + t
        ("wq_r", [128, NDT * O]),       # Q pass slabs: cols p*8192 + n*256 + c
        ("wk_r", [128, NDT * O]),       # K pass slabs: same geometry
        ("wv_r", [128, NDT * O]),       # V slabs: cols oc*16384 + n*512 + c
        ("wo_r", [128, 2 * 32 * 1024]), # woT slabs: cols p*32768 + kt*1024 + d
        ("csq_cos", [128, S]),
        ("csq_sin", [128, S]),
        ("csk_cos", [128, S]),
        ("csk_sin", [128, S]),
        ("dmask", [128, 4 * 512]),      # 4 diagonal-band masks
        ("ones", [128, 128]),
    ]:
        P[name] = nc.declare_dram_parameter(name, shape, bf16, isOutput=False)
    # rows: sc*256 + b*128 + t ; cols: d within this core's d-half
    out = nc.declare_dram_parameter("out", [512, DHALF], bf16, isOutput=True)

    with tile.TileContext(nc) as tc:
        _emit(nc, tc, P, out, no_collective=no_collective)
    nc.compile()
    return nc


def _emit(nc, tc, P, out, no_collective=False):
    import concourse.mybir as mybir
    from concourse.bass import ds

    bf16 = mybir.dt.bfloat16
    fp32 = mybir.dt.float32
    AF = mybir.ActivationFunctionType

    with tc.tile_pool(name="res", bufs=1) as res, \
         tc.tile_pool(name="dram", bufs=1, space="DRAM") as dram:
        onesb = res.tile([128, 128], bf16, tag="onesb")
        dmsk = res.tile([128, 4 * 512], bf16, tag="dmsk")  # 0/1 keep masks

        # A2A staging: rows j*128 + dh (j = dest rank), cols h*128 + t
        a2a_in = [
            dram.tile([1024, 1024], bf16, tag=f"ai{sc}", name=f"a2a_in{sc}")
            for sc in range(2)
        ]
        a2a_out = [
            dram.tile([1024, 1024], bf16, tag=f"ao{sc}", name=f"a2a_out{sc}")
            for sc in range(2)
        ]

        # dmsk/onesb are loaded late in phase 1 (see below) so they don't
        # delay the startup-critical x/weight streams

        # tiny warm-up collective (fired a little into phase 1): absorbs the
        # ~11us first-collective spin-up on the CC stream during projections
        warm_in = dram.tile([8, 128], bf16, tag="wi", name="warm_in")
        warm_out = dram.tile([8, 128], bf16, tag="wo", name="warm_out")

        def fire_warmup():
            if no_collective:
                return
            nc.gpsimd.dma_start(out=warm_in[:, :], in_=P["ones"][ds(0, 8), :])
            nc.gpsimd.collective_compute(
                "AllToAll",
                mybir.AluOpType.bypass,
                replica_groups=A2A_GROUP,
                ins=[warm_in[:, :].opt()],
                outs=[warm_out[:, :].opt()],
            )

        with tc.tile_pool(name="qkv", bufs=1) as qkv:
          qT = qkv.tile([128, H * S], bf16, tag="qT")     # cols h*S + s
          kT = qkv.tile([128, H * T], bf16, tag="kT")     # cols h*T + t
          vsb = qkv.tile([128, 12 * O], bf16, tag="vsb")  # cols tt*O + o

          # ---------------- phase 1: projections + rope ----------------
          with tc.tile_pool(name="xpool", bufs=1) as xpool, \
               tc.tile_pool(name="tabpool", bufs=1) as tabpool, \
               tc.tile_pool(name="wpool", bufs=4) as wpool, \
               tc.tile_pool(name="rtmp", bufs=2) as rtmp, \
               tc.tile_pool(name="ps1", bufs=8, space="PSUM") as ps1:
            # first Q weight half-slab ahead of everything on the sync queue
            def wslab(src, off, n, name):
                wr = wpool.tile([128, n], bf16, tag="wr", name=name)
                nc.sync.dma_start(out=wr[:, :], in_=P[src][:, ds(off, n)])
                return wr

            # first two Q slabs (p0/p1 half 0) in chunks: the first matmuls
            # only need the first columns, so chunked arrival starts
            # compute early
            wr_q0 = {}
            for pi in range(2):
                wr = wpool.tile([128, 16 * 256], bf16, tag="wr",
                                name=f"wr_q{pi}_0")
                for ch in range(2):
                    nc.sync.dma_start(
                        out=wr[:, ds(ch * 2048, 2048)],
                        in_=P["wq_r"][:, ds(pi * 8192 + ch * 2048, 2048)],
                    )
                wr_q0[pi] = wr
            # x slabs: decoder columns first (all the Q pass needs), encoder
            # columns follow (first needed by the K pass ~70us in)
            xh = []
            for dt in range(NDT):
                xt = xpool.tile([128, T], bf16, tag=f"xh{dt}", name=f"xh{dt}")
                (nc.scalar if dt % 2 == 0 else nc.gpsimd).dma_start(
                    out=xt[:, ds(E, S)], in_=P["x_r"][:, ds(dt * T + E, S)]
                )
                xh.append(xt)
                if dt == 7:
                    # rope tables: q tables needed at the first rope (~35us);
                    # the k tables rotate into the same slots later (the
                    # slot WAR makes them wait for the last q rope)
                    csqc = tabpool.tile([128, S], bf16, tag="csc",
                                        name="csqc")
                    csqs = tabpool.tile([128, S], bf16, tag="css",
                                        name="csqs")
                    nc.scalar.dma_start(out=csqc[:, :], in_=P["csq_cos"][:, :])
                    nc.gpsimd.dma_start(out=csqs[:, :], in_=P["csq_sin"][:, :])
            for dt in range(NDT):
                (nc.scalar if dt % 2 == 0 else nc.gpsimd).dma_start(
                    out=xh[dt][:, ds(0, E)], in_=P["x_r"][:, ds(dt * T, E)]
                )
            fire_warmup()
            nc.gpsimd.dma_start(out=dmsk[:, :], in_=P["dmask"][:, :])
            nc.gpsimd.dma_start(out=onesb[:, :], in_=P["ones"][:, :])

            # RoPE: tables are full-height with the 64-row block duplicated
            # (cos) or sign-split (-sin; +sin), so every TensorTensor is
            # partition-aligned. The half-swap goes through an SBUF-SBUF DMA.
            def rope(buf, base, cos, sin, tag):
                swp = rtmp.tile([128, S], bf16, tag="swp", name=f"swp_{tag}")
                nc.gpsimd.dma_start(
                    out=swp[ds(0, 64), :], in_=buf[ds(64, 64), ds(base, S)]
                )
                nc.gpsimd.dma_start(
                    out=swp[ds(64, 64), :], in_=buf[ds(0, 64), ds(base, S)]
                )
                nc.vector.tensor_mul(swp[:, :], swp[:, :], sin[:, :])
                nc.vector.tensor_mul(
                    buf[:, ds(base, S)], buf[:, ds(base, S)], cos[:, :]
                )
                nc.vector.tensor_add(
                    buf[:, ds(base, S)], buf[:, ds(base, S)], swp[:, :]
                )

            # Q: two super-passes of two weight-passes each (8 psum banks),
            # dt OUTERMOST so x-slab consumption spreads over the whole
            # super-pass instead of demanding all 32 slabs in 34us
            for sp in range(2):
                ps = [
                    [
                        [ps1.tile([128, 512], fp32, tag="ps1",
                                  name=f"ps_q_{sp}_{pi}_{oi}_{cc}")
                         for cc in range(2)]
                        for oi in range(2)
                    ]
                    for pi in range(2)
                ]
                wrs = {}
                for half in range(2):
                    for pi in range(2):
                        p = 2 * sp + pi
                        if sp == 0 and half == 0:
                            wrs[(pi, 0)] = wr_q0[pi]
                        else:
                            wrs[(pi, half)] = wslab(
                                "wq_r", (p * 2 + half) * 16 * 256, 16 * 256,
                                f"wr_q{p}_{half}",
                            )
                for dt in range(NDT):
                    half, dtl = divmod(dt, 16)
                    for pi in range(2):
                        for oi in range(2):
                            for cc in range(2):
                                nc.tensor.matmul(
                                    ps[pi][oi][cc][:, :],
                                    wrs[(pi, half)][
                                        :, ds(dtl * 256 + oi * 128, 128)
                                    ],
                                    xh[dt][:, ds(E + cc * 512, 512)],
                                    start=(dt == 0),
                                    stop=(dt == NDT - 1),
                                )
                for pi in range(2):
                    p = 2 * sp + pi
                    for oi in range(2):
                        h = 2 * p + oi
                        for cc in range(2):
                            nc.scalar.copy(
                                qT[:, ds(h * S + cc * 512, 512)],
                                ps[pi][oi][cc][:, :],
                            )
                        rope(qT, h * S, csqc, csqs, f"q{h}")

            # k rope tables rotate into the q tables' slots; emitted here so
            # their slot-WAR wait (last q rope) sits on an idle queue moment
            cskc = tabpool.tile([128, S], bf16, tag="csc", name="cskc")
            csks = tabpool.tile([128, S], bf16, tag="css", name="csks")
            nc.scalar.dma_start(out=cskc[:, :], in_=P["csk_cos"][:, :])
            nc.gpsimd.dma_start(out=csks[:, :], in_=P["csk_sin"][:, :])

            # K: 4 passes x (2 o_tiles x 3 t_chunks), slabs in 16-dt halves
            for p in range(4):
                ps = [
                    [ps1.tile([128, 512], fp32, tag="ps1",
                              name=f"ps_k_{p}_{oi}_{cc}")
                     for cc in range(3)]
                    for oi in range(2)
                ]
                for half in range(2):
                    wr = wslab(
                        "wk_r", (p * 2 + half) * 16 * 256, 16 * 256,
                        f"wr_k{p}_{half}",
                    )
                    for dtl in range(16):
                        dt = half * 16 + dtl
                        for oi in range(2):
                            for cc in range(3):
                                nc.tensor.matmul(
                                    ps[oi][cc][:, :],
                                    wr[:, ds(dtl * 256 + oi * 128, 128)],
                                    xh[dt][:, ds(cc * 512, 512)],
                                    start=(dt == 0),
                                    stop=(dt == NDT - 1),
                                )
                for oi in range(2):
                    h = 2 * p + oi
                    for cc in range(3):
                        nc.scalar.copy(
                            kT[:, ds(h * T + cc * 512, 512)],
                            ps[oi][cc][:, :],
                        )
                    rope(kT, h * T + E, cskc, csks, f"k{h}")

            # V (x-stationary): 2 o_chunks x 2 t_groups of 6 tiles; weight
            # slabs re-streamed per t_group in two 16-dt halves
            for oc in range(2):
                for tg in range(2):
                    tb = tg * 6
                    psv = [ps1.tile([128, 512], fp32, tag="ps1",
                                    name=f"psv_{oc}_{tg}_{ti}")
                           for ti in range(6)]
                    for qr in range(4):
                        wr = wpool.tile(
                            [128, 8 * 512], bf16, tag="wr",
                            name=f"wr_v{oc}_{tg}_{qr}",
                        )
                        nc.sync.dma_start(
                            out=wr[:, :],
                            in_=P["wv_r"][
                                :, ds(oc * NDT * 512 + qr * 8 * 512, 8 * 512)
                            ],
                        )
                        for dtl in range(8):
                            dt = qr * 8 + dtl
                            for ti in range(6):
                                nc.tensor.matmul(
                                    psv[ti][:, :],
                                    xh[dt][:, ds((tb + ti) * 128, 128)],
                                    wr[:, ds(dtl * 512, 512)],
                                    start=(dt == 0),
                                    stop=(dt == NDT - 1),
                                )
                    for ti in range(6):
                        nc.scalar.copy(
                            vsb[:, ds((tb + ti) * O + oc * 512, 512)],
                            psv[ti][:, :],
                        )

          # wo pass-1 slabs (d-chunks 0-2) prefetch during attention; this
          # pool sits in the (dead) phase-1 x region so its DMAs only wait
          # on the last projection matmul, not on attention.
          with tc.tile_pool(name="wpre", bufs=1) as wpre:
            slab1 = [
                wpre.tile([128, 1536], bf16, tag=f"sl1_{kt}", name=f"slab1_{kt}")
                for kt in range(32)
            ]
            # stream pass-1 slabs during attention, all on gpsimd: its only
            # later work is the A2A triggers, so ring-credit waits here
            # never stall a compute-feeding queue (scalar's exp stream
            # stalled ~18us when half of these sat on it)
            for kt in range(32):
                nc.gpsimd.dma_start(
                    out=slab1[kt][:, :],
                    in_=P["wo_r"][:, ds(kt * 1536, 1536)],
                )

            # -------- phase 2: attention (softmax along partitions) --------
            # Per (sc, h): tile list = 4 encoder tiles + decoder tiles that
            # are not fully masked (sc0: 4, sc1: 8). Scores into paired psum
            # banks, exp over the pair, Z via ones-stationary matmul, PV
            # accumulation, then one reciprocal + one mul. After each head's
            # oT is ready it is staged to the A2A input (quarters duplicated
            # to ranks q and q+4); the A2A for a slice fires after its 8th
            # head.
            with tc.tile_pool(name="opool", bufs=3) as opool, \
                 tc.tile_pool(name="ppool", bufs=3) as ppool, \
                 tc.tile_pool(name="zpool", bufs=2) as zpool, \
                 tc.tile_pool(name="psS", bufs=3, space="PSUM") as psS, \
                 tc.tile_pool(name="psZ", bufs=1, space="PSUM") as psZ, \
                 tc.tile_pool(name="psV", bufs=1, space="PSUM") as psV:

              def tiles_for(sc):
                  # (tt, diag_j): tt indexes kT/vsb t-tiles; diag_j is the
                  # diagonal-mask index or None. Fully-masked tiles skipped.
                  lst = [(tt, None) for tt in range(4)]  # encoder
                  if sc == 0:
                      lst += [(4 + j, j) for j in range(4)]
                  else:
                      lst += [(tt, None) for tt in range(4, 8)]
                      lst += [(8 + j, j) for j in range(4)]
                  return lst

              def emit_A(sc, h, pbuf):
                  # diagonal-band tile j only contributes for q >= j*128;
                  # scores/exp/Z/PV all run on the reduced q-range
                  tl = tiles_for(sc)
                  for k0 in range(0, len(tl), 2):
                      pr = psS.tile([128, 1024], fp32, tag="psS",
                                    name=f"sc{sc}h{h}p{k0}")
                      full_pair = all(dj is None for _, dj in tl[k0:k0 + 2])
                      for half in range(2):
                          tt, dj = tl[k0 + half]
                          qo = 0 if dj is None else dj * 128
                          w = 512 - qo
                          nc.tensor.matmul(
                              pr[:, ds(half * 512 + qo, w)],
                              kT[:, ds(h * T + tt * 128, 128)],
                              qT[:, ds(h * S + sc * 512 + qo, w)],
                              start=True,
                              stop=True,
                          )
                      if full_pair:
                          nc.scalar.activation(
                              pbuf[:, ds(k0 * 512, 1024)], pr[:, :], AF.Exp
                          )
                      else:
                          for half in range(2):
                              tt, dj = tl[k0 + half]
                              qo = 0 if dj is None else dj * 128
                              w = 512 - qo
                              nc.scalar.activation(
                                  pbuf[:, ds((k0 + half) * 512 + qo, w)],
                                  pr[:, ds(half * 512 + qo, w)],
                                  AF.Exp,
                              )
                      # causal zeroing of the diagonal staircase chunk (the
                      # first 128 computed columns of a diag tile), post-exp
                      for half in range(2):
                          tt, dj = tl[k0 + half]
                          if dj is not None:
                              qo = dj * 128
                              nc.vector.tensor_mul(
                                  pbuf[:, ds((k0 + half) * 512 + qo, 128)],
                                  pbuf[:, ds((k0 + half) * 512 + qo, 128)],
                                  dmsk[:, ds(dj * 512 + qo, 128)],
                              )

              def emit_B(sc, h, pbuf):
                  tl = tiles_for(sc)
                  n = len(tl)
                  zp = psZ.tile([128, 512], fp32, tag="psZ", name=f"z{sc}{h}")
                  for k, (tt, dj) in enumerate(tl):
                      qo = 0 if dj is None else dj * 128
                      nc.tensor.matmul(
                          zp[:, ds(qo, 512 - qo)],
                          onesb[:, :],
                          pbuf[:, ds(k * 512 + qo, 512 - qo)],
                          start=(k == 0),
                          stop=(k == n - 1),
                      )
                  zr = zpool.tile([128, 512], fp32, tag="zr", name=f"zr{sc}{h}")
                  nc.vector.reciprocal_approx_fast(zr[:, :], zp[:, :])
                  pv = psV.tile([128, 512], fp32, tag="psV", name=f"pv{sc}{h}")
                  for k, (tt, dj) in enumerate(tl):
                      qo = 0 if dj is None else dj * 128
                      nc.tensor.matmul(
                          pv[:, ds(qo, 512 - qo)],
                          vsb[:, ds(tt * O + h * 128, 128)],
                          pbuf[:, ds(k * 512 + qo, 512 - qo)],
                          start=(k == 0),
                          stop=(k == n - 1),
                      )
                  ot = opool.tile([128, 512], bf16, tag="oT",
                                  name=f"oT{sc}{h}")
                  nc.vector.tensor_mul(ot[:, :], pv[:, :], zr[:, :])
                  # stage this head's slice into the A2A input: token quarter
                  # q goes to dest-rank rows q*128 (batch-group 0 dests) and
                  # (4+q)*128 (group 1); one 3-dim DMA per dest group
                  src = ot[:, :].rearrange("dh (q t) -> dh q t", q=4)
                  for g in range(2):
                      dst = a2a_in[sc][
                          ds(g * 512, 512), ds(h * 128, 128)
                      ].rearrange("(q dh) t -> dh q t", q=4)
                      nc.sync.dma_start(out=dst, in_=src)

              def fire_a2a(sc):
                  if no_collective:
                      nc.gpsimd.dma_start(
                          out=a2a_out[sc][:, :], in_=a2a_in[sc][:, :]
                      )
                  else:
                      nc.gpsimd.collective_compute(
                          "AllToAll",
                          mybir.AluOpType.bypass,
                          replica_groups=A2A_GROUP,
                          ins=[a2a_in[sc][:, :].opt()],
                          outs=[a2a_out[sc][:, :].opt()],
                      )

              # software pipeline: 2-head lookahead on A emits
              pend = []
              b_count = 0
              for sc in range(2):
                  for h in range(H):
                      pbuf = ppool.tile(
                          [128, 12 * 512], bf16, tag="p", name=f"pb{sc}{h}"
                      )
                      emit_A(sc, h, pbuf)
                      pend.append((sc, h, pbuf))
                      if len(pend) == 3:
                          s0, h0, pb0 = pend.pop(0)
                          emit_B(s0, h0, pb0)
                          b_count += 1
                          if b_count == 8:
                              fire_a2a(0)
              for s0, h0, pb0 in pend:
                  emit_B(s0, h0, pb0)
                  b_count += 1
              # A2A#2 first: the gpsimd queue sits in the A2A#1 completion
              # wait, and a collective op also waits for completion, so any
              # load placed between the two triggers would delay the second
              # trigger past its own transfer time.
              fire_a2a(1)

            # ---------------- phase 3: wo (y = oT_full.T @ woT) ----------
            # pass 1: d-chunks 0-2 (slabs resident from prefetch); sc0 first
            # (independent of A2A#2, its ~50us hides the collective), then
            # sc1. pass 2: d-chunk 3 with freshly streamed slabs, jointly.
            with tc.tile_pool(name="wo2", bufs=8) as wo2, \
                 tc.tile_pool(name="rtp", bufs=1) as rtp, \
                 tc.tile_pool(name="ypool", bufs=3) as ypool, \
                 tc.tile_pool(name="psW", bufs=6, space="PSUM") as psW, \
                 tc.tile_pool(name="psW2", bufs=2, space="PSUM") as psW2:
                rt0 = [
                    rtp.tile([128, 1024], bf16, tag=f"rt0_{i}",
                             name=f"rt0_{i}")
                    for i in range(8)
                ]
                rt1 = [
                    rtp.tile([128, 1024], bf16, tag=f"rt1_{i}",
                             name=f"rt1_{i}")
                    for i in range(8)
                ]
                # sc0 recv split over scalar+sync (both near-free once the
                # exps/staging end), consumption order so the first k-tiles
                # arrive first
                for k, i in enumerate((0, 4, 1, 5, 2, 6, 3, 7)):
                    (nc.scalar if k % 2 == 0 else nc.sync).dma_start(
                        out=rt0[i][:, :],
                        in_=a2a_out[0][ds(i * 128, 128), :],
                    )
                # sc1 recv: gpsimd is parked right behind the A2A#2
                # completion wait; sync joins it after the rt0 loads
                for k, i in enumerate((0, 4, 1, 5, 2, 6, 3, 7)):
                    (nc.gpsimd if k % 2 == 0 else nc.sync).dma_start(
                        out=rt1[i][:, :], in_=a2a_out[1][ds(i * 128, 128), :]
                    )
                rts = [rt0, rt1]

                def ycopy(sc, b, dc, pw):
                    ys = ypool.tile([128, 512], bf16, tag="ys",
                                    name=f"ys{sc}{b}{dc}")
                    nc.scalar.copy(ys[:, :], pw[:, :])
                    nc.sync.dma_start(
                        out=out[ds(sc * 256 + b * 128, 128),
                                ds(dc * 512, 512)],
                        in_=ys[:, :],
                    )

                def wo_block(sc):
                    # psum[t, d] accumulated over all 32 k-tiles
                    pw = {
                        (b, dc): psW.tile([128, 512], fp32, tag="psW",
                                          name=f"pw{sc}{b}{dc}")
                        for b in range(2) for dc in range(3)
                    }
                    for kt in range(32):
                        ig, h = divmod(kt, 8)
                        for b in range(2):
                            stat = rts[sc][b * 4 + ig][:, ds(h * 128, 128)]
                            for dc in range(3):
                                nc.tensor.matmul(
                                    pw[(b, dc)][:, :],
                                    stat,
                                    slab1[kt][:, ds(dc * 512, 512)],
                                    start=(kt == 0),
                                    stop=(kt == 31),
                                )
                    for b in range(2):
                        for dc in range(3):
                            ycopy(sc, b, dc, pw[(b, dc)])

                # pass-2-sc0 slabs for kt 0-15, emitted before the sc0 pass
                # so their transfers (on the then-idle scalar queue) finish
                # before the matmuls need them
                slab2a = {}
                for kt in range(8):
                    sl = wo2.tile([128, 512], bf16, tag="sl2",
                                  name=f"slab2a_{kt}")
                    nc.scalar.dma_start(
                        out=sl[:, :],
                        in_=P["wo_r"][:, ds(49152 + kt * 512, 512)],
                    )
                    slab2a[kt] = sl
                # pass 1 sc0 (resident slabs) — independent of A2A#2
                wo_block(0)
                # pass-2 sc0 for kt 0-15: more A2A#2-independent work, so a
                # slow collective (and the rt1 transfers) stay hidden; its
                # 2 psum chains live in their own pool and pause across
                # sc1-pass-1 (6 rotating + 2 held = 8 banks)
                pw2 = {
                    (0, b): psW2.tile([128, 512], fp32, tag="psW2",
                                      name=f"p2w0{b}")
                    for b in range(2)
                }
                for kt in range(8):
                    ig, h = divmod(kt, 8)
                    for b in range(2):
                        stat = rts[0][b * 4 + ig][:, ds(h * 128, 128)]
                        nc.tensor.matmul(
                            pw2[(0, b)][:, :],
                            stat,
                            slab2a[kt][:, :],
                            start=(kt == 0),
                            stop=False,
                        )
                # pass 1 sc1 (resident slabs)
                wo_block(1)
                # tail: kt 16-31 for sc0's pass-2 chains + all of sc1's
                # pass 2; slab slots for kt 0-15 are re-streamed for sc1
                # (gpsimd/sync are parked behind the A2A#2 completion, so
                # none of this contends with the collective). The sc1
                # chains are created here so they rotate into wo_block(1)'s
                # freed banks, not into the still-open sc0 chains.
                for b in range(2):
                    pw2[(1, b)] = psW.tile([128, 512], fp32, tag="psW",
                                           name=f"p2w1{b}")
                for kt in range(8, 32):
                    sl = wo2.tile([128, 512], bf16, tag="sl2",
                                  name=f"slab2b_{kt}")
                    (nc.gpsimd if kt % 2 == 0 else nc.sync).dma_start(
                        out=sl[:, :],
                        in_=P["wo_r"][:, ds(49152 + kt * 512, 512)],
                    )
                    ig, h = divmod(kt, 8)
                    for b in range(2):
                        stat = rts[0][b * 4 + ig][:, ds(h * 128, 128)]
                        nc.tensor.matmul(
                            pw2[(0, b)][:, :],
                            stat,
                            sl[:, :],
                            start=False,
                            stop=(kt == 31),
                        )
                    for b in range(2):
                        stat = rts[1][b * 4 + ig][:, ds(h * 128, 128)]
                        nc.tensor.matmul(
                            pw2[(1, b)][:, :],
                            stat,
                            sl[:, :],
                            start=(kt == 8),
                            stop=False,
                        )
                for b in range(2):
                    ycopy(0, b, 3, pw2[(0, b)])
                for kt in range(8):
                    sl = wo2.tile([128, 512], bf16, tag="sl2",
                                  name=f"slab2c_{kt}")
                    (nc.gpsimd if kt % 2 == 0 else nc.sync).dma_start(
                        out=sl[:, :],
                        in_=P["wo_r"][:, ds(49152 + kt * 512, 512)],
                    )
                    ig, h = divmod(kt, 8)
                    for b in range(2):
                        stat = rts[1][b * 4 + ig][:, ds(h * 128, 128)]
                        nc.tensor.matmul(
                            pw2[(1, b)][:, :],
                            stat,
                            sl[:, :],
                            start=False,
                            stop=(kt == 7),
                        )
                for b in range(2):
                    ycopy(1, b, 3, pw2[(1, b)])


def _prep_in_maps(x, freqs_cos, freqs_sin, mask, encoder_output, wq, wk, wv, wo):
    x = np.asarray(x, np.float32)
    encoder_output = np.asarray(encoder_output, np.float32)
    freqs_cos = np.asarray(freqs_cos, np.float32)
    freqs_sin = np.asarray(freqs_sin, np.float32)
    wq = np.asarray(wq, np.float32)
    wk = np.asarray(wk, np.float32)
    wv = np.asarray(wv, np.float32)
    wo = np.asarray(wo, np.float32)

    def perm(w):  # deinterleave rope pairs per head: even dims first
        w4 = w.reshape(H, 64, 2, D)
        return np.ascontiguousarray(w4.transpose(0, 2, 1, 3)).reshape(O, D)

    def slab256(wT):  # [D, O] -> [128, 4*32*256]: pass p, dt n, col c
        w4 = wT.reshape(NDT, 128, 4, 256)            # [n, part, p, c]
        return np.ascontiguousarray(
            w4.transpose(1, 2, 0, 3)
        ).reshape(128, NDT * O)

    def slab512(wT):  # [D, O] -> [128, 2*32*512]: oc, dt n, col c
        w4 = wT.reshape(NDT, 128, 2, 512)
        return np.ascontiguousarray(
            w4.transpose(1, 2, 0, 3)
        ).reshape(128, NDT * O)

    alpha = 1.0 / np.sqrt(DH)
    cosT = freqs_cos.T  # [64, S]
    sinT = freqs_sin.T
    csq_cos = (np.concatenate([cosT, cosT], 0) * alpha).astype(BF16)
    csq_sin = (np.concatenate([-sinT, sinT], 0) * alpha).astype(BF16)
    csk_cos = np.concatenate([cosT, cosT], 0).astype(BF16)
    csk_sin = np.concatenate([-sinT, sinT], 0).astype(BF16)

    # 4 diagonal-band keep-masks (0/1, applied post-exp):
    # dmask[t, j*512+s] = 0 if s < t + j*128 else 1
    t_i = np.arange(128)[:, None]
    s_i = np.arange(512)[None, :]
    dmask = np.concatenate(
        [np.where(s_i < t_i + j * 128, 0.0, 1.0) for j in range(4)], axis=1
    ).astype(BF16)
    ones = np.ones((128, 128), BF16)

    # woT slabs: full wo.T (k = head*128+dh on partitions per k-tile), this
    # core's d-half, split into two 1024-wide passes
    woT = np.ascontiguousarray(wo.T).reshape(32, 128, D)  # [kt, dh, dout]

    in_maps = []
    for c in range(8):
        g, r = divmod(c, 4)
        dhalf = g
        sl = slice(r * O, (r + 1) * O)
        xhat = np.concatenate([encoder_output[g], x[g]], axis=0)  # [T, D]
        xhatT = xhat.T.astype(BF16)                               # [D, T]
        x_r = np.ascontiguousarray(
            xhatT.reshape(NDT, 128, T).transpose(1, 0, 2)
        ).reshape(128, NDT * T)
        wqT = perm(wq[sl]).T.astype(BF16)   # [D, O]
        wkT = perm(wk[sl]).T.astype(BF16)
        wvT = wv[sl].T.astype(BF16)
        wo_c = woT[:, :, dhalf * DHALF:(dhalf + 1) * DHALF]  # [32,128,2048]
        wo_a = np.ascontiguousarray(
            wo_c[:, :, :1536].transpose(1, 0, 2)
        ).reshape(128, 32 * 1536)
        wo_b = np.ascontiguousarray(
            wo_c[:, :, 1536:].transpose(1, 0, 2)
        ).reshape(128, 32 * 512)
        wo_r = np.concatenate([wo_a, wo_b], axis=1).astype(BF16)
        in_maps.append(
            {
                "x_r": x_r,
                "wq_r": slab256(wqT),
                "wk_r": slab256(wkT),
                "wv_r": slab512(wvT),
                "wo_r": wo_r,
                "csq_cos": csq_cos,
                "csq_sin": csq_sin,
                "csk_cos": csk_cos,
                "csk_sin": csk_sin,
                "dmask": dmask,
                "ones": ones,
            }
        )
    return in_maps


def _gather(outs):
    full = np.zeros((2, S, D), np.float32)
    for c in range(8):
        g, q = divmod(c, 4)
        dhalf = g
        o = np.asarray(outs[c]).astype(np.float32)  # [512, 2048]
        for sc in range(2):
            for b in range(2):
                rows = o[sc * 256 + b * 128: sc * 256 + b * 128 + 128]
                full[b, sc * 512 + q * 128: sc * 512 + q * 128 + 128,
                     dhalf * DHALF:(dhalf + 1) * DHALF] = rows
    return full


def kernel(x, start_pos, freqs_cos, freqs_sin, mask, encoder_output, wq, wk, wv, wo):
    global LAST_EXEC_NS
    from concourse.bass_utils import run_bass_kernel_spmd

    if "nc" not in _CACHE:
        _CACHE["nc"] = _build()
    nc = _CACHE["nc"]

    in_maps = _prep_in_maps(
        x, freqs_cos, freqs_sin, mask, encoder_output, wq, wk, wv, wo
    )
    res = run_bass_kernel_spmd(nc, in_maps, core_ids=list(range(8)))
    LAST_EXEC_NS = res.exec_time_ns
    return _gather([res.results[c]["out"] for c in range(8)])


# revision 46
# speedup vs baseline: 1.1119x; 1.0172x over previous
"""Trainium2 Bass kernel for nn_Attention (dense transformer attention block).

Full inputs -> full output. Internally: 8 NeuronCores, 2 data-parallel groups
(batch) x 4-way tensor-parallel (heads). Each core computes 8 heads for one
batch element. The wo projection is redistributed with a single 8-rank
AllToAll per 512-token slice: each core ships its heads' attention output
(oT, feature-major) for token-quarter q to ranks q and q+4, and afterwards
holds the FULL 4096-feature oT for one 128-token quarter of each batch --
it then computes y for those rows over one 2048-wide d-half (group 0 takes
d 0:2048, group 1 d 2048:4096). No ReduceScatter; the A2A moves 2 MB/rank
at ~24 us (mesh) and the sc1 A2A hides under the sc0 wo pass.

Compute in bf16 on the TensorEngine (fp32 matmul is 4x slower), fp32 PSUM
accumulation. All operand layouts are pre-rearranged host-side so every
device DMA is a contiguous per-partition block:
  - projections:  qT/kT = (w-tile).T @ xhatT-tile   -> [feature, seq] layout
  - scores:       scoresT[t, s] = kT-tile.T @ qT    (softmax along partitions)
  - Z:            ones[128,128].T @ probs           -> Z broadcast to all rows
  - PV:           oT[dh, s] = v-tile.T @ probsT
  - wo:           y[t, d] = oT-recv-tile.T @ woT-slab (K=4096 in one psum)
Causal structure is exploited: score tiles that are fully masked are skipped
(scores/exp/Z/PV), and only the 4 diagonal-band tiles per query chunk get a
post-exp 0/1 multiply (from 4 precomputed [128,512] masks). RoPE pairs are
deinterleaved host-side (even dims first); the 1/sqrt(128) score scale is
folded into the q-side cos/sin tables.
"""

import sys

import numpy as np

for _p in ("/opt/trn_rl_repo",):
    if _p not in sys.path:
        sys.path.insert(0, _p)

import ml_dtypes

BF16 = ml_dtypes.bfloat16

D = 4096      # model dim
S = 1024      # decoder sequence length
E = 512       # encoder length
T = E + S     # total key length
H = 8         # heads per core (32 total / 4-way TP)
DH = 128      # head dim
O = H * DH    # per-core projection width = 1024
NDT = D // 128
NEG = -1e9
A2A_GROUP = [[0, 1, 2, 3, 4, 5, 6, 7]]
DHALF = 2048  # per-core output d-half width

_CACHE = {}
LAST_EXEC_NS = None


def _build(no_collective=False):
    import concourse.mybir as mybir
    import concourse.tile as tile
    from concourse import bacc

    bf16 = mybir.dt.bfloat16

    nc = bacc.Bacc(
        "TRN2",
        target_bir_lowering=False,
        debug=False,
        num_devices=8,
    )

    P = {}
    for name, shape in [
        ("x_r", [128, NDT * T]),        # xhatT slabs: cols dt*T + t
        ("wq_r", [128, NDT * O]),       # Q pass slabs: cols p*8192 + n*256 + c
        ("wk_r", [128, NDT * O]),       # K pass slabs: same geometry
        ("wv_r", [128, NDT * O]),       # V slabs: cols oc*16384 + n*512 + c
        ("wo_r", [128, 2 * 32 * 1024]), # woT slabs: cols p*32768 + kt*1024 + d
        ("csq_cos", [128, S]),
        ("csq_sin", [128, S]),
        ("csk_cos", [128, S]),
        ("csk_sin", [128, S]),
        ("dmask", [128, 4 * 512]),      # 4 diagonal-band masks
        ("ones", [128, 128]),
    ]:
        P[name] = nc.declare_dram_parameter(name, shape, bf16, isOutput=False)
    # rows: sc*256 + b*128 + t ; cols: d within this core's d-half
    out = nc.declare_dram_parameter("out", [512, DHALF], bf16, isOutput=True)

    with tile.TileContext(nc) as tc:
        _emit(nc, tc, P, out, no_collective=no_collective)
    nc.compile()
    return nc


def _emit(nc, tc, P, out, no_collective=False):
    import concourse.mybir as mybir
    from concourse.bass import ds

    bf16 = mybir.dt.bfloat16
    fp32 = mybir.dt.float32
    AF = mybir.ActivationFunctionType

    with tc.tile_pool(name="res", bufs=1) as res, \
         tc.tile_pool(name="dram", bufs=1, space="DRAM") as dram:
        onesb = res.tile([128, 128], bf16, tag="onesb")
        dmsk = res.tile([128, 4 * 512], bf16, tag="dmsk")  # 0/1 keep masks

        # A2A staging: rows j*128 + dh (j = dest rank), cols h*128 + t
        a2a_in = [
            dram.tile([1024, 1024], bf16, tag=f"ai{sc}", name=f"a2a_in{sc}")
            for sc in range(2)
        ]
        a2a_out = [
            dram.tile([1024, 1024], bf16, tag=f"ao{sc}", name=f"a2a_out{sc}")
            for sc in range(2)
        ]

        # dmsk/onesb are loaded late in phase 1 (see below) so they don't
        # delay the startup-critical x/weight streams

        # tiny warm-up collective (fired a little into phase 1): absorbs the
        # ~11us first-collective spin-up on the CC stream during projections
        warm_in = dram.tile([8, 128], bf16, tag="wi", name="warm_in")
        warm_out = dram.tile([8, 128], bf16, tag="wo", name="warm_out")

        def fire_warmup():
            if no_collective:
                return
            nc.gpsimd.dma_start(out=warm_in[:, :], in_=P["ones"][ds(0, 8), :])
            nc.gpsimd.collective_compute(
                "AllToAll",
                mybir.AluOpType.bypass,
                replica_groups=A2A_GROUP,
                ins=[warm_in[:, :].opt()],
                outs=[warm_out[:, :].opt()],
            )

        with tc.tile_pool(name="qkv", bufs=1) as qkv:
          qT = qkv.tile([128, H * S], bf16, tag="qT")     # cols h*S + s
          kT = qkv.tile([128, H * T], bf16, tag="kT")     # cols h*T + t
          vsb = qkv.tile([128, 12 * O], bf16, tag="vsb")  # cols tt*O + o

          # ---------------- phase 1: projections + rope ----------------
          with tc.tile_pool(name="xpool", bufs=1) as xpool, \
               tc.tile_pool(name="tabpool", bufs=1) as tabpool, \
               tc.tile_pool(name="wpool", bufs=4) as wpool, \
               tc.tile_pool(name="rtmp", bufs=2) as rtmp, \
               tc.tile_pool(name="ps1", bufs=8, space="PSUM") as ps1:
            # first Q weight half-slab ahead of everything on the sync queue
            def wslab(src, off, n, name):
                wr = wpool.tile([128, n], bf16, tag="wr", name=name)
                nc.sync.dma_start(out=wr[:, :], in_=P[src][:, ds(off, n)])
                return wr

            # first two Q slabs (p0/p1 half 0) in chunks: the first matmuls
            # only need the first columns, so chunked arrival starts
            # compute early
            wr_q0 = {}
            for pi in range(2):
                wr_q0[pi] = wpool.tile([128, 16 * 256], bf16, tag="wr",
                                       name=f"wr_q{pi}_0")
            for ch in range(4):
                for pi in range(2):
                    nc.sync.dma_start(
                        out=wr_q0[pi][:, ds(ch * 1024, 1024)],
                        in_=P["wq_r"][:, ds(pi * 8192 + ch * 1024, 1024)],
                    )
            # x slabs: decoder columns first (all the Q pass needs), encoder
            # columns follow (first needed by the K pass ~70us in)
            xh = []
            for dt in range(NDT):
                xt = xpool.tile([128, T], bf16, tag=f"xh{dt}", name=f"xh{dt}")
                (nc.scalar if dt % 2 == 0 else nc.gpsimd).dma_start(
                    out=xt[:, ds(E, S)], in_=P["x_r"][:, ds(dt * T + E, S)]
                )
                xh.append(xt)
                if dt == 7:
                    # rope tables: q tables needed at the first rope (~35us);
                    # the k tables rotate into the same slots later (the
                    # slot WAR makes them wait for the last q rope)
                    csqc = tabpool.tile([128, S], bf16, tag="csc",
                                        name="csqc")
                    csqs = tabpool.tile([128, S], bf16, tag="css",
                                        name="csqs")
                    nc.scalar.dma_start(out=csqc[:, :], in_=P["csq_cos"][:, :])
                    nc.gpsimd.dma_start(out=csqs[:, :], in_=P["csq_sin"][:, :])
            for dt in range(NDT):
                (nc.scalar if dt % 2 == 0 else nc.gpsimd).dma_start(
                    out=xh[dt][:, ds(0, E)], in_=P["x_r"][:, ds(dt * T, E)]
                )
            fire_warmup()
            nc.gpsimd.dma_start(out=dmsk[:, :], in_=P["dmask"][:, :])
            nc.gpsimd.dma_start(out=onesb[:, :], in_=P["ones"][:, :])

            # RoPE: tables are full-height with the 64-row block duplicated
            # (cos) or sign-split (-sin; +sin), so every TensorTensor is
            # partition-aligned. The half-swap goes through an SBUF-SBUF DMA.
            def rope(buf, base, cos, sin, tag):
                swp = rtmp.tile([128, S], bf16, tag="swp", name=f"swp_{tag}")
                nc.gpsimd.dma_start(
                    out=swp[ds(0, 64), :], in_=buf[ds(64, 64), ds(base, S)]
                )
                nc.gpsimd.dma_start(
                    out=swp[ds(64, 64), :], in_=buf[ds(0, 64), ds(base, S)]
                )
                nc.vector.tensor_mul(swp[:, :], swp[:, :], sin[:, :])
                nc.vector.tensor_mul(
                    buf[:, ds(base, S)], buf[:, ds(base, S)], cos[:, :]
                )
                nc.vector.tensor_add(
                    buf[:, ds(base, S)], buf[:, ds(base, S)], swp[:, :]
                )

            # Q: two super-passes of two weight-passes each (8 psum banks),
            # dt OUTERMOST so x-slab consumption spreads over the whole
            # super-pass instead of demanding all 32 slabs in 34us
            for sp in range(2):
                ps = [
                    [
                        [ps1.tile([128, 512], fp32, tag="ps1",
                                  name=f"ps_q_{sp}_{pi}_{oi}_{cc}")
                         for cc in range(2)]
                        for oi in range(2)
                    ]
                    for pi in range(2)
                ]
                wrs = {}
                for half in range(2):
                    for pi in range(2):
                        p = 2 * sp + pi
                        if sp == 0 and half == 0:
                            wrs[(pi, 0)] = wr_q0[pi]
                        else:
                            wrs[(pi, half)] = wslab(
                                "wq_r", (p * 2 + half) * 16 * 256, 16 * 256,
                                f"wr_q{p}_{half}",
                            )
                for dt in range(NDT):
                    half, dtl = divmod(dt, 16)
                    for pi in range(2):
                        for oi in range(2):
                            for cc in range(2):
                                nc.tensor.matmul(
                                    ps[pi][oi][cc][:, :],
                                    wrs[(pi, half)][
                                        :, ds(dtl * 256 + oi * 128, 128)
                                    ],
                                    xh[dt][:, ds(E + cc * 512, 512)],
                                    start=(dt == 0),
                                    stop=(dt == NDT - 1),
                                )
                for pi in range(2):
                    p = 2 * sp + pi
                    for oi in range(2):
                        h = 2 * p + oi
                        for cc in range(2):
                            nc.scalar.copy(
                                qT[:, ds(h * S + cc * 512, 512)],
                                ps[pi][oi][cc][:, :],
                            )
                        rope(qT, h * S, csqc, csqs, f"q{h}")

            # k rope tables rotate into the q tables' slots; emitted here so
            # their slot-WAR wait (last q rope) sits on an idle queue moment
            cskc = tabpool.tile([128, S], bf16, tag="csc", name="cskc")
            csks = tabpool.tile([128, S], bf16, tag="css", name="csks")
            nc.scalar.dma_start(out=cskc[:, :], in_=P["csk_cos"][:, :])
            nc.gpsimd.dma_start(out=csks[:, :], in_=P["csk_sin"][:, :])

            # K: 4 passes x (2 o_tiles x 3 t_chunks), slabs in 16-dt halves
            for p in range(4):
                ps = [
                    [ps1.tile([128, 512], fp32, tag="ps1",
                              name=f"ps_k_{p}_{oi}_{cc}")
                     for cc in range(3)]
                    for oi in range(2)
                ]
                for half in range(2):
                    wr = wslab(
                        "wk_r", (p * 2 + half) * 16 * 256, 16 * 256,
                        f"wr_k{p}_{half}",
                    )
                    for dtl in range(16):
                        dt = half * 16 + dtl
                        for oi in range(2):
                            for cc in range(3):
                                nc.tensor.matmul(
                                    ps[oi][cc][:, :],
                                    wr[:, ds(dtl * 256 + oi * 128, 128)],
                                    xh[dt][:, ds(cc * 512, 512)],
                                    start=(dt == 0),
                                    stop=(dt == NDT - 1),
                                )
                for oi in range(2):
                    h = 2 * p + oi
                    for cc in range(3):
                        nc.scalar.copy(
                            kT[:, ds(h * T + cc * 512, 512)],
                            ps[oi][cc][:, :],
                        )
                    rope(kT, h * T + E, cskc, csks, f"k{h}")

            # V (x-stationary): 2 o_chunks x 2 t_groups of 6 tiles; weight
            # slabs re-streamed per t_group in two 16-dt halves
            for oc in range(2):
                for tg in range(2):
                    tb = tg * 6
                    psv = [ps1.tile([128, 512], fp32, tag="ps1",
                                    name=f"psv_{oc}_{tg}_{ti}")
                           for ti in range(6)]
                    for qr in range(4):
                        wr = wpool.tile(
                            [128, 8 * 512], bf16, tag="wr",
                            name=f"wr_v{oc}_{tg}_{qr}",
                        )
                        nc.sync.dma_start(
                            out=wr[:, :],
                            in_=P["wv_r"][
                                :, ds(oc * NDT * 512 + qr * 8 * 512, 8 * 512)
                            ],
                        )
                        for dtl in range(8):
                            dt = qr * 8 + dtl
                            for ti in range(6):
                                nc.tensor.matmul(
                                    psv[ti][:, :],
                                    xh[dt][:, ds((tb + ti) * 128, 128)],
                                    wr[:, ds(dtl * 512, 512)],
                                    start=(dt == 0),
                                    stop=(dt == NDT - 1),
                                )
                    for ti in range(6):
                        nc.scalar.copy(
                            vsb[:, ds((tb + ti) * O + oc * 512, 512)],
                            psv[ti][:, :],
                        )

          # wo pass-1 slabs (d-chunks 0-2) prefetch during attention; this
          # pool sits in the (dead) phase-1 x region so its DMAs only wait
          # on the last projection matmul, not on attention.
          with tc.tile_pool(name="wpre", bufs=1) as wpre:
            slab1 = [
                wpre.tile([128, 1536], bf16, tag=f"sl1_{kt}", name=f"slab1_{kt}")
                for kt in range(32)
            ]
            # stream pass-1 slabs during attention, all on gpsimd: its only
            # later work is the A2A triggers, so ring-credit waits here
            # never stall a compute-feeding queue (scalar's exp stream
            # stalled ~18us when half of these sat on it)
            for kt in range(32):
                nc.gpsimd.dma_start(
                    out=slab1[kt][:, :],
                    in_=P["wo_r"][:, ds(kt * 1536, 1536)],
                )

            # -------- phase 2: attention (softmax along partitions) --------
            # Per (sc, h): tile list = 4 encoder tiles + decoder tiles that
            # are not fully masked (sc0: 4, sc1: 8). Scores into paired psum
            # banks, exp over the pair, Z via ones-stationary matmul, PV
            # accumulation, then one reciprocal + one mul. After each head's
            # oT is ready it is staged to the A2A input (quarters duplicated
            # to ranks q and q+4); the A2A for a slice fires after its 8th
            # head.
            with tc.tile_pool(name="opool", bufs=3) as opool, \
                 tc.tile_pool(name="ppool", bufs=3) as ppool, \
                 tc.tile_pool(name="zpool", bufs=2) as zpool, \
                 tc.tile_pool(name="psS", bufs=3, space="PSUM") as psS, \
                 tc.tile_pool(name="psZ", bufs=1, space="PSUM") as psZ, \
                 tc.tile_pool(name="psV", bufs=1, space="PSUM") as psV:

              def tiles_for(sc):
                  # (tt, diag_j): tt indexes kT/vsb t-tiles; diag_j is the
                  # diagonal-mask index or None. Fully-masked tiles skipped.
                  lst = [(tt, None) for tt in range(4)]  # encoder
                  if sc == 0:
                      lst += [(4 + j, j) for j in range(4)]
                  else:
                      lst += [(tt, None) for tt in range(4, 8)]
                      lst += [(8 + j, j) for j in range(4)]
                  return lst

              def emit_A(sc, h, pbuf):
                  # diagonal-band tile j only contributes for q >= j*128;
                  # scores/exp/Z/PV all run on the reduced q-range
                  tl = tiles_for(sc)
                  for k0 in range(0, len(tl), 2):
                      pr = psS.tile([128, 1024], fp32, tag="psS",
                                    name=f"sc{sc}h{h}p{k0}")
                      full_pair = all(dj is None for _, dj in tl[k0:k0 + 2])
                      for half in range(2):
                          tt, dj = tl[k0 + half]
                          qo = 0 if dj is None else dj * 128
                          w = 512 - qo
                          nc.tensor.matmul(
                              pr[:, ds(half * 512 + qo, w)],
                              kT[:, ds(h * T + tt * 128, 128)],
                              qT[:, ds(h * S + sc * 512 + qo, w)],
                              start=True,
                              stop=True,
                          )
                      if full_pair:
                          nc.scalar.activation(
                              pbuf[:, ds(k0 * 512, 1024)], pr[:, :], AF.Exp
                          )
                      else:
                          for half in range(2):
                              tt, dj = tl[k0 + half]
                              qo = 0 if dj is None else dj * 128
                              w = 512 - qo
                              nc.scalar.activation(
                                  pbuf[:, ds((k0 + half) * 512 + qo, w)],
                                  pr[:, ds(half * 512 + qo, w)],
                                  AF.Exp,
                              )
                      # causal zeroing of the diagonal staircase chunk (the
                      # first 128 computed columns of a diag tile), post-exp
                      for half in range(2):
                          tt, dj = tl[k0 + half]
                          if dj is not None:
                              qo = dj * 128
                              nc.vector.tensor_mul(
                                  pbuf[:, ds((k0 + half) * 512 + qo, 128)],
                                  pbuf[:, ds((k0 + half) * 512 + qo, 128)],
                                  dmsk[:, ds(dj * 512 + qo, 128)],
                              )

              def emit_B(sc, h, pbuf):
                  tl = tiles_for(sc)
                  n = len(tl)
                  zp = psZ.tile([128, 512], fp32, tag="psZ", name=f"z{sc}{h}")
                  for k, (tt, dj) in enumerate(tl):
                      qo = 0 if dj is None else dj * 128
                      nc.tensor.matmul(
                          zp[:, ds(qo, 512 - qo)],
                          onesb[:, :],
                          pbuf[:, ds(k * 512 + qo, 512 - qo)],
                          start=(k == 0),
                          stop=(k == n - 1),
                      )
                  zr = zpool.tile([128, 512], fp32, tag="zr", name=f"zr{sc}{h}")
                  nc.vector.reciprocal_approx_fast(zr[:, :], zp[:, :])
                  pv = psV.tile([128, 512], fp32, tag="psV", name=f"pv{sc}{h}")
                  for k, (tt, dj) in enumerate(tl):
                      qo = 0 if dj is None else dj * 128
                      nc.tensor.matmul(
                          pv[:, ds(qo, 512 - qo)],
                          vsb[:, ds(tt * O + h * 128, 128)],
                          pbuf[:, ds(k * 512 + qo, 512 - qo)],
                          start=(k == 0),
                          stop=(k == n - 1),
                      )
                  ot = opool.tile([128, 512], bf16, tag="oT",
                                  name=f"oT{sc}{h}")
                  nc.vector.tensor_mul(ot[:, :], pv[:, :], zr[:, :])
                  # stage this head's slice into the A2A input: token quarter
                  # q goes to dest-rank rows q*128 (batch-group 0 dests) and
                  # (4+q)*128 (group 1); one 3-dim DMA per dest group
                  src = ot[:, :].rearrange("dh (q t) -> dh q t", q=4)
                  for g in range(2):
                      dst = a2a_in[sc][
                          ds(g * 512, 512), ds(h * 128, 128)
                      ].rearrange("(q dh) t -> dh q t", q=4)
                      nc.sync.dma_start(out=dst, in_=src)

              def fire_a2a(sc):
                  if no_collective:
                      nc.gpsimd.dma_start(
                          out=a2a_out[sc][:, :], in_=a2a_in[sc][:, :]
                      )
                  else:
                      nc.gpsimd.collective_compute(
                          "AllToAll",
                          mybir.AluOpType.bypass,
                          replica_groups=A2A_GROUP,
                          ins=[a2a_in[sc][:, :].opt()],
                          outs=[a2a_out[sc][:, :].opt()],
                      )

              # software pipeline: 2-head lookahead on A emits
              pend = []
              b_count = 0
              for sc in range(2):
                  for h in range(H):
                      pbuf = ppool.tile(
                          [128, 12 * 512], bf16, tag="p", name=f"pb{sc}{h}"
                      )
                      emit_A(sc, h, pbuf)
                      pend.append((sc, h, pbuf))
                      if len(pend) == 3:
                          s0, h0, pb0 = pend.pop(0)
                          emit_B(s0, h0, pb0)
                          b_count += 1
                          if b_count == 8:
                              fire_a2a(0)
              for s0, h0, pb0 in pend:
                  emit_B(s0, h0, pb0)
                  b_count += 1
              # sc0 recv tiles load INTO qT's region (dead after the last
              # score matmul; per-head range WAR lets slot i start right
              # after head i's last emit_A). gpsimd unparks from the A2A#1
              # completion wait exactly when this data is available, and
              # the A2A#2 trigger behind it is gated by staging anyway.
              for i in (0, 4, 1, 5, 2, 6, 3, 7):
                  nc.gpsimd.dma_start(
                      out=qT[:, ds(i * 1024, 1024)],
                      in_=a2a_out[0][ds(i * 128, 128), :],
                  )
              fire_a2a(1)

            # ---------------- phase 3: wo (y = oT_full.T @ woT) ----------
            # pass 1: d-chunks 0-2 (slabs resident from prefetch); sc0 first
            # (independent of A2A#2, its ~50us hides the collective), then
            # sc1. pass 2: d-chunk 3 with freshly streamed slabs, jointly.
            with tc.tile_pool(name="wo2", bufs=8) as wo2, \
                 tc.tile_pool(name="rtp", bufs=1) as rtp, \
                 tc.tile_pool(name="ypool", bufs=3) as ypool, \
                 tc.tile_pool(name="psW", bufs=6, space="PSUM") as psW, \
                 tc.tile_pool(name="psW2", bufs=2, space="PSUM") as psW2:
                rt1 = [
                    rtp.tile([128, 1024], bf16, tag=f"rt1_{i}",
                             name=f"rt1_{i}")
                    for i in range(8)
                ]
                # sc1 recv: gpsimd is parked right behind the A2A#2
                # completion wait; sync joins it (free after staging)
                for k, i in enumerate((0, 4, 1, 5, 2, 6, 3, 7)):
                    (nc.gpsimd if k % 2 == 0 else nc.sync).dma_start(
                        out=rt1[i][:, :], in_=a2a_out[1][ds(i * 128, 128), :]
                    )

                def stat_ap(sc, i, h):
                    # sc0 recv lives in qT's recycled region
                    if sc == 0:
                        return qT[:, ds(i * 1024 + h * 128, 128)]
                    return rt1[i][:, ds(h * 128, 128)]

                def ycopy(sc, b, dc, pw):
                    ys = ypool.tile([128, 512], bf16, tag="ys",
                                    name=f"ys{sc}{b}{dc}")
                    nc.scalar.copy(ys[:, :], pw[:, :])
                    nc.sync.dma_start(
                        out=out[ds(sc * 256 + b * 128, 128),
                                ds(dc * 512, 512)],
                        in_=ys[:, :],
                    )

                def wo_block(sc):
                    # psum[t, d] accumulated over all 32 k-tiles
                    pw = {
                        (b, dc): psW.tile([128, 512], fp32, tag="psW",
                                          name=f"pw{sc}{b}{dc}")
                        for b in range(2) for dc in range(3)
                    }
                    for kt in range(32):
                        ig, h = divmod(kt, 8)
                        for b in range(2):
                            stat = stat_ap(sc, b * 4 + ig, h)
                            for dc in range(3):
                                nc.tensor.matmul(
                                    pw[(b, dc)][:, :],
                                    stat,
                                    slab1[kt][:, ds(dc * 512, 512)],
                                    start=(kt == 0),
                                    stop=(kt == 31),
                                )
                    for b in range(2):
                        for dc in range(3):
                            ycopy(sc, b, dc, pw[(b, dc)])

                # pass-2-sc0 slabs for kt 0-15, emitted before the sc0 pass
                # so their transfers (on the then-idle scalar queue) finish
                # before the matmuls need them
                slab2a = {}
                for kt in range(8):
                    sl = wo2.tile([128, 512], bf16, tag="sl2",
                                  name=f"slab2a_{kt}")
                    nc.scalar.dma_start(
                        out=sl[:, :],
                        in_=P["wo_r"][:, ds(49152 + kt * 512, 512)],
                    )
                    slab2a[kt] = sl
                # pass 1 sc0 (resident slabs) — independent of A2A#2
                wo_block(0)
                # pass-2 sc0 for kt 0-15: more A2A#2-independent work, so a
                # slow collective (and the rt1 transfers) stay hidden; its
                # 2 psum chains live in their own pool and pause across
                # sc1-pass-1 (6 rotating + 2 held = 8 banks)
                pw2 = {
                    (0, b): psW2.tile([128, 512], fp32, tag="psW2",
                                      name=f"p2w0{b}")
                    for b in range(2)
                }
                for kt in range(8):
                    ig, h = divmod(kt, 8)
                    for b in range(2):
                        stat = stat_ap(0, b * 4 + ig, h)
                        nc.tensor.matmul(
                            pw2[(0, b)][:, :],
                            stat,
                            slab2a[kt][:, :],
                            start=(kt == 0),
                            stop=False,
                        )
                # pass 1 sc1 (resident slabs)
                wo_block(1)
                # tail: kt 16-31 for sc0's pass-2 chains + all of sc1's
                # pass 2; slab slots for kt 0-15 are re-streamed for sc1
                # (gpsimd/sync are parked behind the A2A#2 completion, so
                # none of this contends with the collective). The sc1
                # chains are created here so they rotate into wo_block(1)'s
                # freed banks, not into the still-open sc0 chains.
                for b in range(2):
                    pw2[(1, b)] = psW.tile([128, 512], fp32, tag="psW",
                                           name=f"p2w1{b}")
                for kt in range(8, 32):
                    sl = wo2.tile([128, 512], bf16, tag="sl2",
                                  name=f"slab2b_{kt}")
                    (nc.gpsimd if kt % 2 == 0 else nc.sync).dma_start(
                        out=sl[:, :],
                        in_=P["wo_r"][:, ds(49152 + kt * 512, 512)],
                    )
                    ig, h = divmod(kt, 8)
                    for b in range(2):
                        stat = stat_ap(0, b * 4 + ig, h)
                        nc.tensor.matmul(
                            pw2[(0, b)][:, :],
                            stat,
                            sl[:, :],
                            start=False,
                            stop=(kt == 31),
                        )
                    for b in range(2):
                        stat = stat_ap(1, b * 4 + ig, h)
                        nc.tensor.matmul(
                            pw2[(1, b)][:, :],
                            stat,
                            sl[:, :],
                            start=(kt == 8),
                            stop=False,
                        )
                for b in range(2):
                    ycopy(0, b, 3, pw2[(0, b)])
                for kt in range(8):
                    sl = wo2.tile([128, 512], bf16, tag="sl2",
                                  name=f"slab2c_{kt}")
                    (nc.gpsimd if kt % 2 == 0 else nc.sync).dma_start(
                        out=sl[:, :],
                        in_=P["wo_r"][:, ds(49152 + kt * 512, 512)],
                    )
                    ig, h = divmod(kt, 8)
                    for b in range(2):
                        stat = stat_ap(1, b * 4 + ig, h)
                        nc.tensor.matmul(
                            pw2[(1, b)][:, :],
                            stat,
                            sl[:, :],
                            start=False,
                            stop=(kt == 7),
                        )
                for b in range(2):
                    ycopy(1, b, 3, pw2[(1, b)])


def _prep_in_maps(x, freqs_cos, freqs_sin, mask, encoder_output, wq, wk, wv, wo):
    x = np.asarray(x, np.float32)
    encoder_output = np.asarray(encoder_output, np.float32)
    freqs_cos = np.asarray(freqs_cos, np.float32)
    freqs_sin = np.asarray(freqs_sin, np.float32)
    wq = np.asarray(wq, np.float32)
    wk = np.asarray(wk, np.float32)
    wv = np.asarray(wv, np.float32)
    wo = np.asarray(wo, np.float32)

    def perm(w):  # deinterleave rope pairs per head: even dims first
        w4 = w.reshape(H, 64, 2, D)
        return np.ascontiguousarray(w4.transpose(0, 2, 1, 3)).reshape(O, D)

    def slab256(wT):  # [D, O] -> [128, 4*32*256]: pass p, dt n, col c
        w4 = wT.reshape(NDT, 128, 4, 256)            # [n, part, p, c]
        return np.ascontiguousarray(
            w4.transpose(1, 2, 0, 3)
        ).reshape(128, NDT * O)

    def slab512(wT):  # [D, O] -> [128, 2*32*512]: oc, dt n, col c
        w4 = wT.reshape(NDT, 128, 2, 512)
        return np.ascontiguousarray(
            w4.transpose(1, 2, 0, 3)
        ).reshape(128, NDT * O)

    alpha = 1.0 / np.sqrt(DH)
    cosT = freqs_cos.T  # [64, S]
    sinT = freqs_sin.T
    csq_cos = (np.concatenate([cosT, cosT], 0) * alpha).astype(BF16)
    csq_sin = (np.concatenate([-sinT, sinT], 0) * alpha).astype(BF16)
    csk_cos = np.concatenate([cosT, cosT], 0).astype(BF16)
    csk_sin = np.concatenate([-sinT, sinT], 0).astype(BF16)

    # 4 diagonal-band keep-masks (0/1, applied post-exp):
    # dmask[t, j*512+s] = 0 if s < t + j*128 else 1
    t_i = np.arange(128)[:, None]
    s_i = np.arange(512)[None, :]
    dmask = np.concatenate(
        [np.where(s_i < t_i + j * 128, 0.0, 1.0) for j in range(4)], axis=1
    ).astype(BF16)
    ones = np.ones((128, 128), BF16)

    # woT slabs: full wo.T (k = head*128+dh on partitions per k-tile), this
    # core's d-half, split into two 1024-wide passes
    woT = np.ascontiguousarray(wo.T).reshape(32, 128, D)  # [kt, dh, dout]

    in_maps = []
    for c in range(8):
        g, r = divmod(c, 4)
        dhalf = g
        sl = slice(r * O, (r + 1) * O)
        xhat = np.concatenate([encoder_output[g], x[g]], axis=0)  # [T, D]
        xhatT = xhat.T.astype(BF16)                               # [D, T]
        x_r = np.ascontiguousarray(
            xhatT.reshape(NDT, 128, T).transpose(1, 0, 2)
        ).reshape(128, NDT * T)
        wqT = perm(wq[sl]).T.astype(BF16)   # [D, O]
        wkT = perm(wk[sl]).T.astype(BF16)
        wvT = wv[sl].T.astype(BF16)
        wo_c = woT[:, :, dhalf * DHALF:(dhalf + 1) * DHALF]  # [32,128,2048]
        wo_a = np.ascontiguousarray(
            wo_c[:, :, :1536].transpose(1, 0, 2)
        ).reshape(128, 32 * 1536)
        wo_b = np.ascontiguousarray(
            wo_c[:, :, 1536:].transpose(1, 0, 2)
        ).reshape(128, 32 * 512)
        wo_r = np.concatenate([wo_a, wo_b], axis=1).astype(BF16)
        in_maps.append(
            {
                "x_r": x_r,
                "wq_r": slab256(wqT),
                "wk_r": slab256(wkT),
                "wv_r": slab512(wvT),
                "wo_r": wo_r,
                "csq_cos": csq_cos,
                "csq_sin": csq_sin,
                "csk_cos": csk_cos,
                "csk_sin": csk_sin,
                "dmask": dmask,
                "ones": ones,
            }
        )
    return in_maps


def _gather(outs):
    full = np.zeros((2, S, D), np.float32)
    for c in range(8):
        g, q = divmod(c, 4)
        dhalf = g
        o = np.asarray(outs[c]).astype(np.float32)  # [512, 2048]
        for sc in range(2):
            for b in range(2):
                rows = o[sc * 256 + b * 128: sc * 256 + b * 128 + 128]
                full[b, sc * 512 + q * 128: sc * 512 + q * 128 + 128,
                     dhalf * DHALF:(dhalf + 1) * DHALF] = rows
    return full


def kernel(x, start_pos, freqs_cos, freqs_sin, mask, encoder_output, wq, wk, wv, wo):
    global LAST_EXEC_NS
    from concourse.bass_utils import run_bass_kernel_spmd

    if "nc" not in _CACHE:
        _CACHE["nc"] = _build()
    nc = _CACHE["nc"]

    in_maps = _prep_in_maps(
        x, freqs_cos, freqs_sin, mask, encoder_output, wq, wk, wv, wo
    )
    res = run_bass_kernel_spmd(nc, in_maps, core_ids=list(range(8)))
    LAST_EXEC_NS = res.exec_time_ns
    return _gather([res.results[c]["out"] for c in range(8)])
